# revision 1
# baseline (speedup 1.0000x reference)
"""Multi-head attention (B=2, S=2048, D=1024, H=16) on 8 TRN2 NeuronCores.

Sharding: core c handles batch b = c//4 and head-group g = c%4 (4 heads each).
Each core computes its heads' attention and a partial output projection
(row-parallel W_o); the host sums the 4 partials per batch and adds b_o.

Device-side layout trick: everything runs in the "transposed world".
The host passes x[b].T and mask.T, so the QK projection produces Q^T/K^T
directly (no on-device transposes), scores are computed as S^T = K·Q^T with
keys on partitions, softmax denominators come from a ones-column appended to
V, and the output projection produces out^T which the host transposes back.

Mask specialization: the host classifies each [128 x 512] tile of mask^T as
fully-masked (<= -1e8 everywhere: the whole tile contributes exp() == 0 and
is skipped), exactly zero (no mask add needed), or partial (mask added on
DVE). The program is built (and cached) per observed pattern, so any additive
mask is handled correctly; a causal mask skips ~45% of the attention work.
"""

import numpy as np

import concourse.bass as bass  # noqa: F401
import concourse.mybir as mybir
import concourse.tile as tile
from concourse import bacc
from concourse.bass import ds, ts
from concourse.bass_utils import run_bass_kernel_spmd

B, S, D, H = 2, 2048, 1024, 16
HD = D // H  # 64
HPC = 4      # heads per core
NCORES = 8
F32R = mybir.dt.float32r
F32 = mybir.dt.float32
AF = mybir.ActivationFunctionType
ADD = mybir.AluOpType.add
MULT = mybir.AluOpType.mult

SKIP, FULL, PART = 0, 1, 2

_CACHE = {}


def _classify_mask(maskT):
    """pattern[hf][kb][nn] for [128, 512] tiles of mask^T (k major, q minor)."""
    pat = []
    for hf in range(2):
        rows = []
        for kb in range(16):
            ents = []
            for nn in range(2):
                blk = maskT[kb * 128 : (kb + 1) * 128,
                            hf * 1024 + nn * 512 : hf * 1024 + (nn + 1) * 512]
                if np.all(blk <= -1e8):
                    ents.append(SKIP)
                elif np.all(blk == 0.0):
                    ents.append(FULL)
                else:
                    ents.append(PART)
            rows.append(tuple(ents))
        pat.append(tuple(rows))
    return tuple(tuple(r) for r in pat)


def _build(pattern):
    nc = bacc.Bacc(None, target_bir_lowering=False, debug=False)
    xT = nc.dram_tensor("xT", [D, S], F32R, kind="ExternalInput")
    wqk = nc.dram_tensor("wqk", [D, 512], F32R, kind="ExternalInput")
    bqk = nc.dram_tensor("bqk", [128, 4], F32R, kind="ExternalInput")
    wv = nc.dram_tensor("wv", [D, 256], F32R, kind="ExternalInput")
    bv = nc.dram_tensor("bv", [1, 256], F32R, kind="ExternalInput")
    wo = nc.dram_tensor("wo", [256, D], F32R, kind="ExternalInput")
    maskT = nc.dram_tensor("maskT", [S, S], F32R, kind="ExternalInput")
    outT = nc.dram_tensor("outT", [D, S], F32, kind="ExternalOutput")

    with tile.TileContext(nc) as tc:
        with (
            tc.tile_pool(name="big", bufs=1) as bigp,
            tc.tile_pool(name="wqkp", bufs=1) as wqkp,
            tc.tile_pool(name="wvp", bufs=1) as wvp,
            tc.tile_pool(name="wop", bufs=1) as wop,
            tc.tile_pool(name="qkp", bufs=1) as qkp,
            tc.tile_pool(name="vp", bufs=1) as vp,
            tc.tile_pool(name="valsp", bufs=1) as valsp,
            tc.tile_pool(name="attnp", bufs=3) as attnp,
            tc.tile_pool(name="maskp", bufs=1) as maskp,
            tc.tile_pool(name="smallp", bufs=1) as smallp,
            tc.tile_pool(name="constp", bufs=1) as constp,
        ):
            ones_t = constp.tile([1, 128], F32R)
            nc.gpsimd.memset(ones_t[:].bitcast(F32), 1.0)
            b_sb = constp.tile([128, 4], F32R)
            nc.sync.dma_start(b_sb[:], bqk[:])
            bv_sb = constp.tile([1, 256], F32R)
            nc.sync.dma_start(bv_sb[:], bv[:])

            # s-half-major DMA: all of x^T's first s-half lands in ~half the
            # time, so half-0 projections and the first q-half's attention
            # can start much earlier
            wqk_sb = wqkp.tile([128, 8, 512], F32R, tag="wqkslot")
            xt_sb = bigp.tile([128, 8, S], F32R, tag="big")
            wv_sb = wvp.tile([128, 8, 256], F32R)
            for dc in range(8):
                nc.sync.dma_start(wqk_sb[:, dc, :], wqk[ds(dc * 128, 128), :])
                nc.sync.dma_start(
                    xt_sb[:, dc, ds(0, 1024)], xT[ds(dc * 128, 128), ds(0, 1024)]
                )
            nc.sync.dma_start(wv_sb[:], wv[:].rearrange("(dc p) c -> p dc c", p=128))

            # qk_sb rows (partition+chunk) = projected qkv column:
            # chunk 0: q of heads 0,1; chunk 1: q of heads 2,3;
            # chunk 2: k of heads 0,1; chunk 3: k of heads 2,3.
            qk_sb = qkp.tile([128, 4, S], F32R)
            # v_sb[s%128, s//128, h, 0:64] = V; [..., 64] = 1.0 (denominator col)
            v_sb = vp.tile([128, 16, HPC, 65], F32R)
            
            nc.gpsimd.memset(v_sb[:, :, :, 64:65].bitcast(F32), 1.0)

            # mask classification; the first-processed half's mask tiles are
            # DMA'd inside the front stream so its PART adds never stall
            def n_full(hf):
                return sum(c == FULL for kb in pattern[hf] for c in kb)

            hf_order = sorted(range(2), key=lambda hf: n_full(hf))
            part_blocks, slots, mask_tiles = {}, {}, {}
            for hf_i, hf in enumerate(hf_order):
                pb = [
                    (kb, nn)
                    for kb in range(16) for nn in range(2)
                    if pattern[hf][kb][nn] == PART
                ]
                part_blocks[hf] = pb
                slots[hf] = {blk: i for i, blk in enumerate(pb)}
                if len(pb) == 0:
                    mask_tiles[hf] = None
                elif len(pb) <= 8:
                    if hf_i == 0:
                        mask_tiles[hf] = maskp.tile(
                            [128, 8, 512], F32R, tag="mask", name=f"mask{hf}")
                    else:
                        mask_tiles[hf] = wqkp.tile(
                            [128, 8, 512], F32R, tag="wqkslot", name=f"mask{hf}")
                else:
                    mask_tiles[hf] = bigp.tile(
                        [128, 32, 512], F32R, tag="big", name=f"mask{hf}")

            def mask_dmas(hf, qs_):
                pb = part_blocks[hf]
                i = 0
                while i < len(pb):
                    kb0, nn0 = pb[i]
                    j = i + 1
                    while (j < len(pb)
                           and pb[j] == (pb[j - 1][0] + 1, nn0)):
                        j += 1
                    n = j - i
                    nc.sync.dma_start(
                        mask_tiles[hf][:, i : i + n, :],
                        maskT[ds(kb0 * 128, n * 128),
                              ds(qs_ + nn0 * 512, 512)].rearrange(
                            "(b p) q -> p b q", p=128),
                    )
                    i = j
            hf_first = hf_order[0]
            if mask_tiles.get(hf_first) is not None and len(part_blocks[hf_first]) <= 8:
                mask_dmas(hf_first, hf_first * 1024)
            # second s-half of x^T + wo
            for dc in range(8):
                nc.sync.dma_start(
                    xt_sb[:, dc, ds(1024, 1024)],
                    xT[ds(dc * 128, 128), ds(1024, 1024)],
                )
            wo_sb = wop.tile([128, 2, D], F32R)
            nc.sync.dma_start(wo_sb[:], wo[:].rearrange("(kc p) d -> p kc d", p=128))

            # ---- projections: QK chunks 0,2 first (unblocks heads 0/1),
            # then V, then QK chunks 1,3 (heads 2/3). qkT[c,s] = W^T x^T.
            def qk_chunk_half(pool, cc, sh):
                ps = pool.tile([128, 1024], F32, tag="psqk", name=f"qkps{cc}{sh}")
                for dc in range(8):
                    lhsT = wqk_sb[:, dc, ts(cc, 128)]
                    for nn in range(2):
                        nc.tensor.matmul(
                            ps[:, ts(nn, 512)], lhsT,
                            xt_sb[:, dc, ds(sh * 1024 + nn * 512, 512)],
                            start=(dc == 0), stop=(dc == 7),
                        )
                nc.scalar.activation(
                    qk_sb[:, cc, ds(sh * 1024, 1024)], ps[:], AF.Identity,
                    bias=b_sb[:, cc : cc + 1],
                )

            def v_blocks(pool, rng):
                for sb_i in rng:
                    ps = pool.tile([128, 256], F32, tag="psv", name=f"vps{sb_i}")
                    for dc in range(8):
                        nc.tensor.matmul(
                            ps[:], xt_sb[:, dc, ts(sb_i, 128)], wv_sb[:, dc, :],
                            start=(dc == 0), stop=False,
                        )
                    # += ones[s] * bv  (rank-1 bias add)
                    nc.tensor.matmul(ps[:], ones_t[:], bv_sb[:], start=False, stop=True)
                    nc.vector.tensor_copy(
                        v_sb[:, sb_i, :, 0:64],
                        ps[:].rearrange("p (h e) -> p h e", h=HPC),
                    )

            # half-0 projections first (q-half-0 attention unblocks early),
            # then half-1
            with (
                tc.tile_pool(name="psqk", bufs=2, space="PSUM") as psqk,
                tc.tile_pool(name="psv", bufs=2, space="PSUM") as psv,
            ):
                for cc in (0, 2, 1, 3):
                    qk_chunk_half(psqk, cc, 0)
                v_blocks(psv, range(8))
                for cc in (0, 2, 1, 3):
                    qk_chunk_half(psqk, cc, 1)
                v_blocks(psv, range(8, 16))

            # ---- attention, in [k, q] layout, q processed in two halves ----
            # (mask tiles + first-half mask DMAs hoisted into the front stream)
            with (
                tc.tile_pool(name="pssc", bufs=3, space="PSUM") as pssc,
                tc.tile_pool(name="psav", bufs=1, space="PSUM") as psav,
            ):
                def outproj(vals_t, qs_):
                    for ob in range(8):
                        ps = pssc.tile([128, 1024], F32, tag="pssc")
                        for kc in range(2):
                            lhsT = wo_sb[:, kc, ts(ob, 128)]
                            for nn in range(2):
                                nc.tensor.matmul(
                                    ps[:, ts(nn, 512)], lhsT,
                                    vals_t[:, kc, ts(nn, 512)],
                                    start=(kc == 0), stop=(kc == 1),
                                )
                        oev = attnp.tile([128, 1024], F32, tag="attn")
                        nc.scalar.activation(oev[:], ps[:], AF.Copy)
                        nc.sync.dma_start(
                            outT[ds(ob * 128, 128), ds(qs_, 1024)], oev[:]
                        )

                pending = None
                for hf_i, hf in enumerate(hf_order):
                    qs = hf * 1024
                    slot = slots[hf]
                    mask_sb = mask_tiles[hf]
                    if mask_sb is not None and (
                        hf_i > 0 or len(part_blocks[hf]) > 8
                    ):
                        mask_dmas(hf, qs)
                    vals_sb = valsp.tile(
                        [128, 2, 1024], F32R, tag="vals", name=f"vals{hf}")
                    kb_order = sorted(
                        (kb for kb in range(16) if pattern[hf][kb] != (SKIP, SKIP)),
                        key=lambda kb: (PART in pattern[hf][kb], kb),
                    )
                    for h in range(HPC):
                        off = 64 * (h % 2)
                        qt = qk_sb[off : off + 64, h // 2, :]
                        kt = qk_sb[off : off + 64, 2 + h // 2, :]
                        act_kbs = [
                            [kb for kb in kb_order if pattern[hf][kb][nn] != SKIP]
                            for nn in range(2)
                        ]
                        ps_av = psav.tile([65, 1024], F32, tag="psav")
                        for kb in kb_order:
                            cls = pattern[hf][kb]
                            ps_sc = pssc.tile([128, 1024], F32, tag="pssc")
                            lhsT = kt[:, ts(kb, 128)]
                            at = attnp.tile([128, 1024], F32R, tag="attn")
                            for nn in range(2):
                                if cls[nn] == SKIP:
                                    continue
                                nc.tensor.matmul(
                                    ps_sc[:, ts(nn, 512)], lhsT,
                                    qt[:, ds(qs + nn * 512, 512)],
                                    start=True, stop=True,
                                )
                            if cls == (FULL, FULL):
                                nc.scalar.activation(at[:], ps_sc[:], AF.Exp)
                            else:
                                for nn in range(2):
                                    if cls[nn] == SKIP:
                                        continue
                                    if cls[nn] == PART:
                                        nc.vector.tensor_tensor(
                                            at[:, ts(nn, 512)], ps_sc[:, ts(nn, 512)],
                                            mask_sb[:, slot[(kb, nn)], :], ADD,
                                        )
                                        nc.scalar.activation(
                                            at[:, ts(nn, 512)], at[:, ts(nn, 512)],
                                            AF.Exp,
                                        )
                                    else:
                                        nc.scalar.activation(
                                            at[:, ts(nn, 512)], ps_sc[:, ts(nn, 512)],
                                            AF.Exp,
                                        )
                            for nn in range(2):
                                if cls[nn] == SKIP:
                                    continue
                                nc.tensor.matmul(
                                    ps_av[:, ts(nn, 512)], v_sb[:, kb, h, :],
                                    at[:, ts(nn, 512)],
                                    start=(kb == act_kbs[nn][0]),
                                    stop=(kb == act_kbs[nn][-1]),
                                )
                        # normalize: vals = av[0:64] * (1 / av[64])
                        recip = smallp.tile([1, 1024], F32R, tag="recip")
                        with nc.allow_low_precision(
                            reason="float32r has fp32 bits; only PE matmul mode differs"
                        ):
                            nc.vector.reciprocal(recip[:], ps_av[64:65, :])
                        bc_sb = smallp.tile([64, 1024], F32R, tag="bc")
                        nc.gpsimd.partition_broadcast(bc_sb[:], recip[:])
                        nc.vector.tensor_tensor(
                            vals_sb[off : off + 64, h // 2, :],
                            ps_av[0:64, :], bc_sb[:], MULT,
                        )
                        if pending is not None and hf_i == 1 and h == 0:
                            outproj(*pending)
                            pending = None

                    # partial output projection for this q-half; the first
                    # half's is deferred into the second half's first head
                    # sweep so it fills gaps instead of competing at the
                    # transition (execution follows deps, not emission order)
                    if hf_i == 0 and len(hf_order) > 1:
                        pending = (vals_sb, qs)
                    else:
                        outproj(vals_sb, qs)
                if pending is not None:
                    outproj(*pending)
                    pending = None

    nc.compile()
    return nc


def _prep_inputs(x, mask, W_qkv, b_qkv, W_o, b_o):
    """Host-side sharding/layout prep: slices, transposes, 1/sqrt(HD) folding."""
    scale = np.float32(1.0 / np.sqrt(HD))
    xT = [np.ascontiguousarray(x[b].T) for b in range(B)]
    maskT = np.ascontiguousarray(mask.T)
    in_maps = []
    for c in range(NCORES):
        b, g = divmod(c, HPC)
        heads = [HPC * g + i for i in range(HPC)]
        qcols = np.concatenate(
            [W_qkv[:, 192 * h : 192 * h + 64] for h in heads], axis=1) * scale
        kcols = np.concatenate(
            [W_qkv[:, 192 * h + 64 : 192 * h + 128] for h in heads], axis=1)
        wqk = np.ascontiguousarray(np.concatenate([qcols, kcols], axis=1))
        bq = np.concatenate([b_qkv[192 * h : 192 * h + 64] for h in heads]) * scale
        bk = np.concatenate([b_qkv[192 * h + 64 : 192 * h + 128] for h in heads])
        bqk_t = np.ascontiguousarray(
            np.concatenate([bq, bk]).reshape(4, 128).T)
        wv = np.ascontiguousarray(np.concatenate(
            [W_qkv[:, 192 * h + 128 : 192 * h + 192] for h in heads], axis=1))
        bv = np.ascontiguousarray(np.concatenate(
            [b_qkv[192 * h + 128 : 192 * h + 192] for h in heads])[None, :])
        wo = np.ascontiguousarray(W_o[256 * g : 256 * (g + 1), :])
        in_maps.append({
            "xT": xT[b], "wqk": wqk, "bqk": bqk_t, "wv": wv, "bv": bv,
            "wo": wo, "maskT": maskT,
        })
    return in_maps


def kernel(x, mask, W_qkv, b_qkv, W_o, b_o):
    x = np.asarray(x, dtype=np.float32)
    mask = np.asarray(mask, dtype=np.float32)
    W_qkv = np.asarray(W_qkv, dtype=np.float32)
    b_qkv = np.asarray(b_qkv, dtype=np.float32)
    W_o = np.asarray(W_o, dtype=np.float32)
    b_o = np.asarray(b_o, dtype=np.float32)

    pattern = _classify_mask(np.ascontiguousarray(mask.T))
    key = ("nc", pattern)
    if key not in _CACHE:
        _CACHE[key] = _build(pattern)
    nc = _CACHE[key]
    _CACHE["nc"] = nc

    in_maps = _prep_inputs(x, mask, W_qkv, b_qkv, W_o, b_o)
    res = run_bass_kernel_spmd(nc, in_maps, core_ids=list(range(NCORES)))
    _CACHE["last_result"] = res

    out = np.empty((B, S, D), dtype=np.float32)
    for b in range(B):
        acc = res.results[HPC * b]["outT"].astype(np.float32)
        for g in range(1, HPC):
            acc = acc + res.results[HPC * b + g]["outT"]
        out[b] = acc.T + b_o
    return out



# revision 62
# speedup vs baseline: 1.4429x; 1.4429x over previous
"""Multi-head attention (B=2, S=2048, D=1024, H=16) on 8 TRN2 NeuronCores.

Sharding: core c handles batch b = c//4 and head-group g = c%4 (4 heads each).
Each core computes its heads' attention and a partial output projection
(row-parallel W_o); the host sums the 4 partials per batch and adds b_o.

Fast path (causal mask): bf16 operands off-PSUM, live-span score matmuls
(fully-masked columns never computed), masking via affine_select on the one
diagonal 128x128 block per score tile (no mask tensor at all), and a
transposed AV: attention tiles act as the stationary operand so each
accumulation step emits only 65 output columns ([q,64] values + a ones-column
denominator that lands as a per-partition scalar). vals are transposed back
with PE-transpose against an on-device identity for the row-parallel output
projection. DMAs are batched and spread across the SP/DVE/Pool queues.

A generic additive-mask fallback (the previous kernel) is kept for non-causal
masks.
"""

import numpy as np
import ml_dtypes

import concourse.bass as bass  # noqa: F401
import concourse.mybir as mybir
import concourse.tile as tile
from concourse import bacc
from concourse.bass import ds, ts
from concourse.bass_utils import run_bass_kernel_spmd

B, S, D, H = 2, 2048, 1024, 16
HD = D // H  # 64
HPC = 4      # heads per core
NCORES = 8
F32R = mybir.dt.float32r
F32 = mybir.dt.float32
BF16 = mybir.dt.bfloat16
NPBF16 = ml_dtypes.bfloat16
AF = mybir.ActivationFunctionType
ADD = mybir.AluOpType.add
MULT = mybir.AluOpType.mult

SKIP, FULL, PART = 0, 1, 2

_CACHE = {}


def _is_causal(mask):
    tril = np.tril(np.ones((S, S), dtype=bool))
    return bool(np.all(mask[tril] == 0.0) and np.all(mask[~tril] <= -1e8))


# ---------------------------------------------------------------------------
# Causal fast path
# ---------------------------------------------------------------------------

def _build_causal(dump=False):
    nc = bacc.Bacc(None, target_bir_lowering=False, debug=False)
    xT = nc.dram_tensor("xT", [D, S], BF16, kind="ExternalInput")
    # wqk host-prepacked per chunk: wqk[cc][p, dc*128 + c] = W[dc*128+p, cc*128+c]
    wqk = nc.dram_tensor("wqk", [4, 128, 1024], BF16, kind="ExternalInput")
    bqk = nc.dram_tensor("bqk", [128, 4], F32, kind="ExternalInput")
    wv = nc.dram_tensor("wv", [D, 256], BF16, kind="ExternalInput")
    bv = nc.dram_tensor("bv", [1, 256], BF16, kind="ExternalInput")
    wo = nc.dram_tensor("wo", [256, D], BF16, kind="ExternalInput")
    outT = nc.dram_tensor("outT", [D, S], BF16, kind="ExternalOutput")
    if dump:
        d_qk = nc.dram_tensor("d_qk", [128, 4, S], BF16, kind="ExternalOutput")
        d_v = nc.dram_tensor("d_v", [128, 16, HPC, 65], BF16, kind="ExternalOutput")
        d_vals = nc.dram_tensor(
            "d_vals", [128, 16, HPC, 64], BF16, kind="ExternalOutput")
        d_valsT = nc.dram_tensor("d_valsT", [128, 2, S], BF16, kind="ExternalOutput")
        d_at = nc.dram_tensor("d_at", [128, 16, 1024], BF16, kind="ExternalOutput")
        d_den = nc.dram_tensor("d_den", [128, 16, 65], F32, kind="ExternalOutput")

    with tile.TileContext(nc) as tc:
        with (
            tc.tile_pool(name="bigp", bufs=1) as bigp,
            tc.tile_pool(name="constp", bufs=1) as constp,
            tc.tile_pool(name="atp", bufs=18) as atp,
            tc.tile_pool(name="outp", bufs=2) as outp,
            tc.tile_pool(name="smallp", bufs=4) as smallp,
            tc.tile_pool(name="psBig", bufs=3, space="PSUM") as psBig,
            tc.tile_pool(name="psC", bufs=2, space="PSUM") as psC,
        ):
            # --- constants (small DMAs issue before the memset chain so
            # they hit the DMA device first) ---------------------------------
            b_sb = constp.tile([128, 4], F32)
            nc.gpsimd.dma_start(b_sb[:], bqk[:])
            bv_sb = constp.tile([1, 256], BF16)
            nc.gpsimd.dma_start(bv_sb[:], bv[:])
            ones_t = constp.tile([1, 128], BF16)
            nc.gpsimd.memset(ones_t[:], 1.0)
            ident = constp.tile([128, 128], BF16)
            nc.gpsimd.memset(ident[:], 1.0)
            nc.gpsimd.affine_select(
                out=ident[:], in_=ident[:],
                compare_op=mybir.AluOpType.is_equal, fill=0.0,
                base=0, pattern=[[-1, 128]], channel_multiplier=1,
            )
            # upper-triangular (incl. diagonal) ones: keeps k <= q when
            # multiplied into the diagonal block of an exp'd score tile
            tri_t = constp.tile([128, 128], BF16)
            nc.gpsimd.memset(tri_t[:], 1.0)
            nc.gpsimd.affine_select(
                out=tri_t[:], in_=tri_t[:],
                compare_op=mybir.AluOpType.is_ge, fill=0.0,
                base=0, pattern=[[1, 128]], channel_multiplier=-1,
            )

            # --- big SBUF tensors -----------------------------------------
            wqk_sb = constp.tile([128, 8, 512], BF16)
            xt_sb = bigp.tile([128, 8, S], BF16)
            wv_sb = constp.tile([128, 8, 256], BF16)
            wo_sb = constp.tile([128, 2, D], BF16)
            qk_sb = bigp.tile([128, 4, S], BF16)
            v_sb = bigp.tile([128, 16, HPC, 65], BF16)
            nc.gpsimd.memset(v_sb[:, :, :, 64:65], 1.0)
            vals_sb = bigp.tile([128, 16, HPC, 64], BF16)
            valsT_sb = bigp.tile([128, 2, S], BF16)

            # --- input DMAs ------------------------------------------------
            # wqk on the ACT queue (idle until the first exp); chunks are
            # host-prepacked contiguous so descriptors stay 2KB
            for cc in range(2):
                nc.scalar.dma_start(
                    wqk_sb[:, :, ts(cc, 128)],
                    wqk[cc].rearrange("p (dc c) -> p dc c", c=128),
                )
            nc.scalar.dma_start(
                wv_sb[:], wv[:].rearrange("(dc p) c -> p dc c", p=128))
            for cc in range(2, 4):
                nc.scalar.dma_start(
                    wqk_sb[:, :, ts(cc, 128)],
                    wqk[cc].rearrange("p (dc c) -> p dc c", c=128),
                )
            # xT + wv + wo on the SP queue: s-half-major then dc-chunks;
            # the first chunk is small so the first projection matmuls can
            # start as early as possible
            for d0, nd in ((0, 2), (2, 4), (6, 2)):
                nc.sync.dma_start(
                    xt_sb[:, ds(d0, nd), ds(0, 1024)],
                    xT[ds(d0 * 128, nd * 128), ds(0, 1024)].rearrange(
                        "(dc p) s -> p dc s", p=128),
                )
            for dh in range(2):
                nc.sync.dma_start(
                    xt_sb[:, ds(dh * 4, 4), ds(1024, 1024)],
                    xT[ds(dh * 512, 512), ds(1024, 1024)].rearrange(
                        "(dc p) s -> p dc s", p=128),
                )
            nc.sync.dma_start(
                wo_sb[:], wo[:].rearrange("(kc p) d -> p kc d", p=128))

            # --- projection helpers ---------------------------------------
            def qk_chunk(cc, sh):
                """qk_sb[:, cc, sh-half] = wqk-chunk-cc^T @ xT-half + bias."""
                ps = psBig.tile([128, 1024], F32, tag="psB", name=f"qkps{cc}{sh}")
                for dc in range(8):
                    lhsT = wqk_sb[:, dc, ts(cc, 128)]
                    for nn in range(2):
                        nc.tensor.matmul(
                            ps[:, ts(nn, 512)], lhsT,
                            xt_sb[:, dc, ds(sh * 1024 + nn * 512, 512)],
                            start=(dc == 0), stop=(dc == 7),
                        )
                nc.vector.tensor_scalar(
                    qk_sb[:, cc, ds(sh * 1024, 1024)], ps[:],
                    b_sb[:, cc : cc + 1], None, ADD,
                )

            def v_blocks(rng):
                for sb_i in rng:
                    ps = psBig.tile([128, 256], F32, tag="psB", name=f"vps{sb_i}")
                    for dc in range(8):
                        nc.tensor.matmul(
                            ps[:], xt_sb[:, dc, ts(sb_i, 128)], wv_sb[:, dc, :],
                            start=(dc == 0), stop=False,
                        )
                    nc.tensor.matmul(ps[:], ones_t[:], bv_sb[:], start=False, stop=True)
                    nc.vector.tensor_copy(
                        v_sb[:, sb_i, :, 0:64],
                        ps[:].rearrange("p (h e) -> p h e", h=HPC),
                    )

            # --- attention for one (head, q-half) -------------------------
            # Filler machinery: the engines execute in-order, so PE starvation
            # during exp-bound attention stretches can only be avoided by
            # interleaving independent PE work (projections, outproj) into
            # the kb loops at fine granularity, paced by the ACT/PE balance.
            filler_q = []   # list of [name, pe_cost_ns, fn, deadline|None]
            balance = [0.0]

            def pop_fillers(here=None):
                # deadline-forced pops (emit everything up to the overdue
                # entry to preserve dependency order), then balance pops
                if here is not None:
                    h, hf, kb = here
                    due = None
                    for i, ent in enumerate(filler_q):
                        dl = ent[3]
                        if dl is not None and dl[0] == h and dl[1] == hf \
                                and kb >= dl[2]:
                            due = i
                    if due is not None:
                        for ent in filler_q[: due + 1]:
                            ent[2]()
                            balance[0] -= ent[1]
                        del filler_q[: due + 1]
                while filler_q and balance[0] >= filler_q[0][1] * 0.4:
                    name, cost, fn, _ = filler_q.pop(0)
                    fn()
                    balance[0] -= cost

            def drain_until(name):
                while filler_q:
                    nm, cost, fn, _ = filler_q.pop(0)
                    fn()
                    balance[0] -= cost
                    if nm == name:
                        break

            def drain_all():
                while filler_q:
                    _, cost, fn, _ = filler_q.pop(0)
                    fn()
                    balance[0] -= cost

            def attn_head(h, hf, pending_tail=None, av_delay=2):
                qs = hf * 1024
                off = 64 * (h % 2)
                qt = qk_sb[off : off + 64, 2 * (h // 2), :]
                kt = qk_sb[off : off + 64, 2 * (h // 2) + 1, :]
                nkb = 8 if hf == 0 else 16
                ats = []

                def av_group(qb_l):
                    # AV, qb-major: at-tiles are the stationary operand, so
                    # each accumulation step emits only 65 output columns; the
                    # ones column of V lands the denominator per-partition.
                    qb_g = hf * 8 + qb_l
                    pv = psC.tile([128, 65], F32, tag="psC", name=f"pav{h}{hf}{qb_l}")
                    for kb2 in range(qb_g + 1):
                        nc.tensor.matmul(
                            pv[:], ats[kb2][:, ts(qb_l, 128)], v_sb[:, kb2, h, :],
                            start=(kb2 == 0), stop=(kb2 == qb_g),
                        )
                    if dump and h == 0:
                        dtmp = smallp.tile([128, 65], F32, tag="dtmp",
                                           name=f"dd{hf}{qb_l}")
                        nc.vector.tensor_copy(dtmp[:], pv[:])
                        nc.sync.dma_start(d_den[:, qb_g, :], dtmp[:])
                    recip = smallp.tile([128, 1], F32, tag="recip")
                    nc.vector.reciprocal(recip[:], pv[:, 64:65])
                    nc.vector.tensor_scalar(
                        vals_sb[:, qb_g, h, :], pv[:, 0:64],
                        recip[:, 0:1], None, MULT,
                    )

                for kb in range(nkb):
                    if kb >= 1:
                        pop_fillers(here=(h, hf, kb))
                    lo = max(0, kb * 128 - qs)
                    if lo < 512:
                        ps = psBig.tile([128, 1024], F32, tag="psB",
                                        name=f"sc{h}{hf}{kb}")
                        base = 0
                    else:
                        ps = psBig.tile([128, 512], F32, tag="psB",
                                          name=f"sc{h}{hf}{kb}")
                        base = 512
                    lhsT = kt[:, ts(kb, 128)]
                    for seg in range(2):
                        a = max(lo, seg * 512)
                        bnd = (seg + 1) * 512
                        if a < bnd:
                            nc.tensor.matmul(
                                ps[:, ds(a - base, bnd - a)], lhsT,
                                qt[:, ds(qs + a, bnd - a)],
                                start=True, stop=True,
                            )
                    at = atp.tile([128, 1024], BF16, tag="at")
                    nc.scalar.activation(
                        at[:, ds(lo, 1024 - lo)], ps[:, ds(lo - base, 1024 - lo)],
                        AF.Exp)
                    if kb * 128 >= qs:
                        dlo = kb * 128 - qs
                        nc.gpsimd.tensor_tensor(
                            at[:, ds(dlo, 128)], at[:, ds(dlo, 128)],
                            tri_t[:], MULT,
                        )
                    if dump and h == 0:
                        nc.sync.dma_start(
                            d_at[:, hf * 8 + kb if hf == 0 else kb,
                                 ds(lo, 1024 - lo)],
                            at[:, ds(lo, 1024 - lo)])
                    ats.append(at)
                    if kb == 1 and pending_tail is not None:
                        pending_tail()
                    # AV groups run a couple of kbs behind so the
                    # scores->exp->mask chain of the diagonal tile never
                    # stalls the PE queue
                    pe_rows = 1024 - lo
                    if kb - hf * 8 - av_delay >= 0:
                        av_group(kb - hf * 8 - av_delay)
                        pe_rows += 65 * (kb - av_delay + 1)
                    balance[0] += ((1024 - lo) * 0.833 + 185
                                   - pe_rows * 0.4167 - 150)

                def tail():
                    for qb_l in range(8 - av_delay, 8):
                        av_group(qb_l)
                return tail

            # --- vals transpose + output projection for one q-half --------
            def transposes_half(hf, qbs=range(8), hps=(0, 1)):
                qs = hf * 1024
                for qb_l in qbs:
                    qb_g = hf * 8 + qb_l
                    for hp in hps:
                        pst = psBig.tile(
                            [128, 128], BF16, tag="psB", name=f"tr{hf}{qb_l}{hp}")
                        nc.tensor.transpose(
                            pst[:], vals_sb[:, qb_g, ds(hp * 2, 2), :], ident[:])
                        nc.vector.tensor_copy(
                            valsT_sb[:, hp, ds(qs + qb_l * 128, 128)], pst[:])

            _ot_state = {}

            def outproj_ob(hf, ob):
                qs = hf * 1024
                obg, obi = divmod(ob, 4)
                if (hf, obg) not in _ot_state:
                    _ot_state[(hf, obg)] = outp.tile(
                        [128, 4, 1024], BF16, tag="out", name=f"ot{hf}{obg}")
                ot = _ot_state[(hf, obg)]
                ps = psBig.tile([128, 1024], F32, tag="psB", name=f"op{hf}{ob}")
                for kc in range(2):
                    lhsT = wo_sb[:, kc, ts(ob, 128)]
                    for nn in range(2):
                        nc.tensor.matmul(
                            ps[:, ts(nn, 512)], lhsT,
                            valsT_sb[:, kc, ds(qs + nn * 512, 512)],
                            start=(kc == 0), stop=(kc == 1),
                        )
                # Pool cannot read PSUM (walrus restriction): hf0 copies go
                # on DVE; hf1 copies (tail, ACT idle) alternate DVE/ACT
                if hf == 1 and obi % 2 == 0:
                    nc.scalar.activation(ot[:, obi, :], ps[:], AF.Copy)
                else:
                    nc.vector.tensor_copy(ot[:, obi, :], ps[:])
                if obi % 2 == 1:
                    dma_eng = nc.sync if obg % 2 == 0 else nc.scalar
                    dma_eng.dma_start(
                        outT[ds(obg * 512 + (obi - 1) * 128, 256),
                             ds(qs, 1024)].rearrange(
                            "(ob p) q -> p ob q", p=128),
                        ot[:, ds(obi - 1, 2), :],
                    )

            def outproj_half(hf):
                for ob in range(8):
                    outproj_ob(hf, ob)

            def qk_unit(cc, sh, h, hf, kb):
                """A projection chunk as one deadline-pinned filler unit
                (the open psum accumulation group cannot be interleaved with
                other pool allocations, so the chunk stays monolithic)."""
                return [f"qk{cc}{sh}", 10 ** 6,
                        (lambda: qk_chunk(cc, sh)), (h, hf, kb)]

            # --- emission order: everything that is not on the critical
            # dependency prefix is deadline-pinned into the attention kb
            # loops so the in-order PE queue never runs dry.
            BIGC = 10 ** 6
            qk_chunk(0, 0)
            qk_chunk(1, 0)
            v_blocks(range(8))
            filler_q.append(qk_unit(2, 0, 0, 0, 3))
            filler_q.append(qk_unit(3, 0, 1, 0, 2))
            t = attn_head(0, 0)
            filler_q.append(qk_unit(0, 1, 2, 0, 2))
            filler_q.append(qk_unit(1, 1, 3, 0, 2))
            t = attn_head(1, 0, t)
            t = attn_head(2, 0, t)
            t = attn_head(3, 0, t)
            drain_all()
            filler_q.extend(
                [
                    [f"V{sb_i}", BIGC,
                     (lambda sb_i=sb_i: v_blocks([sb_i])),
                     (0, 1, sb_i - 6)]
                    for sb_i in range(8, 16)
                ]
                + [
                    ["tr0a", BIGC, lambda: transposes_half(0, qbs=range(0, 3)),
                     (1, 1, 2)],
                    ["tr0b", BIGC, lambda: transposes_half(0, qbs=range(3, 6)),
                     (1, 1, 3)],
                    ["tr0c", BIGC, lambda: transposes_half(0, qbs=range(6, 8)),
                     (1, 1, 4)],
                ]
                + [qk_unit(2, 1, 1, 1, 8)]
            )
            t = attn_head(0, 1, t)
            t = attn_head(1, 1, t)
            drain_all()
            filler_q.append(qk_unit(3, 1, 2, 1, 6))
            t = attn_head(2, 1, t)
            filler_q.extend([
                ["tr1a", BIGC,
                 lambda: transposes_half(1, qbs=range(0, 4), hps=(0,)),
                 (3, 1, 2)],
                ["tr1b", BIGC,
                 lambda: transposes_half(1, qbs=range(4, 8), hps=(0,)),
                 (3, 1, 3)],
            ] + [
                # outproj(0) is the only arbitrarily-deferrable PE work:
                # deadline-pin it into the last head
                [f"op0{ob}", BIGC, (lambda ob=ob: outproj_ob(0, ob)),
                 (3, 1, 4 + ob)]
                for ob in range(8)
            ])
            t = attn_head(3, 1, t)
            t()
            drain_all()
            transposes_half(1, hps=(1,))
            outproj_half(1)
            if dump:
                nc.sync.dma_start(d_qk[:], qk_sb[:])
                nc.sync.dma_start(d_v[:], v_sb[:])
                nc.sync.dma_start(d_vals[:], vals_sb[:])
                nc.sync.dma_start(d_valsT[:], valsT_sb[:])

    nc.compile()
    return nc


def _prep_inputs_causal(x, mask, W_qkv, b_qkv, W_o, b_o):
    scale = np.float32(1.0 / np.sqrt(HD))
    xT = [np.ascontiguousarray(x[b].T).astype(NPBF16) for b in range(B)]
    in_maps = []
    for c in range(NCORES):
        b, g = divmod(c, HPC)
        heads = [HPC * g + i for i in range(HPC)]
        qc = [W_qkv[:, 192 * h : 192 * h + 64] * scale for h in heads]
        kc = [W_qkv[:, 192 * h + 64 : 192 * h + 128] for h in heads]
        # chunk order: q01 | k01 | q23 | k23; each chunk prepacked so the
        # per-partition row (dc, c) is contiguous in DRAM
        chunks = [
            np.concatenate([qc[0], qc[1]], axis=1),
            np.concatenate([kc[0], kc[1]], axis=1),
            np.concatenate([qc[2], qc[3]], axis=1),
            np.concatenate([kc[2], kc[3]], axis=1),
        ]
        wqk = np.stack([
            c.reshape(8, 128, 128).transpose(1, 0, 2).reshape(128, 1024)
            for c in chunks
        ])
        bq = [b_qkv[192 * h : 192 * h + 64] * scale for h in heads]
        bk = [b_qkv[192 * h + 64 : 192 * h + 128] for h in heads]
        bqk_t = np.concatenate(
            [bq[0], bq[1], bk[0], bk[1], bq[2], bq[3], bk[2], bk[3]]
        ).reshape(4, 128).T
        wv = np.concatenate(
            [W_qkv[:, 192 * h + 128 : 192 * h + 192] for h in heads], axis=1)
        bv = np.concatenate(
            [b_qkv[192 * h + 128 : 192 * h + 192] for h in heads])[None, :]
        wo = W_o[256 * g : 256 * (g + 1), :]
        in_maps.append({
            "xT": xT[b],
            "wqk": np.ascontiguousarray(wqk).astype(NPBF16),
            "bqk": np.ascontiguousarray(bqk_t).astype(np.float32),
            "wv": np.ascontiguousarray(wv).astype(NPBF16),
            "bv": np.ascontiguousarray(bv).astype(NPBF16),
            "wo": np.ascontiguousarray(wo).astype(NPBF16),
        })
    return in_maps


# ---------------------------------------------------------------------------
# Generic additive-mask fallback (previous kernel)
# ---------------------------------------------------------------------------

def _classify_mask(maskT):
    """pattern[hf][kb][nn] for [128, 512] tiles of mask^T (k major, q minor)."""
    pat = []
    for hf in range(2):
        rows = []
        for kb in range(16):
            ents = []
            for nn in range(2):
                blk = maskT[kb * 128 : (kb + 1) * 128,
                            hf * 1024 + nn * 512 : hf * 1024 + (nn + 1) * 512]
                if np.all(blk <= -1e8):
                    ents.append(SKIP)
                elif np.all(blk == 0.0):
                    ents.append(FULL)
                else:
                    ents.append(PART)
            rows.append(tuple(ents))
        pat.append(tuple(rows))
    return tuple(tuple(r) for r in pat)


def _build_generic(pattern):
    nc = bacc.Bacc(None, target_bir_lowering=False, debug=False)
    xT = nc.dram_tensor("xT", [D, S], F32R, kind="ExternalInput")
    wqk = nc.dram_tensor("wqk", [D, 512], F32R, kind="ExternalInput")
    bqk = nc.dram_tensor("bqk", [128, 4], F32R, kind="ExternalInput")
    wv = nc.dram_tensor("wv", [D, 256], F32R, kind="ExternalInput")
    bv = nc.dram_tensor("bv", [1, 256], F32R, kind="ExternalInput")
    wo = nc.dram_tensor("wo", [256, D], F32R, kind="ExternalInput")
    maskT = nc.dram_tensor("maskT", [S, S], F32R, kind="ExternalInput")
    outT = nc.dram_tensor("outT", [D, S], F32, kind="ExternalOutput")

    with tile.TileContext(nc) as tc:
        with (
            tc.tile_pool(name="big", bufs=1) as bigp,
            tc.tile_pool(name="wqkp", bufs=1) as wqkp,
            tc.tile_pool(name="wvp", bufs=1) as wvp,
            tc.tile_pool(name="wop", bufs=1) as wop,
            tc.tile_pool(name="qkp", bufs=1) as qkp,
            tc.tile_pool(name="vp", bufs=1) as vp,
            tc.tile_pool(name="valsp", bufs=1) as valsp,
            tc.tile_pool(name="attnp", bufs=3) as attnp,
            tc.tile_pool(name="maskp", bufs=1) as maskp,
            tc.tile_pool(name="smallp", bufs=1) as smallp,
            tc.tile_pool(name="constp", bufs=1) as constp,
        ):
            ones_t = constp.tile([1, 128], F32R)
            nc.gpsimd.memset(ones_t[:].bitcast(F32), 1.0)
            b_sb = constp.tile([128, 4], F32R)
            nc.sync.dma_start(b_sb[:], bqk[:])
            bv_sb = constp.tile([1, 256], F32R)
            nc.sync.dma_start(bv_sb[:], bv[:])

            wqk_sb = wqkp.tile([128, 8, 512], F32R, tag="wqkslot")
            xt_sb = bigp.tile([128, 8, S], F32R, tag="big")
            wv_sb = wvp.tile([128, 8, 256], F32R)
            for dc in range(8):
                nc.sync.dma_start(wqk_sb[:, dc, :], wqk[ds(dc * 128, 128), :])
                nc.sync.dma_start(
                    xt_sb[:, dc, ds(0, 1024)], xT[ds(dc * 128, 128), ds(0, 1024)]
                )
            nc.sync.dma_start(wv_sb[:], wv[:].rearrange("(dc p) c -> p dc c", p=128))

            qk_sb = qkp.tile([128, 4, S], F32R)
            v_sb = vp.tile([128, 16, HPC, 65], F32R)

            nc.gpsimd.memset(v_sb[:, :, :, 64:65].bitcast(F32), 1.0)

            def n_full(hf):
                return sum(c == FULL for kb in pattern[hf] for c in kb)

            hf_order = sorted(range(2), key=lambda hf: n_full(hf))
            part_blocks, slots, mask_tiles = {}, {}, {}
            for hf_i, hf in enumerate(hf_order):
                pb = [
                    (kb, nn)
                    for kb in range(16) for nn in range(2)
                    if pattern[hf][kb][nn] == PART
                ]
                part_blocks[hf] = pb
                slots[hf] = {blk: i for i, blk in enumerate(pb)}
                if len(pb) == 0:
                    mask_tiles[hf] = None
                elif len(pb) <= 8:
                    if hf_i == 0:
                        mask_tiles[hf] = maskp.tile(
                            [128, 8, 512], F32R, tag="mask", name=f"mask{hf}")
                    else:
                        mask_tiles[hf] = wqkp.tile(
                            [128, 8, 512], F32R, tag="wqkslot", name=f"mask{hf}")
                else:
                    mask_tiles[hf] = bigp.tile(
                        [128, 32, 512], F32R, tag="big", name=f"mask{hf}")

            def mask_dmas(hf, qs_):
                pb = part_blocks[hf]
                i = 0
                while i < len(pb):
                    kb0, nn0 = pb[i]
                    j = i + 1
                    while (j < len(pb)
                           and pb[j] == (pb[j - 1][0] + 1, nn0)):
                        j += 1
                    n = j - i
                    nc.sync.dma_start(
                        mask_tiles[hf][:, i : i + n, :],
                        maskT[ds(kb0 * 128, n * 128),
                              ds(qs_ + nn0 * 512, 512)].rearrange(
                            "(b p) q -> p b q", p=128),
                    )
                    i = j
            hf_first = hf_order[0]
            if mask_tiles.get(hf_first) is not None and len(part_blocks[hf_first]) <= 8:
                mask_dmas(hf_first, hf_first * 1024)
            for dc in range(8):
                nc.sync.dma_start(
                    xt_sb[:, dc, ds(1024, 1024)],
                    xT[ds(dc * 128, 128), ds(1024, 1024)],
                )
            wo_sb = wop.tile([128, 2, D], F32R)
            nc.sync.dma_start(wo_sb[:], wo[:].rearrange("(kc p) d -> p kc d", p=128))

            def qk_chunk_half(pool, cc, sh):
                ps = pool.tile([128, 1024], F32, tag="psqk", name=f"qkps{cc}{sh}")
                for dc in range(8):
                    lhsT = wqk_sb[:, dc, ts(cc, 128)]
                    for nn in range(2):
                        nc.tensor.matmul(
                            ps[:, ts(nn, 512)], lhsT,
                            xt_sb[:, dc, ds(sh * 1024 + nn * 512, 512)],
                            start=(dc == 0), stop=(dc == 7),
                        )
                nc.scalar.activation(
                    qk_sb[:, cc, ds(sh * 1024, 1024)], ps[:], AF.Identity,
                    bias=b_sb[:, cc : cc + 1],
                )

            def v_blocks(pool, rng):
                for sb_i in rng:
                    ps = pool.tile([128, 256], F32, tag="psv", name=f"vps{sb_i}")
                    for dc in range(8):
                        nc.tensor.matmul(
                            ps[:], xt_sb[:, dc, ts(sb_i, 128)], wv_sb[:, dc, :],
                            start=(dc == 0), stop=False,
                        )
                    nc.tensor.matmul(ps[:], ones_t[:], bv_sb[:], start=False, stop=True)
                    nc.vector.tensor_copy(
                        v_sb[:, sb_i, :, 0:64],
                        ps[:].rearrange("p (h e) -> p h e", h=HPC),
                    )

            with (
                tc.tile_pool(name="psqk", bufs=2, space="PSUM") as psqk,
                tc.tile_pool(name="psv", bufs=2, space="PSUM") as psv,
            ):
                for cc in (0, 2, 1, 3):
                    qk_chunk_half(psqk, cc, 0)
                v_blocks(psv, range(8))
                for cc in (0, 2, 1, 3):
                    qk_chunk_half(psqk, cc, 1)
                v_blocks(psv, range(8, 16))

            with (
                tc.tile_pool(name="pssc", bufs=3, space="PSUM") as pssc,
                tc.tile_pool(name="psav", bufs=1, space="PSUM") as psav,
            ):
                def outproj(vals_t, qs_):
                    for ob in range(8):
                        ps = pssc.tile([128, 1024], F32, tag="pssc")
                        for kc in range(2):
                            lhsT = wo_sb[:, kc, ts(ob, 128)]
                            for nn in range(2):
                                nc.tensor.matmul(
                                    ps[:, ts(nn, 512)], lhsT,
                                    vals_t[:, kc, ts(nn, 512)],
                                    start=(kc == 0), stop=(kc == 1),
                                )
                        oev = attnp.tile([128, 1024], F32, tag="attn")
                        nc.scalar.activation(oev[:], ps[:], AF.Copy)
                        nc.sync.dma_start(
                            outT[ds(ob * 128, 128), ds(qs_, 1024)], oev[:]
                        )

                pending = None
                for hf_i, hf in enumerate(hf_order):
                    qs = hf * 1024
                    slot = slots[hf]
                    mask_sb = mask_tiles[hf]
                    if mask_sb is not None and (
                        hf_i > 0 or len(part_blocks[hf]) > 8
                    ):
                        mask_dmas(hf, qs)
                    vals_sb = valsp.tile(
                        [128, 2, 1024], F32R, tag="vals", name=f"vals{hf}")
                    kb_order = sorted(
                        (kb for kb in range(16) if pattern[hf][kb] != (SKIP, SKIP)),
                        key=lambda kb: (PART in pattern[hf][kb], kb),
                    )
                    for h in range(HPC):
                        off = 64 * (h % 2)
                        qt = qk_sb[off : off + 64, h // 2, :]
                        kt = qk_sb[off : off + 64, 2 + h // 2, :]
                        act_kbs = [
                            [kb for kb in kb_order if pattern[hf][kb][nn] != SKIP]
                            for nn in range(2)
                        ]
                        ps_av = psav.tile([65, 1024], F32, tag="psav")
                        for kb in kb_order:
                            cls = pattern[hf][kb]
                            ps_sc = pssc.tile([128, 1024], F32, tag="pssc")
                            lhsT = kt[:, ts(kb, 128)]
                            at = attnp.tile([128, 1024], F32R, tag="attn")
                            for nn in range(2):
                                if cls[nn] == SKIP:
                                    continue
                                nc.tensor.matmul(
                                    ps_sc[:, ts(nn, 512)], lhsT,
                                    qt[:, ds(qs + nn * 512, 512)],
                                    start=True, stop=True,
                                )
                            if cls == (FULL, FULL):
                                nc.scalar.activation(at[:], ps_sc[:], AF.Exp)
                            else:
                                for nn in range(2):
                                    if cls[nn] == SKIP:
                                        continue
                                    if cls[nn] == PART:
                                        nc.vector.tensor_tensor(
                                            at[:, ts(nn, 512)], ps_sc[:, ts(nn, 512)],
                                            mask_sb[:, slot[(kb, nn)], :], ADD,
                                        )
                                        nc.scalar.activation(
                                            at[:, ts(nn, 512)], at[:, ts(nn, 512)],
                                            AF.Exp,
                                        )
                                    else:
                                        nc.scalar.activation(
                                            at[:, ts(nn, 512)], ps_sc[:, ts(nn, 512)],
                                            AF.Exp,
                                        )
                            for nn in range(2):
                                if cls[nn] == SKIP:
                                    continue
                                nc.tensor.matmul(
                                    ps_av[:, ts(nn, 512)], v_sb[:, kb, h, :],
                                    at[:, ts(nn, 512)],
                                    start=(kb == act_kbs[nn][0]),
                                    stop=(kb == act_kbs[nn][-1]),
                                )
                        recip = smallp.tile([1, 1024], F32R, tag="recip")
                        with nc.allow_low_precision(
                            reason="float32r has fp32 bits; only PE matmul mode differs"
                        ):
                            nc.vector.reciprocal(recip[:], ps_av[64:65, :])
                        bc_sb = smallp.tile([64, 1024], F32R, tag="bc")
                        nc.gpsimd.partition_broadcast(bc_sb[:], recip[:])
                        nc.vector.tensor_tensor(
                            vals_sb[off : off + 64, h // 2, :],
                            ps_av[0:64, :], bc_sb[:], MULT,
                        )
                        if pending is not None and hf_i == 1 and h == 0:
                            outproj(*pending)
                            pending = None

                    if hf_i == 0 and len(hf_order) > 1:
                        pending = (vals_sb, qs)
                    else:
                        outproj(vals_sb, qs)
                if pending is not None:
                    outproj(*pending)
                    pending = None

    nc.compile()
    return nc


def _prep_inputs_generic(x, mask, W_qkv, b_qkv, W_o, b_o):
    scale = np.float32(1.0 / np.sqrt(HD))
    xT = [np.ascontiguousarray(x[b].T) for b in range(B)]
    maskT = np.ascontiguousarray(mask.T)
    in_maps = []
    for c in range(NCORES):
        b, g = divmod(c, HPC)
        heads = [HPC * g + i for i in range(HPC)]
        qcols = np.concatenate(
            [W_qkv[:, 192 * h : 192 * h + 64] for h in heads], axis=1) * scale
        kcols = np.concatenate(
            [W_qkv[:, 192 * h + 64 : 192 * h + 128] for h in heads], axis=1)
        wqk = np.ascontiguousarray(np.concatenate([qcols, kcols], axis=1))
        bq = np.concatenate([b_qkv[192 * h : 192 * h + 64] for h in heads]) * scale
        bk = np.concatenate([b_qkv[192 * h + 64 : 192 * h + 128] for h in heads])
        bqk_t = np.ascontiguousarray(
            np.concatenate([bq, bk]).reshape(4, 128).T)
        wv = np.ascontiguousarray(np.concatenate(
            [W_qkv[:, 192 * h + 128 : 192 * h + 192] for h in heads], axis=1))
        bv = np.ascontiguousarray(np.concatenate(
            [b_qkv[192 * h + 128 : 192 * h + 192] for h in heads])[None, :])
        wo = np.ascontiguousarray(W_o[256 * g : 256 * (g + 1), :])
        in_maps.append({
            "xT": xT[b], "wqk": wqk, "bqk": bqk_t, "wv": wv, "bv": bv,
            "wo": wo, "maskT": maskT,
        })
    return in_maps


# ---------------------------------------------------------------------------
# Entry point
# ---------------------------------------------------------------------------

def _prep_inputs(x, mask, W_qkv, b_qkv, W_o, b_o):
    if _is_causal(mask):
        return _prep_inputs_causal(x, mask, W_qkv, b_qkv, W_o, b_o)
    return _prep_inputs_generic(x, mask, W_qkv, b_qkv, W_o, b_o)


def kernel(x, mask, W_qkv, b_qkv, W_o, b_o):
    x = np.asarray(x, dtype=np.float32)
    mask = np.asarray(mask, dtype=np.float32)
    W_qkv = np.asarray(W_qkv, dtype=np.float32)
    b_qkv = np.asarray(b_qkv, dtype=np.float32)
    W_o = np.asarray(W_o, dtype=np.float32)
    b_o = np.asarray(b_o, dtype=np.float32)

    if _is_causal(mask):
        key = "causal"
        if key not in _CACHE:
            _CACHE[key] = _build_causal()
        nc = _CACHE[key]
        in_maps = _prep_inputs_causal(x, mask, W_qkv, b_qkv, W_o, b_o)
    else:
        pattern = _classify_mask(np.ascontiguousarray(mask.T))
        key = ("nc", pattern)
        if key not in _CACHE:
            _CACHE[key] = _build_generic(pattern)
        nc = _CACHE[key]
        in_maps = _prep_inputs_generic(x, mask, W_qkv, b_qkv, W_o, b_o)
    _CACHE["nc"] = nc

    res = run_bass_kernel_spmd(nc, in_maps, core_ids=list(range(NCORES)))
    _CACHE["last_result"] = res

    out = np.empty((B, S, D), dtype=np.float32)
    for b in range(B):
        acc = res.results[HPC * b]["outT"].astype(np.float32)
        for g in range(1, HPC):
            acc = acc + res.results[HPC * b + g]["outT"].astype(np.float32)
        out[b] = acc.T + b_o
    return out


# revision 110
# speedup vs baseline: 1.5507x; 1.0747x over previous
"""Multi-head attention (B=2, S=2048, D=1024, H=16) on 8 TRN2 NeuronCores.

Sharding: core c handles batch b = c//4 and head-group g = c%4 (4 heads each).
Each core computes its heads' attention and a partial output projection
(row-parallel W_o); the host sums the 4 partials per batch and adds b_o.

Fast path (causal mask): bf16 operands off-PSUM, live-span score matmuls
(fully-masked columns never computed), masking via affine_select on the one
diagonal 128x128 block per score tile (no mask tensor at all), and a
transposed AV: attention tiles act as the stationary operand so each
accumulation step emits only 65 output columns ([q,64] values + a ones-column
denominator that lands as a per-partition scalar). vals are transposed back
with PE-transpose against an on-device identity for the row-parallel output
projection. DMAs are batched and spread across the SP/DVE/Pool queues.

A generic additive-mask fallback (the previous kernel) is kept for non-causal
masks.
"""

import numpy as np
import ml_dtypes

import concourse.bass as bass  # noqa: F401
import concourse.mybir as mybir
import concourse.tile as tile
from concourse import bacc
from concourse.bass import ds, ts
from concourse.bass_utils import run_bass_kernel_spmd

B, S, D, H = 2, 2048, 1024, 16
HD = D // H  # 64
HPC = 4      # heads per core
NCORES = 8
F32R = mybir.dt.float32r
F32 = mybir.dt.float32
BF16 = mybir.dt.bfloat16
NPBF16 = ml_dtypes.bfloat16
AF = mybir.ActivationFunctionType
ADD = mybir.AluOpType.add
MULT = mybir.AluOpType.mult

SKIP, FULL, PART = 0, 1, 2

_CACHE = {}


def _is_causal(mask):
    tril = np.tril(np.ones((S, S), dtype=bool))
    return bool(np.all(mask[tril] == 0.0) and np.all(mask[~tril] <= -1e8))


# ---------------------------------------------------------------------------
# Causal fast path
# ---------------------------------------------------------------------------

def _build_causal(dump=False):
    nc = bacc.Bacc(None, target_bir_lowering=False, debug=False)
    xT = nc.dram_tensor("xT", [D, S], BF16, kind="ExternalInput")
    # wqk host-prepacked per chunk: wqk[cc][p, dc*128 + c] = W[dc*128+p, cc*128+c]
    wqk = nc.dram_tensor("wqk", [4, 128, 1024], BF16, kind="ExternalInput")
    bqk = nc.dram_tensor("bqk", [128, 4], F32, kind="ExternalInput")
    wv = nc.dram_tensor("wv", [D, 256], BF16, kind="ExternalInput")
    bv = nc.dram_tensor("bv", [1, 256], BF16, kind="ExternalInput")
    wo = nc.dram_tensor("wo", [256, D], BF16, kind="ExternalInput")
    outT = nc.dram_tensor("outT", [D, S], BF16, kind="ExternalOutput")
    if dump:
        d_qk = nc.dram_tensor("d_qk", [128, 4, S], BF16, kind="ExternalOutput")
        d_v = nc.dram_tensor("d_v", [128, 16, HPC, 65], BF16, kind="ExternalOutput")
        d_vals = nc.dram_tensor(
            "d_vals", [128, 16, HPC, 64], BF16, kind="ExternalOutput")
        d_valsT = nc.dram_tensor("d_valsT", [128, 2, S], BF16, kind="ExternalOutput")
        d_at = nc.dram_tensor("d_at", [128, 16, 1024], BF16, kind="ExternalOutput")
        d_den = nc.dram_tensor("d_den", [128, 16, 65], F32, kind="ExternalOutput")

    with tile.TileContext(nc) as tc:
        with (
            tc.tile_pool(name="bigp", bufs=1) as bigp,
            tc.tile_pool(name="constp", bufs=1) as constp,
            tc.tile_pool(name="atp", bufs=20) as atp,
            tc.tile_pool(name="outp", bufs=2) as outp,
            tc.tile_pool(name="smallp", bufs=4) as smallp,
            tc.tile_pool(name="psBig", bufs=3, space="PSUM") as psBig,
            tc.tile_pool(name="psC", bufs=2, space="PSUM") as psC,
        ):
            # --- constants (small DMAs issue before the memset chain so
            # they hit the DMA device first) ---------------------------------
            b_sb = constp.tile([128, 4], F32)
            nc.gpsimd.dma_start(b_sb[:], bqk[:])
            bv_sb = constp.tile([1, 256], BF16)
            nc.gpsimd.dma_start(bv_sb[:], bv[:])
            # broadcast the V bias across partitions once so the projection
            # bias-add rides the PSUM->SBUF copy instead of a rank-1 matmul
            bv32 = constp.tile([1, 256], F32)
            nc.vector.tensor_copy(bv32[:], bv_sb[:])
            bv_bc = constp.tile([128, 256], F32)
            nc.gpsimd.partition_broadcast(bv_bc[:], bv32[:])
            ident = constp.tile([128, 128], BF16)
            nc.gpsimd.memset(ident[:], 1.0)
            nc.gpsimd.affine_select(
                out=ident[:], in_=ident[:],
                compare_op=mybir.AluOpType.is_equal, fill=0.0,
                base=0, pattern=[[-1, 128]], channel_multiplier=1,
            )
            # upper-triangular (incl. diagonal) ones: keeps k <= q when
            # multiplied into the diagonal block of an exp'd score tile
            tri_t = constp.tile([128, 128], BF16)
            nc.gpsimd.memset(tri_t[:], 1.0)
            nc.gpsimd.affine_select(
                out=tri_t[:], in_=tri_t[:],
                compare_op=mybir.AluOpType.is_ge, fill=0.0,
                base=0, pattern=[[1, 128]], channel_multiplier=-1,
            )

            # --- big SBUF tensors -----------------------------------------
            wqk_sb = constp.tile([128, 8, 512], BF16)
            xt_sb = bigp.tile([128, 8, S], BF16)
            wv_sb = constp.tile([128, 8, 256], BF16)
            wo_sb = constp.tile([128, 2, D], BF16)
            qk_sb = bigp.tile([128, 4, S], BF16)
            v_sb = bigp.tile([128, 16, HPC, 65], BF16)
            nc.gpsimd.memset(v_sb[:, :, :, 64:65], 1.0)
            vals_sb = bigp.tile([128, 16, HPC, 64], BF16)
            valsT_sb = bigp.tile([128, 2, S], BF16)

            # --- input DMAs ------------------------------------------------
            # wqk on the ACT queue (idle until the first exp); chunks are
            # host-prepacked contiguous so descriptors stay 2KB
            for cc in range(2):
                nc.scalar.dma_start(
                    wqk_sb[:, :, ts(cc, 128)],
                    wqk[cc].rearrange("p (dc c) -> p dc c", c=128),
                )
            nc.scalar.dma_start(
                wv_sb[:], wv[:].rearrange("(dc p) c -> p dc c", p=128))
            for cc in range(2, 4):
                nc.scalar.dma_start(
                    wqk_sb[:, :, ts(cc, 128)],
                    wqk[cc].rearrange("p (dc c) -> p dc c", c=128),
                )
            # xT + wv + wo on the SP queue: s-half-major then dc-chunks;
            # the first chunk is small so the first projection matmuls can
            # start as early as possible
            for d0, nd in ((0, 1), (1, 1), (2, 2), (4, 4)):
                nc.sync.dma_start(
                    xt_sb[:, ds(d0, nd), ds(0, 1024)],
                    xT[ds(d0 * 128, nd * 128), ds(0, 1024)].rearrange(
                        "(dc p) s -> p dc s", p=128),
                )
            for dh in range(2):
                nc.sync.dma_start(
                    xt_sb[:, ds(dh * 4, 4), ds(1024, 1024)],
                    xT[ds(dh * 512, 512), ds(1024, 1024)].rearrange(
                        "(dc p) s -> p dc s", p=128),
                )
            nc.sync.dma_start(
                wo_sb[:], wo[:].rearrange("(kc p) d -> p kc d", p=128))

            # --- projection helpers ---------------------------------------
            def qk_chunk(cc, sh):
                """qk_sb[:, cc, sh-half] = wqk-chunk-cc^T @ xT-half + bias."""
                ps = psBig.tile([128, 1024], F32, tag="psB", name=f"qkps{cc}{sh}")
                for dc in range(8):
                    lhsT = wqk_sb[:, dc, ts(cc, 128)]
                    for nn in range(2):
                        nc.tensor.matmul(
                            ps[:, ts(nn, 512)], lhsT,
                            xt_sb[:, dc, ds(sh * 1024 + nn * 512, 512)],
                            start=(dc == 0), stop=(dc == 7),
                        )
                nc.vector.tensor_scalar(
                    qk_sb[:, cc, ds(sh * 1024, 1024)], ps[:],
                    b_sb[:, cc : cc + 1], None, ADD,
                )

            def v_blocks(rng):
                for sb_i in rng:
                    ps = psBig.tile([128, 256], F32, tag="psB", name=f"vps{sb_i}")
                    for dc in range(8):
                        nc.tensor.matmul(
                            ps[:], xt_sb[:, dc, ts(sb_i, 128)], wv_sb[:, dc, :],
                            start=(dc == 0), stop=(dc == 7),
                        )
                    nc.vector.tensor_tensor(
                        v_sb[:, sb_i, :, 0:64],
                        ps[:].rearrange("p (h e) -> p h e", h=HPC),
                        bv_bc[:].rearrange("p (h e) -> p h e", h=HPC),
                        ADD,
                    )

            # --- attention for one (head, q-half) -------------------------
            # Filler machinery: the engines execute in-order, so PE starvation
            # during exp-bound attention stretches can only be avoided by
            # interleaving independent PE work (projections, outproj) into
            # the kb loops at fine granularity, paced by the ACT/PE balance.
            filler_q = []   # list of [name, pe_cost_ns, fn, deadline|None]
            balance = [0.0]

            def pop_fillers(here=None):
                # deadline-forced pops (emit everything up to the overdue
                # entry to preserve dependency order), then balance pops
                if here is not None:
                    h, hf, kb = here
                    due = None
                    for i, ent in enumerate(filler_q):
                        dl = ent[3]
                        if dl is not None and dl[0] == h and dl[1] == hf \
                                and kb >= dl[2]:
                            due = i
                    if due is not None:
                        for ent in filler_q[: due + 1]:
                            ent[2]()
                            balance[0] -= ent[1]
                        del filler_q[: due + 1]
                while filler_q and balance[0] >= filler_q[0][1] * 0.4:
                    name, cost, fn, _ = filler_q.pop(0)
                    fn()
                    balance[0] -= cost

            def drain_until(name):
                while filler_q:
                    nm, cost, fn, _ = filler_q.pop(0)
                    fn()
                    balance[0] -= cost
                    if nm == name:
                        break

            def drain_all():
                while filler_q:
                    _, cost, fn, _ = filler_q.pop(0)
                    fn()
                    balance[0] -= cost

            def attn_head(h, hf, pending_tail=None, av_delay=3):
                qs = hf * 1024
                off = 64 * (h % 2)
                qt = qk_sb[off : off + 64, 2 * (h // 2), :]
                kt = qk_sb[off : off + 64, 2 * (h // 2) + 1, :]
                nkb = 8 if hf == 0 else 16
                ats = []

                def av_group(qb_l):
                    # AV, qb-major: at-tiles are the stationary operand, so
                    # each accumulation step emits only 65 output columns; the
                    # ones column of V lands the denominator per-partition.
                    qb_g = hf * 8 + qb_l
                    pv = psC.tile([128, 65], F32, tag="psC", name=f"pav{h}{hf}{qb_l}")
                    for kb2 in range(qb_g + 1):
                        nc.tensor.matmul(
                            pv[:], ats[kb2][:, ts(qb_l, 128)], v_sb[:, kb2, h, :],
                            start=(kb2 == 0), stop=(kb2 == qb_g),
                        )
                    if dump and h == 0:
                        dtmp = smallp.tile([128, 65], F32, tag="dtmp",
                                           name=f"dd{hf}{qb_l}")
                        nc.vector.tensor_copy(dtmp[:], pv[:])
                        nc.sync.dma_start(d_den[:, qb_g, :], dtmp[:])
                    recip = smallp.tile([128, 1], F32, tag="recip")
                    nc.vector.reciprocal(recip[:], pv[:, 64:65])
                    nc.vector.tensor_scalar(
                        vals_sb[:, qb_g, h, :], pv[:, 0:64],
                        recip[:, 0:1], None, MULT,
                    )

                for kb in range(nkb):
                    if kb >= 1:
                        pop_fillers(here=(h, hf, kb))
                    lo = max(0, kb * 128 - qs)
                    if lo < 512:
                        ps = psBig.tile([128, 1024], F32, tag="psB",
                                        name=f"sc{h}{hf}{kb}")
                        base = 0
                    else:
                        ps = psBig.tile([128, 512], F32, tag="psB",
                                        name=f"sc{h}{hf}{kb}")
                        base = 512
                    lhsT = kt[:, ts(kb, 128)]
                    for seg in range(2):
                        a = max(lo, seg * 512)
                        bnd = (seg + 1) * 512
                        if a < bnd:
                            nc.tensor.matmul(
                                ps[:, ds(a - base, bnd - a)], lhsT,
                                qt[:, ds(qs + a, bnd - a)],
                                start=True, stop=True,
                            )
                    at = atp.tile([128, 1024], BF16, tag="at")
                    nc.scalar.activation(
                        at[:, ds(lo, 1024 - lo)], ps[:, ds(lo - base, 1024 - lo)],
                        AF.Exp)
                    if kb * 128 >= qs:
                        dlo = kb * 128 - qs
                        nc.gpsimd.tensor_tensor(
                            at[:, ds(dlo, 128)], at[:, ds(dlo, 128)],
                            tri_t[:], MULT,
                        )
                    if dump and h == 0:
                        nc.sync.dma_start(
                            d_at[:, hf * 8 + kb if hf == 0 else kb,
                                 ds(lo, 1024 - lo)],
                            at[:, ds(lo, 1024 - lo)])
                    ats.append(at)
                    if kb == 1 and pending_tail is not None:
                        pending_tail()
                    # AV groups run a couple of kbs behind so the
                    # scores->exp->mask chain of the diagonal tile never
                    # stalls the PE queue
                    pe_rows = 1024 - lo
                    if kb - hf * 8 - av_delay >= 0:
                        av_group(kb - hf * 8 - av_delay)
                        pe_rows += 65 * (kb - av_delay + 1)
                    balance[0] += ((1024 - lo) * 0.833 + 185
                                   - pe_rows * 0.4167 - 150)

                def tail():
                    for qb_l in range(8 - av_delay, 8):
                        av_group(qb_l)
                return tail

            # --- vals transpose + output projection for one q-half --------
            def transposes_half(hf, qbs=range(8), hps=(0, 1)):
                qs = hf * 1024
                for qb_l in qbs:
                    qb_g = hf * 8 + qb_l
                    for hp in hps:
                        pst = psBig.tile(
                            [128, 128], BF16, tag="psB", name=f"tr{hf}{qb_l}{hp}")
                        nc.tensor.transpose(
                            pst[:], vals_sb[:, qb_g, ds(hp * 2, 2), :], ident[:])
                        nc.vector.tensor_copy(
                            valsT_sb[:, hp, ds(qs + qb_l * 128, 128)], pst[:])

            _ot_state = {}

            def outproj_ob(hf, ob):
                qs = hf * 1024
                obg, obi = divmod(ob, 4)
                if (hf, obg) not in _ot_state:
                    _ot_state[(hf, obg)] = outp.tile(
                        [128, 4, 1024], BF16, tag="out", name=f"ot{hf}{obg}")
                ot = _ot_state[(hf, obg)]
                ps = psBig.tile([128, 1024], F32, tag="psB", name=f"op{hf}{ob}")
                for kc in range(2):
                    lhsT = wo_sb[:, kc, ts(ob, 128)]
                    for nn in range(2):
                        nc.tensor.matmul(
                            ps[:, ts(nn, 512)], lhsT,
                            valsT_sb[:, kc, ds(qs + nn * 512, 512)],
                            start=(kc == 0), stop=(kc == 1),
                        )
                # Pool cannot read PSUM (walrus restriction): hf0 copies go
                # on DVE; hf1 copies (tail, ACT idle) alternate DVE/ACT
                if hf == 1 and obi % 2 == 0:
                    nc.scalar.activation(ot[:, obi, :], ps[:], AF.Copy)
                else:
                    nc.vector.tensor_copy(ot[:, obi, :], ps[:])
                if hf == 1:
                    # per-ob DMAs at the tail, alternating queues, so the
                    # last transfer is small and starts as late as possible
                    dma_eng = nc.sync if obi % 2 == 0 else nc.scalar
                    dma_eng.dma_start(
                        outT[ds(ob * 128, 128), ds(qs, 1024)],
                        ot[:, obi, :],
                    )
                elif obi % 2 == 1:
                    dma_eng = nc.sync if obg % 2 == 0 else nc.scalar
                    dma_eng.dma_start(
                        outT[ds(obg * 512 + (obi - 1) * 128, 256),
                             ds(qs, 1024)].rearrange(
                            "(ob p) q -> p ob q", p=128),
                        ot[:, ds(obi - 1, 2), :],
                    )

            def outproj_half(hf):
                for ob in range(8):
                    outproj_ob(hf, ob)

            def qk_unit(cc, sh, h, hf, kb):
                """A projection chunk as one deadline-pinned filler unit
                (the open psum accumulation group cannot be interleaved with
                other pool allocations, so the chunk stays monolithic)."""
                return [f"qk{cc}{sh}", 10 ** 6,
                        (lambda: qk_chunk(cc, sh)), (h, hf, kb)]

            def qk_unit2(cc, sh, h, hf, kb):
                """A projection chunk as TWO filler units at consecutive kb
                deadlines. Safe against psum-slot recycling because only one
                pool allocation (that kb's score tile) lands between them,
                which is below the pool depth."""
                state = {}

                def half(d0):
                    def f():
                        if "ps" not in state:
                            state["ps"] = psBig.tile(
                                [128, 1024], F32, tag="psB",
                                name=f"qkps{cc}{sh}")
                        ps = state["ps"]
                        for dc in range(d0, d0 + 4):
                            lhsT = wqk_sb[:, dc, ts(cc, 128)]
                            for nn in range(2):
                                nc.tensor.matmul(
                                    ps[:, ts(nn, 512)], lhsT,
                                    xt_sb[:, dc, ds(sh * 1024 + nn * 512, 512)],
                                    start=(dc == 0), stop=(dc == 7),
                                )
                        if d0 == 4:
                            nc.vector.tensor_scalar(
                                qk_sb[:, cc, ds(sh * 1024, 1024)], ps[:],
                                b_sb[:, cc : cc + 1], None, ADD,
                            )
                    return f

                return [
                    [f"qk{cc}{sh}_a", 10 ** 6, half(0), (h, hf, kb)],
                    [f"qk{cc}{sh}_b", 10 ** 6, half(4), (h, hf, kb + 3)],
                ]

            # --- emission order: everything that is not on the critical
            # dependency prefix is deadline-pinned into the attention kb
            # loops so the in-order PE queue never runs dry.
            BIGC = 10 ** 6
            qk_chunk(0, 0)
            qk_chunk(1, 0)
            v_blocks(range(8))
            filler_q.extend(qk_unit2(2, 0, 0, 0, 3))
            filler_q.extend(qk_unit2(3, 0, 1, 0, 2))
            t = attn_head(0, 0)
            filler_q.extend(qk_unit2(0, 1, 2, 0, 2))
            filler_q.extend(qk_unit2(1, 1, 3, 0, 2))
            t = attn_head(1, 0, t)
            t = attn_head(2, 0, t)
            t = attn_head(3, 0, t)
            drain_all()
            filler_q.extend(
                [
                    [f"V{sb_i}", BIGC,
                     (lambda sb_i=sb_i: v_blocks([sb_i])),
                     (0, 1, sb_i - 5)]
                    for sb_i in range(8, 16)
                ]
                + [
                    ["tr0a", BIGC, lambda: transposes_half(0, qbs=range(0, 3)),
                     (1, 1, 2)],
                    ["tr0b", BIGC, lambda: transposes_half(0, qbs=range(3, 6)),
                     (1, 1, 3)],
                    ["tr0c", BIGC, lambda: transposes_half(0, qbs=range(6, 8)),
                     (1, 1, 4)],
                ]
                + qk_unit2(2, 1, 1, 1, 7)
            )
            t = attn_head(0, 1, t)
            t = attn_head(1, 1, t)
            drain_all()
            filler_q.append(qk_unit(3, 1, 2, 1, 6))
            filler_q.extend([
                # outproj(0) is the only arbitrarily-deferrable PE work:
                # spread it across the last two heads
                [f"op0{ob}", BIGC, (lambda ob=ob: outproj_ob(0, ob)),
                 (2, 1, 9 + 2 * ob) if ob < 4 else (3, 1, 2 * ob - 4)]
                for ob in range(4)
            ])
            t = attn_head(2, 1, t)
            filler_q.extend([
                ["tr1a", BIGC,
                 lambda: transposes_half(1, qbs=range(0, 4), hps=(0,)),
                 (3, 1, 2)],
                ["tr1b", BIGC,
                 lambda: transposes_half(1, qbs=range(4, 8), hps=(0,)),
                 (3, 1, 3)],

            ] + [
                [f"op0{ob}", BIGC, (lambda ob=ob: outproj_ob(0, ob)),
                 (3, 1, 2 * ob - 4)]
                for ob in range(4, 8)
            ])
            t = attn_head(3, 1, t, av_delay=2)
            t()
            drain_all()
            transposes_half(1, hps=(1,))
            outproj_half(1)
            if dump:
                nc.sync.dma_start(d_qk[:], qk_sb[:])
                nc.sync.dma_start(d_v[:], v_sb[:])
                nc.sync.dma_start(d_vals[:], vals_sb[:])
                nc.sync.dma_start(d_valsT[:], valsT_sb[:])

    nc.compile()
    return nc


def _prep_inputs_causal(x, mask, W_qkv, b_qkv, W_o, b_o):
    scale = np.float32(1.0 / np.sqrt(HD))
    xT = [np.ascontiguousarray(x[b].T).astype(NPBF16) for b in range(B)]
    in_maps = []
    for c in range(NCORES):
        b, g = divmod(c, HPC)
        heads = [HPC * g + i for i in range(HPC)]
        qc = [W_qkv[:, 192 * h : 192 * h + 64] * scale for h in heads]
        kc = [W_qkv[:, 192 * h + 64 : 192 * h + 128] for h in heads]
        # chunk order: q01 | k01 | q23 | k23; each chunk prepacked so the
        # per-partition row (dc, c) is contiguous in DRAM
        chunks = [
            np.concatenate([qc[0], qc[1]], axis=1),
            np.concatenate([kc[0], kc[1]], axis=1),
            np.concatenate([qc[2], qc[3]], axis=1),
            np.concatenate([kc[2], kc[3]], axis=1),
        ]
        wqk = np.stack([
            c.reshape(8, 128, 128).transpose(1, 0, 2).reshape(128, 1024)
            for c in chunks
        ])
        bq = [b_qkv[192 * h : 192 * h + 64] * scale for h in heads]
        bk = [b_qkv[192 * h + 64 : 192 * h + 128] for h in heads]
        bqk_t = np.concatenate(
            [bq[0], bq[1], bk[0], bk[1], bq[2], bq[3], bk[2], bk[3]]
        ).reshape(4, 128).T
        wv = np.concatenate(
            [W_qkv[:, 192 * h + 128 : 192 * h + 192] for h in heads], axis=1)
        bv = np.concatenate(
            [b_qkv[192 * h + 128 : 192 * h + 192] for h in heads])[None, :]
        wo = W_o[256 * g : 256 * (g + 1), :]
        in_maps.append({
            "xT": xT[b],
            "wqk": np.ascontiguousarray(wqk).astype(NPBF16),
            "bqk": np.ascontiguousarray(bqk_t).astype(np.float32),
            "wv": np.ascontiguousarray(wv).astype(NPBF16),
            "bv": np.ascontiguousarray(bv).astype(NPBF16),
            "wo": np.ascontiguousarray(wo).astype(NPBF16),
        })
    return in_maps


# ---------------------------------------------------------------------------
# Generic additive-mask fallback (previous kernel)
# ---------------------------------------------------------------------------

def _classify_mask(maskT):
    """pattern[hf][kb][nn] for [128, 512] tiles of mask^T (k major, q minor)."""
    pat = []
    for hf in range(2):
        rows = []
        for kb in range(16):
            ents = []
            for nn in range(2):
                blk = maskT[kb * 128 : (kb + 1) * 128,
                            hf * 1024 + nn * 512 : hf * 1024 + (nn + 1) * 512]
                if np.all(blk <= -1e8):
                    ents.append(SKIP)
                elif np.all(blk == 0.0):
                    ents.append(FULL)
                else:
                    ents.append(PART)
            rows.append(tuple(ents))
        pat.append(tuple(rows))
    return tuple(tuple(r) for r in pat)


def _build_generic(pattern):
    nc = bacc.Bacc(None, target_bir_lowering=False, debug=False)
    xT = nc.dram_tensor("xT", [D, S], F32R, kind="ExternalInput")
    wqk = nc.dram_tensor("wqk", [D, 512], F32R, kind="ExternalInput")
    bqk = nc.dram_tensor("bqk", [128, 4], F32R, kind="ExternalInput")
    wv = nc.dram_tensor("wv", [D, 256], F32R, kind="ExternalInput")
    bv = nc.dram_tensor("bv", [1, 256], F32R, kind="ExternalInput")
    wo = nc.dram_tensor("wo", [256, D], F32R, kind="ExternalInput")
    maskT = nc.dram_tensor("maskT", [S, S], F32R, kind="ExternalInput")
    outT = nc.dram_tensor("outT", [D, S], F32, kind="ExternalOutput")

    with tile.TileContext(nc) as tc:
        with (
            tc.tile_pool(name="big", bufs=1) as bigp,
            tc.tile_pool(name="wqkp", bufs=1) as wqkp,
            tc.tile_pool(name="wvp", bufs=1) as wvp,
            tc.tile_pool(name="wop", bufs=1) as wop,
            tc.tile_pool(name="qkp", bufs=1) as qkp,
            tc.tile_pool(name="vp", bufs=1) as vp,
            tc.tile_pool(name="valsp", bufs=1) as valsp,
            tc.tile_pool(name="attnp", bufs=3) as attnp,
            tc.tile_pool(name="maskp", bufs=1) as maskp,
            tc.tile_pool(name="smallp", bufs=1) as smallp,
            tc.tile_pool(name="constp", bufs=1) as constp,
        ):
            ones_t = constp.tile([1, 128], F32R)
            nc.gpsimd.memset(ones_t[:].bitcast(F32), 1.0)
            b_sb = constp.tile([128, 4], F32R)
            nc.sync.dma_start(b_sb[:], bqk[:])
            bv_sb = constp.tile([1, 256], F32R)
            nc.sync.dma_start(bv_sb[:], bv[:])

            wqk_sb = wqkp.tile([128, 8, 512], F32R, tag="wqkslot")
            xt_sb = bigp.tile([128, 8, S], F32R, tag="big")
            wv_sb = wvp.tile([128, 8, 256], F32R)
            for dc in range(8):
                nc.sync.dma_start(wqk_sb[:, dc, :], wqk[ds(dc * 128, 128), :])
                nc.sync.dma_start(
                    xt_sb[:, dc, ds(0, 1024)], xT[ds(dc * 128, 128), ds(0, 1024)]
                )
            nc.sync.dma_start(wv_sb[:], wv[:].rearrange("(dc p) c -> p dc c", p=128))

            qk_sb = qkp.tile([128, 4, S], F32R)
            v_sb = vp.tile([128, 16, HPC, 65], F32R)

            nc.gpsimd.memset(v_sb[:, :, :, 64:65].bitcast(F32), 1.0)

            def n_full(hf):
                return sum(c == FULL for kb in pattern[hf] for c in kb)

            hf_order = sorted(range(2), key=lambda hf: n_full(hf))
            part_blocks, slots, mask_tiles = {}, {}, {}
            for hf_i, hf in enumerate(hf_order):
                pb = [
                    (kb, nn)
                    for kb in range(16) for nn in range(2)
                    if pattern[hf][kb][nn] == PART
                ]
                part_blocks[hf] = pb
                slots[hf] = {blk: i for i, blk in enumerate(pb)}
                if len(pb) == 0:
                    mask_tiles[hf] = None
                elif len(pb) <= 8:
                    if hf_i == 0:
                        mask_tiles[hf] = maskp.tile(
                            [128, 8, 512], F32R, tag="mask", name=f"mask{hf}")
                    else:
                        mask_tiles[hf] = wqkp.tile(
                            [128, 8, 512], F32R, tag="wqkslot", name=f"mask{hf}")
                else:
                    mask_tiles[hf] = bigp.tile(
                        [128, 32, 512], F32R, tag="big", name=f"mask{hf}")

            def mask_dmas(hf, qs_):
                pb = part_blocks[hf]
                i = 0
                while i < len(pb):
                    kb0, nn0 = pb[i]
                    j = i + 1
                    while (j < len(pb)
                           and pb[j] == (pb[j - 1][0] + 1, nn0)):
                        j += 1
                    n = j - i
                    nc.sync.dma_start(
                        mask_tiles[hf][:, i : i + n, :],
                        maskT[ds(kb0 * 128, n * 128),
                              ds(qs_ + nn0 * 512, 512)].rearrange(
                            "(b p) q -> p b q", p=128),
                    )
                    i = j
            hf_first = hf_order[0]
            if mask_tiles.get(hf_first) is not None and len(part_blocks[hf_first]) <= 8:
                mask_dmas(hf_first, hf_first * 1024)
            for dc in range(8):
                nc.sync.dma_start(
                    xt_sb[:, dc, ds(1024, 1024)],
                    xT[ds(dc * 128, 128), ds(1024, 1024)],
                )
            wo_sb = wop.tile([128, 2, D], F32R)
            nc.sync.dma_start(wo_sb[:], wo[:].rearrange("(kc p) d -> p kc d", p=128))

            def qk_chunk_half(pool, cc, sh):
                ps = pool.tile([128, 1024], F32, tag="psqk", name=f"qkps{cc}{sh}")
                for dc in range(8):
                    lhsT = wqk_sb[:, dc, ts(cc, 128)]
                    for nn in range(2):
                        nc.tensor.matmul(
                            ps[:, ts(nn, 512)], lhsT,
                            xt_sb[:, dc, ds(sh * 1024 + nn * 512, 512)],
                            start=(dc == 0), stop=(dc == 7),
                        )
                nc.scalar.activation(
                    qk_sb[:, cc, ds(sh * 1024, 1024)], ps[:], AF.Identity,
                    bias=b_sb[:, cc : cc + 1],
                )

            def v_blocks(pool, rng):
                for sb_i in rng:
                    ps = pool.tile([128, 256], F32, tag="psv", name=f"vps{sb_i}")
                    for dc in range(8):
                        nc.tensor.matmul(
                            ps[:], xt_sb[:, dc, ts(sb_i, 128)], wv_sb[:, dc, :],
                            start=(dc == 0), stop=False,
                        )
                    nc.tensor.matmul(ps[:], ones_t[:], bv_sb[:], start=False, stop=True)
                    nc.vector.tensor_copy(
                        v_sb[:, sb_i, :, 0:64],
                        ps[:].rearrange("p (h e) -> p h e", h=HPC),
                    )

            with (
                tc.tile_pool(name="psqk", bufs=2, space="PSUM") as psqk,
                tc.tile_pool(name="psv", bufs=2, space="PSUM") as psv,
            ):
                for cc in (0, 2, 1, 3):
                    qk_chunk_half(psqk, cc, 0)
                v_blocks(psv, range(8))
                for cc in (0, 2, 1, 3):
                    qk_chunk_half(psqk, cc, 1)
                v_blocks(psv, range(8, 16))

            with (
                tc.tile_pool(name="pssc", bufs=3, space="PSUM") as pssc,
                tc.tile_pool(name="psav", bufs=1, space="PSUM") as psav,
            ):
                def outproj(vals_t, qs_):
                    for ob in range(8):
                        ps = pssc.tile([128, 1024], F32, tag="pssc")
                        for kc in range(2):
                            lhsT = wo_sb[:, kc, ts(ob, 128)]
                            for nn in range(2):
                                nc.tensor.matmul(
                                    ps[:, ts(nn, 512)], lhsT,
                                    vals_t[:, kc, ts(nn, 512)],
                                    start=(kc == 0), stop=(kc == 1),
                                )
                        oev = attnp.tile([128, 1024], F32, tag="attn")
                        nc.scalar.activation(oev[:], ps[:], AF.Copy)
                        nc.sync.dma_start(
                            outT[ds(ob * 128, 128), ds(qs_, 1024)], oev[:]
                        )

                pending = None
                for hf_i, hf in enumerate(hf_order):
                    qs = hf * 1024
                    slot = slots[hf]
                    mask_sb = mask_tiles[hf]
                    if mask_sb is not None and (
                        hf_i > 0 or len(part_blocks[hf]) > 8
                    ):
                        mask_dmas(hf, qs)
                    vals_sb = valsp.tile(
                        [128, 2, 1024], F32R, tag="vals", name=f"vals{hf}")
                    kb_order = sorted(
                        (kb for kb in range(16) if pattern[hf][kb] != (SKIP, SKIP)),
                        key=lambda kb: (PART in pattern[hf][kb], kb),
                    )
                    for h in range(HPC):
                        off = 64 * (h % 2)
                        qt = qk_sb[off : off + 64, h // 2, :]
                        kt = qk_sb[off : off + 64, 2 + h // 2, :]
                        act_kbs = [
                            [kb for kb in kb_order if pattern[hf][kb][nn] != SKIP]
                            for nn in range(2)
                        ]
                        ps_av = psav.tile([65, 1024], F32, tag="psav")
                        for kb in kb_order:
                            cls = pattern[hf][kb]
                            ps_sc = pssc.tile([128, 1024], F32, tag="pssc")
                            lhsT = kt[:, ts(kb, 128)]
                            at = attnp.tile([128, 1024], F32R, tag="attn")
                            for nn in range(2):
                                if cls[nn] == SKIP:
                                    continue
                                nc.tensor.matmul(
                                    ps_sc[:, ts(nn, 512)], lhsT,
                                    qt[:, ds(qs + nn * 512, 512)],
                                    start=True, stop=True,
                                )
                            if cls == (FULL, FULL):
                                nc.scalar.activation(at[:], ps_sc[:], AF.Exp)
                            else:
                                for nn in range(2):
                                    if cls[nn] == SKIP:
                                        continue
                                    if cls[nn] == PART:
                                        nc.vector.tensor_tensor(
                                            at[:, ts(nn, 512)], ps_sc[:, ts(nn, 512)],
                                            mask_sb[:, slot[(kb, nn)], :], ADD,
                                        )
                                        nc.scalar.activation(
                                            at[:, ts(nn, 512)], at[:, ts(nn, 512)],
                                            AF.Exp,
                                        )
                                    else:
                                        nc.scalar.activation(
                                            at[:, ts(nn, 512)], ps_sc[:, ts(nn, 512)],
                                            AF.Exp,
                                        )
                            for nn in range(2):
                                if cls[nn] == SKIP:
                                    continue
                                nc.tensor.matmul(
                                    ps_av[:, ts(nn, 512)], v_sb[:, kb, h, :],
                                    at[:, ts(nn, 512)],
                                    start=(kb == act_kbs[nn][0]),
                                    stop=(kb == act_kbs[nn][-1]),
                                )
                        recip = smallp.tile([1, 1024], F32R, tag="recip")
                        with nc.allow_low_precision(
                            reason="float32r has fp32 bits; only PE matmul mode differs"
                        ):
                            nc.vector.reciprocal(recip[:], ps_av[64:65, :])
                        bc_sb = smallp.tile([64, 1024], F32R, tag="bc")
                        nc.gpsimd.partition_broadcast(bc_sb[:], recip[:])
                        nc.vector.tensor_tensor(
                            vals_sb[off : off + 64, h // 2, :],
                            ps_av[0:64, :], bc_sb[:], MULT,
                        )
                        if pending is not None and hf_i == 1 and h == 0:
                            outproj(*pending)
                            pending = None

                    if hf_i == 0 and len(hf_order) > 1:
                        pending = (vals_sb, qs)
                    else:
                        outproj(vals_sb, qs)
                if pending is not None:
                    outproj(*pending)
                    pending = None

    nc.compile()
    return nc


def _prep_inputs_generic(x, mask, W_qkv, b_qkv, W_o, b_o):
    scale = np.float32(1.0 / np.sqrt(HD))
    xT = [np.ascontiguousarray(x[b].T) for b in range(B)]
    maskT = np.ascontiguousarray(mask.T)
    in_maps = []
    for c in range(NCORES):
        b, g = divmod(c, HPC)
        heads = [HPC * g + i for i in range(HPC)]
        qcols = np.concatenate(
            [W_qkv[:, 192 * h : 192 * h + 64] for h in heads], axis=1) * scale
        kcols = np.concatenate(
            [W_qkv[:, 192 * h + 64 : 192 * h + 128] for h in heads], axis=1)
        wqk = np.ascontiguousarray(np.concatenate([qcols, kcols], axis=1))
        bq = np.concatenate([b_qkv[192 * h : 192 * h + 64] for h in heads]) * scale
        bk = np.concatenate([b_qkv[192 * h + 64 : 192 * h + 128] for h in heads])
        bqk_t = np.ascontiguousarray(
            np.concatenate([bq, bk]).reshape(4, 128).T)
        wv = np.ascontiguousarray(np.concatenate(
            [W_qkv[:, 192 * h + 128 : 192 * h + 192] for h in heads], axis=1))
        bv = np.ascontiguousarray(np.concatenate(
            [b_qkv[192 * h + 128 : 192 * h + 192] for h in heads])[None, :])
        wo = np.ascontiguousarray(W_o[256 * g : 256 * (g + 1), :])
        in_maps.append({
            "xT": xT[b], "wqk": wqk, "bqk": bqk_t, "wv": wv, "bv": bv,
            "wo": wo, "maskT": maskT,
        })
    return in_maps


# ---------------------------------------------------------------------------
# Entry point
# ---------------------------------------------------------------------------

def _prep_inputs(x, mask, W_qkv, b_qkv, W_o, b_o):
    if _is_causal(mask):
        return _prep_inputs_causal(x, mask, W_qkv, b_qkv, W_o, b_o)
    return _prep_inputs_generic(x, mask, W_qkv, b_qkv, W_o, b_o)


def kernel(x, mask, W_qkv, b_qkv, W_o, b_o):
    x = np.asarray(x, dtype=np.float32)
    mask = np.asarray(mask, dtype=np.float32)
    W_qkv = np.asarray(W_qkv, dtype=np.float32)
    b_qkv = np.asarray(b_qkv, dtype=np.float32)
    W_o = np.asarray(W_o, dtype=np.float32)
    b_o = np.asarray(b_o, dtype=np.float32)

    if _is_causal(mask):
        key = "causal"
        if key not in _CACHE:
            _CACHE[key] = _build_causal()
        nc = _CACHE[key]
        in_maps = _prep_inputs_causal(x, mask, W_qkv, b_qkv, W_o, b_o)
    else:
        pattern = _classify_mask(np.ascontiguousarray(mask.T))
        key = ("nc", pattern)
        if key not in _CACHE:
            _CACHE[key] = _build_generic(pattern)
        nc = _CACHE[key]
        in_maps = _prep_inputs_generic(x, mask, W_qkv, b_qkv, W_o, b_o)
    _CACHE["nc"] = nc

    res = run_bass_kernel_spmd(nc, in_maps, core_ids=list(range(NCORES)))
    _CACHE["last_result"] = res

    out = np.empty((B, S, D), dtype=np.float32)
    for b in range(B):
        acc = res.results[HPC * b]["outT"].astype(np.float32)
        for g in range(1, HPC):
            acc = acc + res.results[HPC * b + g]["outT"].astype(np.float32)
        out[b] = acc.T + b_o
    return out


# revision 112
# speedup vs baseline: 1.5690x; 1.0118x over previous
"""Multi-head attention (B=2, S=2048, D=1024, H=16) on 8 TRN2 NeuronCores.

Sharding: core c handles batch b = c//4 and head-group g = c%4 (4 heads each).
Each core computes its heads' attention and a partial output projection
(row-parallel W_o); the host sums the 4 partials per batch and adds b_o.

Fast path (causal mask): bf16 operands off-PSUM, live-span score matmuls
(fully-masked columns never computed), masking via affine_select on the one
diagonal 128x128 block per score tile (no mask tensor at all), and a
transposed AV: attention tiles act as the stationary operand so each
accumulation step emits only 65 output columns ([q,64] values + a ones-column
denominator that lands as a per-partition scalar). vals are transposed back
with PE-transpose against an on-device identity for the row-parallel output
projection. DMAs are batched and spread across the SP/DVE/Pool queues.

A generic additive-mask fallback (the previous kernel) is kept for non-causal
masks.
"""

import numpy as np
import ml_dtypes

import concourse.bass as bass  # noqa: F401
import concourse.mybir as mybir
import concourse.tile as tile
from concourse import bacc
from concourse.bass import ds, ts
from concourse.bass_utils import run_bass_kernel_spmd

B, S, D, H = 2, 2048, 1024, 16
HD = D // H  # 64
HPC = 4      # heads per core
NCORES = 8
F32R = mybir.dt.float32r
F32 = mybir.dt.float32
BF16 = mybir.dt.bfloat16
NPBF16 = ml_dtypes.bfloat16
AF = mybir.ActivationFunctionType
ADD = mybir.AluOpType.add
MULT = mybir.AluOpType.mult

SKIP, FULL, PART = 0, 1, 2

_CACHE = {}


def _is_causal(mask):
    tril = np.tril(np.ones((S, S), dtype=bool))
    return bool(np.all(mask[tril] == 0.0) and np.all(mask[~tril] <= -1e8))


# ---------------------------------------------------------------------------
# Causal fast path
# ---------------------------------------------------------------------------

def _build_causal(dump=False):
    nc = bacc.Bacc(None, target_bir_lowering=False, debug=False)
    xT = nc.dram_tensor("xT", [D, S], BF16, kind="ExternalInput")
    # wqk host-prepacked per chunk: wqk[cc][p, dc*128 + c] = W[dc*128+p, cc*128+c]
    wqk = nc.dram_tensor("wqk", [4, 128, 1024], BF16, kind="ExternalInput")
    bqk = nc.dram_tensor("bqk", [128, 4], F32, kind="ExternalInput")
    wv = nc.dram_tensor("wv", [D, 256], BF16, kind="ExternalInput")
    bv = nc.dram_tensor("bv", [1, 256], BF16, kind="ExternalInput")
    wo = nc.dram_tensor("wo", [256, D], BF16, kind="ExternalInput")
    outT = nc.dram_tensor("outT", [D, S], BF16, kind="ExternalOutput")
    if dump:
        d_qk = nc.dram_tensor("d_qk", [128, 4, S], BF16, kind="ExternalOutput")
        d_v = nc.dram_tensor("d_v", [128, 16, HPC, 65], BF16, kind="ExternalOutput")
        d_vals = nc.dram_tensor(
            "d_vals", [128, 16, HPC, 64], BF16, kind="ExternalOutput")
        d_valsT = nc.dram_tensor("d_valsT", [128, 2, S], BF16, kind="ExternalOutput")
        d_at = nc.dram_tensor("d_at", [128, 16, 1024], BF16, kind="ExternalOutput")
        d_den = nc.dram_tensor("d_den", [128, 16, 65], F32, kind="ExternalOutput")

    with tile.TileContext(nc) as tc:
        with (
            tc.tile_pool(name="bigp", bufs=1) as bigp,
            tc.tile_pool(name="constp", bufs=1) as constp,
            tc.tile_pool(name="atp", bufs=20) as atp,
            tc.tile_pool(name="outp", bufs=2) as outp,
            tc.tile_pool(name="smallp", bufs=4) as smallp,
            tc.tile_pool(name="psBig", bufs=3, space="PSUM") as psBig,
            tc.tile_pool(name="psC", bufs=2, space="PSUM") as psC,
        ):
            # --- constants (small DMAs issue before the memset chain so
            # they hit the DMA device first) ---------------------------------
            b_sb = constp.tile([128, 4], F32)
            nc.gpsimd.dma_start(b_sb[:], bqk[:])
            bv_sb = constp.tile([1, 256], BF16)
            nc.gpsimd.dma_start(bv_sb[:], bv[:])
            # broadcast the V bias across partitions once so the projection
            # bias-add rides the PSUM->SBUF copy instead of a rank-1 matmul
            bv32 = constp.tile([1, 256], F32)
            nc.vector.tensor_copy(bv32[:], bv_sb[:])
            bv_bc = constp.tile([128, 256], F32)
            nc.gpsimd.partition_broadcast(bv_bc[:], bv32[:])
            ident = constp.tile([128, 128], BF16)
            nc.gpsimd.memset(ident[:], 1.0)
            nc.gpsimd.affine_select(
                out=ident[:], in_=ident[:],
                compare_op=mybir.AluOpType.is_equal, fill=0.0,
                base=0, pattern=[[-1, 128]], channel_multiplier=1,
            )
            # upper-triangular (incl. diagonal) ones: keeps k <= q when
            # multiplied into the diagonal block of an exp'd score tile
            tri_t = constp.tile([128, 128], BF16)
            nc.gpsimd.memset(tri_t[:], 1.0)
            nc.gpsimd.affine_select(
                out=tri_t[:], in_=tri_t[:],
                compare_op=mybir.AluOpType.is_ge, fill=0.0,
                base=0, pattern=[[1, 128]], channel_multiplier=-1,
            )

            # --- big SBUF tensors -----------------------------------------
            wqk_sb = constp.tile([128, 8, 512], BF16)
            xt_sb = bigp.tile([128, 8, S], BF16)
            wv_sb = constp.tile([128, 8, 256], BF16)
            wo_sb = constp.tile([128, 2, D], BF16)
            qk_sb = bigp.tile([128, 4, S], BF16)
            v_sb = bigp.tile([128, 16, HPC, 65], BF16)
            nc.gpsimd.memset(v_sb[:, :, :, 64:65], 1.0)
            vals_sb = bigp.tile([128, 16, HPC, 64], BF16)
            valsT_sb = bigp.tile([128, 2, S], BF16)

            # --- input DMAs ------------------------------------------------
            # wqk on the ACT queue (idle until the first exp); chunks are
            # host-prepacked contiguous so descriptors stay 2KB
            for cc in range(2):
                nc.scalar.dma_start(
                    wqk_sb[:, :, ts(cc, 128)],
                    wqk[cc].rearrange("p (dc c) -> p dc c", c=128),
                )
            nc.scalar.dma_start(
                wv_sb[:], wv[:].rearrange("(dc p) c -> p dc c", p=128))
            for cc in range(2, 4):
                nc.scalar.dma_start(
                    wqk_sb[:, :, ts(cc, 128)],
                    wqk[cc].rearrange("p (dc c) -> p dc c", c=128),
                )
            # xT + wv + wo on the SP queue: s-half-major then dc-chunks;
            # the first chunk is small so the first projection matmuls can
            # start as early as possible
            for d0, nd in ((0, 1), (1, 1), (2, 2), (4, 4)):
                nc.sync.dma_start(
                    xt_sb[:, ds(d0, nd), ds(0, 1024)],
                    xT[ds(d0 * 128, nd * 128), ds(0, 1024)].rearrange(
                        "(dc p) s -> p dc s", p=128),
                )
            for dh in range(2):
                nc.sync.dma_start(
                    xt_sb[:, ds(dh * 4, 4), ds(1024, 1024)],
                    xT[ds(dh * 512, 512), ds(1024, 1024)].rearrange(
                        "(dc p) s -> p dc s", p=128),
                )
            nc.sync.dma_start(
                wo_sb[:], wo[:].rearrange("(kc p) d -> p kc d", p=128))

            # --- projection helpers ---------------------------------------
            def qk_chunk(cc, sh):
                """qk_sb[:, cc, sh-half] = wqk-chunk-cc^T @ xT-half + bias."""
                ps = psBig.tile([128, 1024], F32, tag="psB", name=f"qkps{cc}{sh}")
                for dc in range(8):
                    lhsT = wqk_sb[:, dc, ts(cc, 128)]
                    for nn in range(2):
                        nc.tensor.matmul(
                            ps[:, ts(nn, 512)], lhsT,
                            xt_sb[:, dc, ds(sh * 1024 + nn * 512, 512)],
                            start=(dc == 0), stop=(dc == 7),
                        )
                nc.vector.tensor_scalar(
                    qk_sb[:, cc, ds(sh * 1024, 1024)], ps[:],
                    b_sb[:, cc : cc + 1], None, ADD,
                )

            def v_blocks(rng):
                for sb_i in rng:
                    ps = psBig.tile([128, 256], F32, tag="psB", name=f"vps{sb_i}")
                    for dc in range(8):
                        nc.tensor.matmul(
                            ps[:], xt_sb[:, dc, ts(sb_i, 128)], wv_sb[:, dc, :],
                            start=(dc == 0), stop=(dc == 7),
                        )
                    nc.vector.tensor_tensor(
                        v_sb[:, sb_i, :, 0:64],
                        ps[:].rearrange("p (h e) -> p h e", h=HPC),
                        bv_bc[:].rearrange("p (h e) -> p h e", h=HPC),
                        ADD,
                    )

            # --- attention for one (head, q-half) -------------------------
            # Filler machinery: the engines execute in-order, so PE starvation
            # during exp-bound attention stretches can only be avoided by
            # interleaving independent PE work (projections, outproj) into
            # the kb loops at fine granularity, paced by the ACT/PE balance.
            filler_q = []   # list of [name, pe_cost_ns, fn, deadline|None]
            balance = [0.0]

            def pop_fillers(here=None):
                # deadline-forced pops (emit everything up to the overdue
                # entry to preserve dependency order), then balance pops
                if here is not None:
                    h, hf, kb = here
                    due = None
                    for i, ent in enumerate(filler_q):
                        dl = ent[3]
                        if dl is not None and dl[0] == h and dl[1] == hf \
                                and kb >= dl[2]:
                            due = i
                    if due is not None:
                        for ent in filler_q[: due + 1]:
                            ent[2]()
                            balance[0] -= ent[1]
                        del filler_q[: due + 1]
                while filler_q and balance[0] >= filler_q[0][1] * 0.4:
                    name, cost, fn, _ = filler_q.pop(0)
                    fn()
                    balance[0] -= cost

            def drain_until(name):
                while filler_q:
                    nm, cost, fn, _ = filler_q.pop(0)
                    fn()
                    balance[0] -= cost
                    if nm == name:
                        break

            def drain_all():
                while filler_q:
                    _, cost, fn, _ = filler_q.pop(0)
                    fn()
                    balance[0] -= cost

            def attn_head(h, hf, pending_tail=None, av_delay=3):
                qs = hf * 1024
                off = 64 * (h % 2)
                qt = qk_sb[off : off + 64, 2 * (h // 2), :]
                kt = qk_sb[off : off + 64, 2 * (h // 2) + 1, :]
                nkb = 8 if hf == 0 else 16
                ats = []

                def av_group(qb_l):
                    # AV, qb-major: at-tiles are the stationary operand, so
                    # each accumulation step emits only 65 output columns; the
                    # ones column of V lands the denominator per-partition.
                    qb_g = hf * 8 + qb_l
                    pv = psC.tile([128, 65], F32, tag="psC", name=f"pav{h}{hf}{qb_l}")
                    for kb2 in range(qb_g + 1):
                        nc.tensor.matmul(
                            pv[:], ats[kb2][:, ts(qb_l, 128)], v_sb[:, kb2, h, :],
                            start=(kb2 == 0), stop=(kb2 == qb_g),
                        )
                    if dump and h == 0:
                        dtmp = smallp.tile([128, 65], F32, tag="dtmp",
                                           name=f"dd{hf}{qb_l}")
                        nc.vector.tensor_copy(dtmp[:], pv[:])
                        nc.sync.dma_start(d_den[:, qb_g, :], dtmp[:])
                    recip = smallp.tile([128, 1], F32, tag="recip")
                    nc.vector.reciprocal(recip[:], pv[:, 64:65])
                    nc.vector.tensor_scalar(
                        vals_sb[:, qb_g, h, :], pv[:, 0:64],
                        recip[:, 0:1], None, MULT,
                    )

                for kb in range(nkb):
                    if kb >= 1:
                        pop_fillers(here=(h, hf, kb))
                    lo = max(0, kb * 128 - qs)
                    if lo < 512:
                        ps = psBig.tile([128, 1024], F32, tag="psB",
                                        name=f"sc{h}{hf}{kb}")
                        base = 0
                    else:
                        ps = psBig.tile([128, 512], F32, tag="psB",
                                        name=f"sc{h}{hf}{kb}")
                        base = 512
                    lhsT = kt[:, ts(kb, 128)]
                    for seg in range(2):
                        a = max(lo, seg * 512)
                        bnd = (seg + 1) * 512
                        if a < bnd:
                            nc.tensor.matmul(
                                ps[:, ds(a - base, bnd - a)], lhsT,
                                qt[:, ds(qs + a, bnd - a)],
                                start=True, stop=True,
                            )
                    at = atp.tile([128, 1024], BF16, tag="at")
                    nc.scalar.activation(
                        at[:, ds(lo, 1024 - lo)], ps[:, ds(lo - base, 1024 - lo)],
                        AF.Exp)
                    if kb * 128 >= qs:
                        dlo = kb * 128 - qs
                        nc.gpsimd.tensor_tensor(
                            at[:, ds(dlo, 128)], at[:, ds(dlo, 128)],
                            tri_t[:], MULT,
                        )
                    if dump and h == 0:
                        nc.sync.dma_start(
                            d_at[:, hf * 8 + kb if hf == 0 else kb,
                                 ds(lo, 1024 - lo)],
                            at[:, ds(lo, 1024 - lo)])
                    ats.append(at)
                    if kb == 1 and pending_tail is not None:
                        pending_tail()
                    # AV groups run a couple of kbs behind so the
                    # scores->exp->mask chain of the diagonal tile never
                    # stalls the PE queue
                    pe_rows = 1024 - lo
                    if kb - hf * 8 - av_delay >= 0:
                        av_group(kb - hf * 8 - av_delay)
                        pe_rows += 65 * (kb - av_delay + 1)
                    balance[0] += ((1024 - lo) * 0.833 + 185
                                   - pe_rows * 0.4167 - 150)

                def tail():
                    for qb_l in range(8 - av_delay, 8):
                        av_group(qb_l)
                return tail

            # --- vals transpose + output projection for one q-half --------
            def transposes_half(hf, qbs=range(8), hps=(0, 1)):
                qs = hf * 1024
                for qb_l in qbs:
                    qb_g = hf * 8 + qb_l
                    for hp in hps:
                        pst = psBig.tile(
                            [128, 128], BF16, tag="psB", name=f"tr{hf}{qb_l}{hp}")
                        nc.tensor.transpose(
                            pst[:], vals_sb[:, qb_g, ds(hp * 2, 2), :], ident[:])
                        nc.vector.tensor_copy(
                            valsT_sb[:, hp, ds(qs + qb_l * 128, 128)], pst[:])

            _ot_state = {}

            def outproj_ob(hf, ob):
                qs = hf * 1024
                obg, obi = divmod(ob, 4)
                if (hf, obg) not in _ot_state:
                    _ot_state[(hf, obg)] = outp.tile(
                        [128, 4, 1024], BF16, tag="out", name=f"ot{hf}{obg}")
                ot = _ot_state[(hf, obg)]
                ps = psBig.tile([128, 1024], F32, tag="psB", name=f"op{hf}{ob}")
                for kc in range(2):
                    lhsT = wo_sb[:, kc, ts(ob, 128)]
                    for nn in range(2):
                        nc.tensor.matmul(
                            ps[:, ts(nn, 512)], lhsT,
                            valsT_sb[:, kc, ds(qs + nn * 512, 512)],
                            start=(kc == 0), stop=(kc == 1),
                        )
                # Pool cannot read PSUM (walrus restriction): hf0 copies go
                # on DVE; hf1 copies (tail, ACT idle) alternate DVE/ACT
                if hf == 1 and obi % 2 == 0:
                    nc.scalar.activation(ot[:, obi, :], ps[:], AF.Copy)
                else:
                    nc.vector.tensor_copy(ot[:, obi, :], ps[:])
                if hf == 1:
                    # per-ob DMAs at the tail, alternating queues, so the
                    # last transfer is small and starts as late as possible
                    dma_eng = nc.sync if obi % 2 == 0 else nc.scalar
                    dma_eng.dma_start(
                        outT[ds(ob * 128, 128), ds(qs, 1024)],
                        ot[:, obi, :],
                    )
                elif obi % 2 == 1:
                    dma_eng = nc.sync if obg % 2 == 0 else nc.scalar
                    dma_eng.dma_start(
                        outT[ds(obg * 512 + (obi - 1) * 128, 256),
                             ds(qs, 1024)].rearrange(
                            "(ob p) q -> p ob q", p=128),
                        ot[:, ds(obi - 1, 2), :],
                    )

            def outproj_half(hf):
                for ob in range(8):
                    outproj_ob(hf, ob)

            def qk_unit(cc, sh, h, hf, kb):
                """A projection chunk as one deadline-pinned filler unit
                (the open psum accumulation group cannot be interleaved with
                other pool allocations, so the chunk stays monolithic)."""
                return [f"qk{cc}{sh}", 10 ** 6,
                        (lambda: qk_chunk(cc, sh)), (h, hf, kb)]

            def qk_unit2(cc, sh, h, hf, kb):
                """A projection chunk as TWO filler units at consecutive kb
                deadlines. Safe against psum-slot recycling because only one
                pool allocation (that kb's score tile) lands between them,
                which is below the pool depth."""
                state = {}

                def half(d0, nd=4):
                    def f():
                        if "ps" not in state:
                            state["ps"] = psBig.tile(
                                [128, 1024], F32, tag="psB",
                                name=f"qkps{cc}{sh}")
                        ps = state["ps"]
                        for dc in range(d0, d0 + nd):
                            lhsT = wqk_sb[:, dc, ts(cc, 128)]
                            for nn in range(2):
                                nc.tensor.matmul(
                                    ps[:, ts(nn, 512)], lhsT,
                                    xt_sb[:, dc, ds(sh * 1024 + nn * 512, 512)],
                                    start=(dc == 0), stop=(dc == 7),
                                )
                        if d0 + nd == 8:
                            nc.vector.tensor_scalar(
                                qk_sb[:, cc, ds(sh * 1024, 1024)], ps[:],
                                b_sb[:, cc : cc + 1], None, ADD,
                            )
                    return f

                nkb_h = 8 if hf == 0 else 16
                return [
                    [f"qk{cc}{sh}_{d0}", 10 ** 6, half(d0, 2),
                     (h, hf, min(kb + (d0 // 2) * 2, nkb_h - 1))]
                    for d0 in (0, 2, 4, 6)
                ]

            # --- emission order: everything that is not on the critical
            # dependency prefix is deadline-pinned into the attention kb
            # loops so the in-order PE queue never runs dry.
            BIGC = 10 ** 6
            qk_chunk(0, 0)
            qk_chunk(1, 0)
            v_blocks(range(8))
            filler_q.extend(qk_unit2(2, 0, 0, 0, 3))
            filler_q.extend(qk_unit2(3, 0, 1, 0, 2))
            t = attn_head(0, 0)
            filler_q.extend(qk_unit2(0, 1, 2, 0, 2))
            filler_q.extend(qk_unit2(1, 1, 3, 0, 2))
            t = attn_head(1, 0, t)
            t = attn_head(2, 0, t)
            t = attn_head(3, 0, t)
            drain_all()
            filler_q.extend(
                [
                    [f"V{sb_i}", BIGC,
                     (lambda sb_i=sb_i: v_blocks([sb_i])),
                     (0, 1, sb_i - 5)]
                    for sb_i in range(8, 16)
                ]
                + [
                    ["tr0a", BIGC, lambda: transposes_half(0, qbs=range(0, 3)),
                     (1, 1, 2)],
                    ["tr0b", BIGC, lambda: transposes_half(0, qbs=range(3, 6)),
                     (1, 1, 3)],
                    ["tr0c", BIGC, lambda: transposes_half(0, qbs=range(6, 8)),
                     (1, 1, 4)],
                ]
                + qk_unit2(2, 1, 1, 1, 7)
            )
            t = attn_head(0, 1, t)
            t = attn_head(1, 1, t)
            drain_all()
            filler_q.append(qk_unit(3, 1, 2, 1, 6))
            filler_q.extend([
                # outproj(0) is the only arbitrarily-deferrable PE work:
                # spread it across the last two heads
                [f"op0{ob}", BIGC, (lambda ob=ob: outproj_ob(0, ob)),
                 (2, 1, 9 + 2 * ob) if ob < 4 else (3, 1, 2 * ob - 4)]
                for ob in range(4)
            ])
            t = attn_head(2, 1, t)
            filler_q.extend([
                ["tr1a", BIGC,
                 lambda: transposes_half(1, qbs=range(0, 4), hps=(0,)),
                 (3, 1, 2)],
                ["tr1b", BIGC,
                 lambda: transposes_half(1, qbs=range(4, 8), hps=(0,)),
                 (3, 1, 3)],

            ] + [
                [f"op0{ob}", BIGC, (lambda ob=ob: outproj_ob(0, ob)),
                 (3, 1, 2 * ob - 4)]
                for ob in range(4, 8)
            ])
            t = attn_head(3, 1, t, av_delay=2)
            t()
            drain_all()
            transposes_half(1, hps=(1,))
            outproj_half(1)
            if dump:
                nc.sync.dma_start(d_qk[:], qk_sb[:])
                nc.sync.dma_start(d_v[:], v_sb[:])
                nc.sync.dma_start(d_vals[:], vals_sb[:])
                nc.sync.dma_start(d_valsT[:], valsT_sb[:])

    nc.compile()
    return nc


def _prep_inputs_causal(x, mask, W_qkv, b_qkv, W_o, b_o):
    scale = np.float32(1.0 / np.sqrt(HD))
    xT = [np.ascontiguousarray(x[b].T).astype(NPBF16) for b in range(B)]
    in_maps = []
    for c in range(NCORES):
        b, g = divmod(c, HPC)
        heads = [HPC * g + i for i in range(HPC)]
        qc = [W_qkv[:, 192 * h : 192 * h + 64] * scale for h in heads]
        kc = [W_qkv[:, 192 * h + 64 : 192 * h + 128] for h in heads]
        # chunk order: q01 | k01 | q23 | k23; each chunk prepacked so the
        # per-partition row (dc, c) is contiguous in DRAM
        chunks = [
            np.concatenate([qc[0], qc[1]], axis=1),
            np.concatenate([kc[0], kc[1]], axis=1),
            np.concatenate([qc[2], qc[3]], axis=1),
            np.concatenate([kc[2], kc[3]], axis=1),
        ]
        wqk = np.stack([
            c.reshape(8, 128, 128).transpose(1, 0, 2).reshape(128, 1024)
            for c in chunks
        ])
        bq = [b_qkv[192 * h : 192 * h + 64] * scale for h in heads]
        bk = [b_qkv[192 * h + 64 : 192 * h + 128] for h in heads]
        bqk_t = np.concatenate(
            [bq[0], bq[1], bk[0], bk[1], bq[2], bq[3], bk[2], bk[3]]
        ).reshape(4, 128).T
        wv = np.concatenate(
            [W_qkv[:, 192 * h + 128 : 192 * h + 192] for h in heads], axis=1)
        bv = np.concatenate(
            [b_qkv[192 * h + 128 : 192 * h + 192] for h in heads])[None, :]
        wo = W_o[256 * g : 256 * (g + 1), :]
        in_maps.append({
            "xT": xT[b],
            "wqk": np.ascontiguousarray(wqk).astype(NPBF16),
            "bqk": np.ascontiguousarray(bqk_t).astype(np.float32),
            "wv": np.ascontiguousarray(wv).astype(NPBF16),
            "bv": np.ascontiguousarray(bv).astype(NPBF16),
            "wo": np.ascontiguousarray(wo).astype(NPBF16),
        })
    return in_maps


# ---------------------------------------------------------------------------
# Generic additive-mask fallback (previous kernel)
# ---------------------------------------------------------------------------

def _classify_mask(maskT):
    """pattern[hf][kb][nn] for [128, 512] tiles of mask^T (k major, q minor)."""
    pat = []
    for hf in range(2):
        rows = []
        for kb in range(16):
            ents = []
            for nn in range(2):
                blk = maskT[kb * 128 : (kb + 1) * 128,
                            hf * 1024 + nn * 512 : hf * 1024 + (nn + 1) * 512]
                if np.all(blk <= -1e8):
                    ents.append(SKIP)
                elif np.all(blk == 0.0):
                    ents.append(FULL)
                else:
                    ents.append(PART)
            rows.append(tuple(ents))
        pat.append(tuple(rows))
    return tuple(tuple(r) for r in pat)


def _build_generic(pattern):
    nc = bacc.Bacc(None, target_bir_lowering=False, debug=False)
    xT = nc.dram_tensor("xT", [D, S], F32R, kind="ExternalInput")
    wqk = nc.dram_tensor("wqk", [D, 512], F32R, kind="ExternalInput")
    bqk = nc.dram_tensor("bqk", [128, 4], F32R, kind="ExternalInput")
    wv = nc.dram_tensor("wv", [D, 256], F32R, kind="ExternalInput")
    bv = nc.dram_tensor("bv", [1, 256], F32R, kind="ExternalInput")
    wo = nc.dram_tensor("wo", [256, D], F32R, kind="ExternalInput")
    maskT = nc.dram_tensor("maskT", [S, S], F32R, kind="ExternalInput")
    outT = nc.dram_tensor("outT", [D, S], F32, kind="ExternalOutput")

    with tile.TileContext(nc) as tc:
        with (
            tc.tile_pool(name="big", bufs=1) as bigp,
            tc.tile_pool(name="wqkp", bufs=1) as wqkp,
            tc.tile_pool(name="wvp", bufs=1) as wvp,
            tc.tile_pool(name="wop", bufs=1) as wop,
            tc.tile_pool(name="qkp", bufs=1) as qkp,
            tc.tile_pool(name="vp", bufs=1) as vp,
            tc.tile_pool(name="valsp", bufs=1) as valsp,
            tc.tile_pool(name="attnp", bufs=3) as attnp,
            tc.tile_pool(name="maskp", bufs=1) as maskp,
            tc.tile_pool(name="smallp", bufs=1) as smallp,
            tc.tile_pool(name="constp", bufs=1) as constp,
        ):
            ones_t = constp.tile([1, 128], F32R)
            nc.gpsimd.memset(ones_t[:].bitcast(F32), 1.0)
            b_sb = constp.tile([128, 4], F32R)
            nc.sync.dma_start(b_sb[:], bqk[:])
            bv_sb = constp.tile([1, 256], F32R)
            nc.sync.dma_start(bv_sb[:], bv[:])

            wqk_sb = wqkp.tile([128, 8, 512], F32R, tag="wqkslot")
            xt_sb = bigp.tile([128, 8, S], F32R, tag="big")
            wv_sb = wvp.tile([128, 8, 256], F32R)
            for dc in range(8):
                nc.sync.dma_start(wqk_sb[:, dc, :], wqk[ds(dc * 128, 128), :])
                nc.sync.dma_start(
                    xt_sb[:, dc, ds(0, 1024)], xT[ds(dc * 128, 128), ds(0, 1024)]
                )
            nc.sync.dma_start(wv_sb[:], wv[:].rearrange("(dc p) c -> p dc c", p=128))

            qk_sb = qkp.tile([128, 4, S], F32R)
            v_sb = vp.tile([128, 16, HPC, 65], F32R)

            nc.gpsimd.memset(v_sb[:, :, :, 64:65].bitcast(F32), 1.0)

            def n_full(hf):
                return sum(c == FULL for kb in pattern[hf] for c in kb)

            hf_order = sorted(range(2), key=lambda hf: n_full(hf))
            part_blocks, slots, mask_tiles = {}, {}, {}
            for hf_i, hf in enumerate(hf_order):
                pb = [
                    (kb, nn)
                    for kb in range(16) for nn in range(2)
                    if pattern[hf][kb][nn] == PART
                ]
                part_blocks[hf] = pb
                slots[hf] = {blk: i for i, blk in enumerate(pb)}
                if len(pb) == 0:
                    mask_tiles[hf] = None
                elif len(pb) <= 8:
                    if hf_i == 0:
                        mask_tiles[hf] = maskp.tile(
                            [128, 8, 512], F32R, tag="mask", name=f"mask{hf}")
                    else:
                        mask_tiles[hf] = wqkp.tile(
                            [128, 8, 512], F32R, tag="wqkslot", name=f"mask{hf}")
                else:
                    mask_tiles[hf] = bigp.tile(
                        [128, 32, 512], F32R, tag="big", name=f"mask{hf}")

            def mask_dmas(hf, qs_):
                pb = part_blocks[hf]
                i = 0
                while i < len(pb):
                    kb0, nn0 = pb[i]
                    j = i + 1
                    while (j < len(pb)
                           and pb[j] == (pb[j - 1][0] + 1, nn0)):
                        j += 1
                    n = j - i
                    nc.sync.dma_start(
                        mask_tiles[hf][:, i : i + n, :],
                        maskT[ds(kb0 * 128, n * 128),
                              ds(qs_ + nn0 * 512, 512)].rearrange(
                            "(b p) q -> p b q", p=128),
                    )
                    i = j
            hf_first = hf_order[0]
            if mask_tiles.get(hf_first) is not None and len(part_blocks[hf_first]) <= 8:
                mask_dmas(hf_first, hf_first * 1024)
            for dc in range(8):
                nc.sync.dma_start(
                    xt_sb[:, dc, ds(1024, 1024)],
                    xT[ds(dc * 128, 128), ds(1024, 1024)],
                )
            wo_sb = wop.tile([128, 2, D], F32R)
            nc.sync.dma_start(wo_sb[:], wo[:].rearrange("(kc p) d -> p kc d", p=128))

            def qk_chunk_half(pool, cc, sh):
                ps = pool.tile([128, 1024], F32, tag="psqk", name=f"qkps{cc}{sh}")
                for dc in range(8):
                    lhsT = wqk_sb[:, dc, ts(cc, 128)]
                    for nn in range(2):
                        nc.tensor.matmul(
                            ps[:, ts(nn, 512)], lhsT,
                            xt_sb[:, dc, ds(sh * 1024 + nn * 512, 512)],
                            start=(dc == 0), stop=(dc == 7),
                        )
                nc.scalar.activation(
                    qk_sb[:, cc, ds(sh * 1024, 1024)], ps[:], AF.Identity,
                    bias=b_sb[:, cc : cc + 1],
                )

            def v_blocks(pool, rng):
                for sb_i in rng:
                    ps = pool.tile([128, 256], F32, tag="psv", name=f"vps{sb_i}")
                    for dc in range(8):
                        nc.tensor.matmul(
                            ps[:], xt_sb[:, dc, ts(sb_i, 128)], wv_sb[:, dc, :],
                            start=(dc == 0), stop=False,
                        )
                    nc.tensor.matmul(ps[:], ones_t[:], bv_sb[:], start=False, stop=True)
                    nc.vector.tensor_copy(
                        v_sb[:, sb_i, :, 0:64],
                        ps[:].rearrange("p (h e) -> p h e", h=HPC),
                    )

            with (
                tc.tile_pool(name="psqk", bufs=2, space="PSUM") as psqk,
                tc.tile_pool(name="psv", bufs=2, space="PSUM") as psv,
            ):
                for cc in (0, 2, 1, 3):
                    qk_chunk_half(psqk, cc, 0)
                v_blocks(psv, range(8))
                for cc in (0, 2, 1, 3):
                    qk_chunk_half(psqk, cc, 1)
                v_blocks(psv, range(8, 16))

            with (
                tc.tile_pool(name="pssc", bufs=3, space="PSUM") as pssc,
                tc.tile_pool(name="psav", bufs=1, space="PSUM") as psav,
            ):
                def outproj(vals_t, qs_):
                    for ob in range(8):
                        ps = pssc.tile([128, 1024], F32, tag="pssc")
                        for kc in range(2):
                            lhsT = wo_sb[:, kc, ts(ob, 128)]
                            for nn in range(2):
                                nc.tensor.matmul(
                                    ps[:, ts(nn, 512)], lhsT,
                                    vals_t[:, kc, ts(nn, 512)],
                                    start=(kc == 0), stop=(kc == 1),
                                )
                        oev = attnp.tile([128, 1024], F32, tag="attn")
                        nc.scalar.activation(oev[:], ps[:], AF.Copy)
                        nc.sync.dma_start(
                            outT[ds(ob * 128, 128), ds(qs_, 1024)], oev[:]
                        )

                pending = None
                for hf_i, hf in enumerate(hf_order):
                    qs = hf * 1024
                    slot = slots[hf]
                    mask_sb = mask_tiles[hf]
                    if mask_sb is not None and (
                        hf_i > 0 or len(part_blocks[hf]) > 8
                    ):
                        mask_dmas(hf, qs)
                    vals_sb = valsp.tile(
                        [128, 2, 1024], F32R, tag="vals", name=f"vals{hf}")
                    kb_order = sorted(
                        (kb for kb in range(16) if pattern[hf][kb] != (SKIP, SKIP)),
                        key=lambda kb: (PART in pattern[hf][kb], kb),
                    )
                    for h in range(HPC):
                        off = 64 * (h % 2)
                        qt = qk_sb[off : off + 64, h // 2, :]
                        kt = qk_sb[off : off + 64, 2 + h // 2, :]
                        act_kbs = [
                            [kb for kb in kb_order if pattern[hf][kb][nn] != SKIP]
                            for nn in range(2)
                        ]
                        ps_av = psav.tile([65, 1024], F32, tag="psav")
                        for kb in kb_order:
                            cls = pattern[hf][kb]
                            ps_sc = pssc.tile([128, 1024], F32, tag="pssc")
                            lhsT = kt[:, ts(kb, 128)]
                            at = attnp.tile([128, 1024], F32R, tag="attn")
                            for nn in range(2):
                                if cls[nn] == SKIP:
                                    continue
                                nc.tensor.matmul(
                                    ps_sc[:, ts(nn, 512)], lhsT,
                                    qt[:, ds(qs + nn * 512, 512)],
                                    start=True, stop=True,
                                )
                            if cls == (FULL, FULL):
                                nc.scalar.activation(at[:], ps_sc[:], AF.Exp)
                            else:
                                for nn in range(2):
                                    if cls[nn] == SKIP:
                                        continue
                                    if cls[nn] == PART:
                                        nc.vector.tensor_tensor(
                                            at[:, ts(nn, 512)], ps_sc[:, ts(nn, 512)],
                                            mask_sb[:, slot[(kb, nn)], :], ADD,
                                        )
                                        nc.scalar.activation(
                                            at[:, ts(nn, 512)], at[:, ts(nn, 512)],
                                            AF.Exp,
                                        )
                                    else:
                                        nc.scalar.activation(
                                            at[:, ts(nn, 512)], ps_sc[:, ts(nn, 512)],
                                            AF.Exp,
                                        )
                            for nn in range(2):
                                if cls[nn] == SKIP:
                                    continue
                                nc.tensor.matmul(
                                    ps_av[:, ts(nn, 512)], v_sb[:, kb, h, :],
                                    at[:, ts(nn, 512)],
                                    start=(kb == act_kbs[nn][0]),
                                    stop=(kb == act_kbs[nn][-1]),
                                )
                        recip = smallp.tile([1, 1024], F32R, tag="recip")
                        with nc.allow_low_precision(
                            reason="float32r has fp32 bits; only PE matmul mode differs"
                        ):
                            nc.vector.reciprocal(recip[:], ps_av[64:65, :])
                        bc_sb = smallp.tile([64, 1024], F32R, tag="bc")
                        nc.gpsimd.partition_broadcast(bc_sb[:], recip[:])
                        nc.vector.tensor_tensor(
                            vals_sb[off : off + 64, h // 2, :],
                            ps_av[0:64, :], bc_sb[:], MULT,
                        )
                        if pending is not None and hf_i == 1 and h == 0:
                            outproj(*pending)
                            pending = None

                    if hf_i == 0 and len(hf_order) > 1:
                        pending = (vals_sb, qs)
                    else:
                        outproj(vals_sb, qs)
                if pending is not None:
                    outproj(*pending)
                    pending = None

    nc.compile()
    return nc


def _prep_inputs_generic(x, mask, W_qkv, b_qkv, W_o, b_o):
    scale = np.float32(1.0 / np.sqrt(HD))
    xT = [np.ascontiguousarray(x[b].T) for b in range(B)]
    maskT = np.ascontiguousarray(mask.T)
    in_maps = []
    for c in range(NCORES):
        b, g = divmod(c, HPC)
        heads = [HPC * g + i for i in range(HPC)]
        qcols = np.concatenate(
            [W_qkv[:, 192 * h : 192 * h + 64] for h in heads], axis=1) * scale
        kcols = np.concatenate(
            [W_qkv[:, 192 * h + 64 : 192 * h + 128] for h in heads], axis=1)
        wqk = np.ascontiguousarray(np.concatenate([qcols, kcols], axis=1))
        bq = np.concatenate([b_qkv[192 * h : 192 * h + 64] for h in heads]) * scale
        bk = np.concatenate([b_qkv[192 * h + 64 : 192 * h + 128] for h in heads])
        bqk_t = np.ascontiguousarray(
            np.concatenate([bq, bk]).reshape(4, 128).T)
        wv = np.ascontiguousarray(np.concatenate(
            [W_qkv[:, 192 * h + 128 : 192 * h + 192] for h in heads], axis=1))
        bv = np.ascontiguousarray(np.concatenate(
            [b_qkv[192 * h + 128 : 192 * h + 192] for h in heads])[None, :])
        wo = np.ascontiguousarray(W_o[256 * g : 256 * (g + 1), :])
        in_maps.append({
            "xT": xT[b], "wqk": wqk, "bqk": bqk_t, "wv": wv, "bv": bv,
            "wo": wo, "maskT": maskT,
        })
    return in_maps


# ---------------------------------------------------------------------------
# Entry point
# ---------------------------------------------------------------------------

def _prep_inputs(x, mask, W_qkv, b_qkv, W_o, b_o):
    if _is_causal(mask):
        return _prep_inputs_causal(x, mask, W_qkv, b_qkv, W_o, b_o)
    return _prep_inputs_generic(x, mask, W_qkv, b_qkv, W_o, b_o)


def kernel(x, mask, W_qkv, b_qkv, W_o, b_o):
    x = np.asarray(x, dtype=np.float32)
    mask = np.asarray(mask, dtype=np.float32)
    W_qkv = np.asarray(W_qkv, dtype=np.float32)
    b_qkv = np.asarray(b_qkv, dtype=np.float32)
    W_o = np.asarray(W_o, dtype=np.float32)
    b_o = np.asarray(b_o, dtype=np.float32)

    if _is_causal(mask):
        key = "causal"
        if key not in _CACHE:
            _CACHE[key] = _build_causal()
        nc = _CACHE[key]
        in_maps = _prep_inputs_causal(x, mask, W_qkv, b_qkv, W_o, b_o)
    else:
        pattern = _classify_mask(np.ascontiguousarray(mask.T))
        key = ("nc", pattern)
        if key not in _CACHE:
            _CACHE[key] = _build_generic(pattern)
        nc = _CACHE[key]
        in_maps = _prep_inputs_generic(x, mask, W_qkv, b_qkv, W_o, b_o)
    _CACHE["nc"] = nc

    res = run_bass_kernel_spmd(nc, in_maps, core_ids=list(range(NCORES)))
    _CACHE["last_result"] = res

    out = np.empty((B, S, D), dtype=np.float32)
    for b in range(B):
        acc = res.results[HPC * b]["outT"].astype(np.float32)
        for g in range(1, HPC):
            acc = acc + res.results[HPC * b + g]["outT"].astype(np.float32)
        out[b] = acc.T + b_o
    return out


# revision 117
# speedup vs baseline: 1.5732x; 1.0027x over previous
"""Multi-head attention (B=2, S=2048, D=1024, H=16) on 8 TRN2 NeuronCores.

Sharding: core c handles batch b = c//4 and head-group g = c%4 (4 heads each).
Each core computes its heads' attention and a partial output projection
(row-parallel W_o); the host sums the 4 partials per batch and adds b_o.

Fast path (causal mask): bf16 operands off-PSUM, live-span score matmuls
(fully-masked columns never computed), masking via affine_select on the one
diagonal 128x128 block per score tile (no mask tensor at all), and a
transposed AV: attention tiles act as the stationary operand so each
accumulation step emits only 65 output columns ([q,64] values + a ones-column
denominator that lands as a per-partition scalar). vals are transposed back
with PE-transpose against an on-device identity for the row-parallel output
projection. DMAs are batched and spread across the SP/DVE/Pool queues.

A generic additive-mask fallback (the previous kernel) is kept for non-causal
masks.
"""

import numpy as np
import ml_dtypes

import concourse.bass as bass  # noqa: F401
import concourse.mybir as mybir
import concourse.tile as tile
from concourse import bacc
from concourse.bass import ds, ts
from concourse.bass_utils import run_bass_kernel_spmd

B, S, D, H = 2, 2048, 1024, 16
HD = D // H  # 64
HPC = 4      # heads per core
NCORES = 8
F32R = mybir.dt.float32r
F32 = mybir.dt.float32
BF16 = mybir.dt.bfloat16
NPBF16 = ml_dtypes.bfloat16
AF = mybir.ActivationFunctionType
ADD = mybir.AluOpType.add
MULT = mybir.AluOpType.mult

SKIP, FULL, PART = 0, 1, 2

_CACHE = {}


def _is_causal(mask):
    tril = np.tril(np.ones((S, S), dtype=bool))
    return bool(np.all(mask[tril] == 0.0) and np.all(mask[~tril] <= -1e8))


# ---------------------------------------------------------------------------
# Causal fast path
# ---------------------------------------------------------------------------

def _build_causal(dump=False):
    nc = bacc.Bacc(None, target_bir_lowering=False, debug=False)
    xT = nc.dram_tensor("xT", [D, S], BF16, kind="ExternalInput")
    # wqk host-prepacked per chunk: wqk[cc][p, dc*128 + c] = W[dc*128+p, cc*128+c]
    wqk = nc.dram_tensor("wqk", [4, 128, 1024], BF16, kind="ExternalInput")
    bqk = nc.dram_tensor("bqk", [128, 4], F32, kind="ExternalInput")
    wv = nc.dram_tensor("wv", [D, 256], BF16, kind="ExternalInput")
    bv = nc.dram_tensor("bv", [1, 256], BF16, kind="ExternalInput")
    wo = nc.dram_tensor("wo", [256, D], BF16, kind="ExternalInput")
    outT = nc.dram_tensor("outT", [D, S], BF16, kind="ExternalOutput")
    if dump:
        d_qk = nc.dram_tensor("d_qk", [128, 4, S], BF16, kind="ExternalOutput")
        d_v = nc.dram_tensor("d_v", [128, 16, HPC, 65], BF16, kind="ExternalOutput")
        d_vals = nc.dram_tensor(
            "d_vals", [128, 16, HPC, 64], BF16, kind="ExternalOutput")
        d_valsT = nc.dram_tensor("d_valsT", [128, 2, S], BF16, kind="ExternalOutput")
        d_at = nc.dram_tensor("d_at", [128, 16, 1024], BF16, kind="ExternalOutput")
        d_den = nc.dram_tensor("d_den", [128, 16, 65], F32, kind="ExternalOutput")

    with tile.TileContext(nc) as tc:
        with (
            tc.tile_pool(name="bigp", bufs=1) as bigp,
            tc.tile_pool(name="constp", bufs=1) as constp,
            tc.tile_pool(name="atp", bufs=20) as atp,
            tc.tile_pool(name="outp", bufs=2) as outp,
            tc.tile_pool(name="smallp", bufs=4) as smallp,
            tc.tile_pool(name="psBig", bufs=3, space="PSUM") as psBig,
            tc.tile_pool(name="psC", bufs=2, space="PSUM") as psC,
        ):
            # --- constants (small DMAs issue before the memset chain so
            # they hit the DMA device first) ---------------------------------
            b_sb = constp.tile([128, 4], F32)
            nc.gpsimd.dma_start(b_sb[:], bqk[:])
            bv_sb = constp.tile([1, 256], BF16)
            nc.gpsimd.dma_start(bv_sb[:], bv[:])
            # broadcast the V bias across partitions once so the projection
            # bias-add rides the PSUM->SBUF copy instead of a rank-1 matmul
            bv32 = constp.tile([1, 256], F32)
            nc.vector.tensor_copy(bv32[:], bv_sb[:])
            bv_bc = constp.tile([128, 256], F32)
            nc.gpsimd.partition_broadcast(bv_bc[:], bv32[:])
            ident = constp.tile([128, 128], BF16)
            nc.gpsimd.memset(ident[:], 1.0)
            nc.gpsimd.affine_select(
                out=ident[:], in_=ident[:],
                compare_op=mybir.AluOpType.is_equal, fill=0.0,
                base=0, pattern=[[-1, 128]], channel_multiplier=1,
            )
            # upper-triangular (incl. diagonal) ones: keeps k <= q when
            # multiplied into the diagonal block of an exp'd score tile
            tri_t = constp.tile([128, 128], BF16)
            nc.gpsimd.memset(tri_t[:], 1.0)
            nc.gpsimd.affine_select(
                out=tri_t[:], in_=tri_t[:],
                compare_op=mybir.AluOpType.is_ge, fill=0.0,
                base=0, pattern=[[1, 128]], channel_multiplier=-1,
            )

            # --- big SBUF tensors -----------------------------------------
            wqk_sb = constp.tile([128, 8, 512], BF16)
            xt_sb = bigp.tile([128, 8, S], BF16)
            wv_sb = constp.tile([128, 8, 256], BF16)
            wo_sb = constp.tile([128, 2, D], BF16)
            qk_sb = bigp.tile([128, 4, S], BF16)
            v_sb = bigp.tile([128, 16, HPC, 65], BF16)
            nc.gpsimd.memset(v_sb[:, :, :, 64:65], 1.0)
            vals_sb = bigp.tile([128, 16, HPC, 64], BF16)
            valsT_sb = bigp.tile([128, 2, S], BF16)

            # --- input DMAs ------------------------------------------------
            # wqk on the ACT queue (idle until the first exp); chunks are
            # host-prepacked contiguous so descriptors stay 2KB
            for cc in range(2):
                nc.scalar.dma_start(
                    wqk_sb[:, :, ts(cc, 128)],
                    wqk[cc].rearrange("p (dc c) -> p dc c", c=128),
                )
            nc.scalar.dma_start(
                wv_sb[:], wv[:].rearrange("(dc p) c -> p dc c", p=128))
            for cc in range(2, 4):
                nc.scalar.dma_start(
                    wqk_sb[:, :, ts(cc, 128)],
                    wqk[cc].rearrange("p (dc c) -> p dc c", c=128),
                )
            # xT + wv + wo on the SP queue: s-half-major then dc-chunks;
            # the first chunk is small so the first projection matmuls can
            # start as early as possible
            for d0, nd in ((0, 1), (1, 1), (2, 2), (4, 4)):
                nc.sync.dma_start(
                    xt_sb[:, ds(d0, nd), ds(0, 1024)],
                    xT[ds(d0 * 128, nd * 128), ds(0, 1024)].rearrange(
                        "(dc p) s -> p dc s", p=128),
                )
            for dh in range(2):
                nc.sync.dma_start(
                    xt_sb[:, ds(dh * 4, 4), ds(1024, 1024)],
                    xT[ds(dh * 512, 512), ds(1024, 1024)].rearrange(
                        "(dc p) s -> p dc s", p=128),
                )
            nc.sync.dma_start(
                wo_sb[:], wo[:].rearrange("(kc p) d -> p kc d", p=128))

            # --- projection helpers ---------------------------------------
            def qk_chunk(cc, sh):
                """qk_sb[:, cc, sh-half] = wqk-chunk-cc^T @ xT-half + bias."""
                ps = psBig.tile([128, 1024], F32, tag="psB", name=f"qkps{cc}{sh}")
                for dc in range(8):
                    lhsT = wqk_sb[:, dc, ts(cc, 128)]
                    for nn in range(2):
                        nc.tensor.matmul(
                            ps[:, ts(nn, 512)], lhsT,
                            xt_sb[:, dc, ds(sh * 1024 + nn * 512, 512)],
                            start=(dc == 0), stop=(dc == 7),
                        )
                nc.vector.tensor_scalar(
                    qk_sb[:, cc, ds(sh * 1024, 1024)], ps[:],
                    b_sb[:, cc : cc + 1], None, ADD,
                )

            def v_blocks(rng):
                for sb_i in rng:
                    ps = psBig.tile([128, 256], F32, tag="psB", name=f"vps{sb_i}")
                    for dc in range(8):
                        nc.tensor.matmul(
                            ps[:], xt_sb[:, dc, ts(sb_i, 128)], wv_sb[:, dc, :],
                            start=(dc == 0), stop=(dc == 7),
                        )
                    nc.vector.tensor_tensor(
                        v_sb[:, sb_i, :, 0:64],
                        ps[:].rearrange("p (h e) -> p h e", h=HPC),
                        bv_bc[:].rearrange("p (h e) -> p h e", h=HPC),
                        ADD,
                    )

            # --- attention for one (head, q-half) -------------------------
            # Filler machinery: the engines execute in-order, so PE starvation
            # during exp-bound attention stretches can only be avoided by
            # interleaving independent PE work (projections, outproj) into
            # the kb loops at fine granularity, paced by the ACT/PE balance.
            filler_q = []   # list of [name, pe_cost_ns, fn, deadline|None]
            balance = [0.0]

            def pop_fillers(here=None):
                # deadline-forced pops (emit everything up to the overdue
                # entry to preserve dependency order), then balance pops
                if here is not None:
                    h, hf, kb = here
                    due = None
                    for i, ent in enumerate(filler_q):
                        dl = ent[3]
                        if dl is not None and dl[0] == h and dl[1] == hf \
                                and kb >= dl[2]:
                            due = i
                    if due is not None:
                        for ent in filler_q[: due + 1]:
                            ent[2]()
                            balance[0] -= ent[1]
                        del filler_q[: due + 1]
                while filler_q and balance[0] >= filler_q[0][1] * 0.4:
                    name, cost, fn, _ = filler_q.pop(0)
                    fn()
                    balance[0] -= cost

            def drain_until(name):
                while filler_q:
                    nm, cost, fn, _ = filler_q.pop(0)
                    fn()
                    balance[0] -= cost
                    if nm == name:
                        break

            def drain_all():
                while filler_q:
                    _, cost, fn, _ = filler_q.pop(0)
                    fn()
                    balance[0] -= cost

            def attn_head(h, hf, pending_tail=None, av_delay=3):
                qs = hf * 1024
                off = 64 * (h % 2)
                qt = qk_sb[off : off + 64, 2 * (h // 2), :]
                kt = qk_sb[off : off + 64, 2 * (h // 2) + 1, :]
                nkb = 8 if hf == 0 else 16
                ats = []

                def av_group(qb_l):
                    # AV, qb-major: at-tiles are the stationary operand, so
                    # each accumulation step emits only 65 output columns; the
                    # ones column of V lands the denominator per-partition.
                    qb_g = hf * 8 + qb_l
                    pv = psC.tile([128, 65], F32, tag="psC", name=f"pav{h}{hf}{qb_l}")
                    for kb2 in range(qb_g + 1):
                        nc.tensor.matmul(
                            pv[:], ats[kb2][:, ts(qb_l, 128)], v_sb[:, kb2, h, :],
                            start=(kb2 == 0), stop=(kb2 == qb_g),
                        )
                    if dump and h == 0:
                        dtmp = smallp.tile([128, 65], F32, tag="dtmp",
                                           name=f"dd{hf}{qb_l}")
                        nc.vector.tensor_copy(dtmp[:], pv[:])
                        nc.sync.dma_start(d_den[:, qb_g, :], dtmp[:])
                    recip = smallp.tile([128, 1], F32, tag="recip")
                    nc.vector.reciprocal(recip[:], pv[:, 64:65])
                    nc.vector.tensor_scalar(
                        vals_sb[:, qb_g, h, :], pv[:, 0:64],
                        recip[:, 0:1], None, MULT,
                    )

                for kb in range(nkb):
                    if kb >= 1:
                        pop_fillers(here=(h, hf, kb))
                    lo = max(0, kb * 128 - qs)
                    if lo < 512:
                        ps = psBig.tile([128, 1024], F32, tag="psB",
                                        name=f"sc{h}{hf}{kb}")
                        base = 0
                    else:
                        ps = psBig.tile([128, 512], F32, tag="psB",
                                        name=f"sc{h}{hf}{kb}")
                        base = 512
                    lhsT = kt[:, ts(kb, 128)]
                    for seg in range(2):
                        a = max(lo, seg * 512)
                        bnd = (seg + 1) * 512
                        if a < bnd:
                            nc.tensor.matmul(
                                ps[:, ds(a - base, bnd - a)], lhsT,
                                qt[:, ds(qs + a, bnd - a)],
                                start=True, stop=True,
                            )
                    at = atp.tile([128, 1024], BF16, tag="at")
                    nc.scalar.activation(
                        at[:, ds(lo, 1024 - lo)], ps[:, ds(lo - base, 1024 - lo)],
                        AF.Exp)
                    if kb * 128 >= qs:
                        dlo = kb * 128 - qs
                        nc.gpsimd.tensor_tensor(
                            at[:, ds(dlo, 128)], at[:, ds(dlo, 128)],
                            tri_t[:], MULT,
                        )
                    if dump and h == 0:
                        nc.sync.dma_start(
                            d_at[:, hf * 8 + kb if hf == 0 else kb,
                                 ds(lo, 1024 - lo)],
                            at[:, ds(lo, 1024 - lo)])
                    ats.append(at)
                    if kb == 1 and pending_tail is not None:
                        pending_tail()
                    # AV groups run a couple of kbs behind so the
                    # scores->exp->mask chain of the diagonal tile never
                    # stalls the PE queue
                    pe_rows = 1024 - lo
                    if kb - hf * 8 - av_delay >= 0:
                        av_group(kb - hf * 8 - av_delay)
                        pe_rows += 65 * (kb - av_delay + 1)
                    balance[0] += ((1024 - lo) * 0.833 + 185
                                   - pe_rows * 0.4167 - 150)

                def tail():
                    for qb_l in range(8 - av_delay, 8):
                        av_group(qb_l)
                return tail

            # --- vals transpose + output projection for one q-half --------
            def transposes_half(hf, qbs=range(8), hps=(0, 1)):
                qs = hf * 1024
                for qb_l in qbs:
                    qb_g = hf * 8 + qb_l
                    for hp in hps:
                        pst = psBig.tile(
                            [128, 128], BF16, tag="psB", name=f"tr{hf}{qb_l}{hp}")
                        nc.tensor.transpose(
                            pst[:], vals_sb[:, qb_g, ds(hp * 2, 2), :], ident[:])
                        nc.vector.tensor_copy(
                            valsT_sb[:, hp, ds(qs + qb_l * 128, 128)], pst[:])

            _ot_state = {}

            def outproj_ob(hf, ob):
                qs = hf * 1024
                obg, obi = divmod(ob, 4)
                if (hf, obg) not in _ot_state:
                    _ot_state[(hf, obg)] = outp.tile(
                        [128, 4, 1024], BF16, tag="out", name=f"ot{hf}{obg}")
                ot = _ot_state[(hf, obg)]
                ps = psBig.tile([128, 1024], F32, tag="psB", name=f"op{hf}{ob}")
                for kc in range(2):
                    lhsT = wo_sb[:, kc, ts(ob, 128)]
                    for nn in range(2):
                        nc.tensor.matmul(
                            ps[:, ts(nn, 512)], lhsT,
                            valsT_sb[:, kc, ds(qs + nn * 512, 512)],
                            start=(kc == 0), stop=(kc == 1),
                        )
                # Pool cannot read PSUM (walrus restriction): hf0 copies go
                # on DVE; hf1 copies (tail, ACT idle) alternate DVE/ACT
                if hf == 1 and obi % 2 == 0:
                    nc.scalar.activation(ot[:, obi, :], ps[:], AF.Copy)
                else:
                    nc.vector.tensor_copy(ot[:, obi, :], ps[:])
                if hf == 1:
                    # per-ob DMAs at the tail, alternating queues, so the
                    # last transfer is small and starts as late as possible
                    dma_eng = nc.sync if obi % 2 == 0 else nc.scalar
                    dma_eng.dma_start(
                        outT[ds(ob * 128, 128), ds(qs, 1024)],
                        ot[:, obi, :],
                    )
                elif obi % 2 == 1:
                    dma_eng = nc.sync if obg % 2 == 0 else nc.scalar
                    dma_eng.dma_start(
                        outT[ds(obg * 512 + (obi - 1) * 128, 256),
                             ds(qs, 1024)].rearrange(
                            "(ob p) q -> p ob q", p=128),
                        ot[:, ds(obi - 1, 2), :],
                    )

            def outproj_half(hf):
                for ob in range(8):
                    outproj_ob(hf, ob)

            def qk_unit(cc, sh, h, hf, kb):
                """A projection chunk as one deadline-pinned filler unit
                (the open psum accumulation group cannot be interleaved with
                other pool allocations, so the chunk stays monolithic)."""
                return [f"qk{cc}{sh}", 10 ** 6,
                        (lambda: qk_chunk(cc, sh)), (h, hf, kb)]

            def qk_unit2(cc, sh, h, hf, kb):
                """A projection chunk as TWO filler units at consecutive kb
                deadlines. Safe against psum-slot recycling because only one
                pool allocation (that kb's score tile) lands between them,
                which is below the pool depth."""
                state = {}

                def half(d0, nd=4):
                    def f():
                        if "ps" not in state:
                            state["ps"] = psBig.tile(
                                [128, 1024], F32, tag="psB",
                                name=f"qkps{cc}{sh}")
                        ps = state["ps"]
                        for dc in range(d0, d0 + nd):
                            lhsT = wqk_sb[:, dc, ts(cc, 128)]
                            for nn in range(2):
                                nc.tensor.matmul(
                                    ps[:, ts(nn, 512)], lhsT,
                                    xt_sb[:, dc, ds(sh * 1024 + nn * 512, 512)],
                                    start=(dc == 0), stop=(dc == 7),
                                )
                        if d0 + nd == 8:
                            nc.vector.tensor_scalar(
                                qk_sb[:, cc, ds(sh * 1024, 1024)], ps[:],
                                b_sb[:, cc : cc + 1], None, ADD,
                            )
                    return f

                nkb_h = 8 if hf == 0 else 16
                return [
                    [f"qk{cc}{sh}_{d0}", 10 ** 6, half(d0, 2),
                     (h, hf, min(kb + (d0 // 2) * 2, nkb_h - 1))]
                    for d0 in (0, 2, 4, 6)
                ]

            # --- emission order: everything that is not on the critical
            # dependency prefix is deadline-pinned into the attention kb
            # loops so the in-order PE queue never runs dry.
            BIGC = 10 ** 6
            qk_chunk(0, 0)
            qk_chunk(1, 0)
            v_blocks(range(8))
            filler_q.extend(qk_unit2(2, 0, 0, 0, 1))
            filler_q.extend(qk_unit2(3, 0, 1, 0, 1))
            t = attn_head(0, 0)
            filler_q.extend(qk_unit2(0, 1, 2, 0, 1))
            filler_q.extend(qk_unit2(1, 1, 3, 0, 1))
            t = attn_head(1, 0, t)
            t = attn_head(2, 0, t)
            t = attn_head(3, 0, t)
            drain_all()
            filler_q.extend(
                [
                    [f"V{sb_i}", BIGC,
                     (lambda sb_i=sb_i: v_blocks([sb_i])),
                     (0, 1, sb_i - 5)]
                    for sb_i in range(8, 16)
                ]
                + [
                    ["tr0a", BIGC, lambda: transposes_half(0, qbs=range(0, 3)),
                     (1, 1, 2)],
                    ["tr0b", BIGC, lambda: transposes_half(0, qbs=range(3, 6)),
                     (1, 1, 3)],
                    ["tr0c", BIGC, lambda: transposes_half(0, qbs=range(6, 8)),
                     (1, 1, 4)],
                ]
                + qk_unit2(2, 1, 1, 1, 7)
            )
            t = attn_head(0, 1, t)
            t = attn_head(1, 1, t)
            drain_all()
            filler_q.append(qk_unit(3, 1, 2, 1, 6))
            filler_q.extend([
                # outproj(0) is the only arbitrarily-deferrable PE work:
                # spread it across the last two heads
                [f"op0{ob}", BIGC, (lambda ob=ob: outproj_ob(0, ob)),
                 (2, 1, 9 + 2 * ob) if ob < 4 else (3, 1, 2 * ob - 4)]
                for ob in range(4)
            ])
            t = attn_head(2, 1, t)
            filler_q.extend([
                ["tr1a", BIGC,
                 lambda: transposes_half(1, qbs=range(0, 4), hps=(0,)),
                 (3, 1, 2)],
                ["tr1b", BIGC,
                 lambda: transposes_half(1, qbs=range(4, 8), hps=(0,)),
                 (3, 1, 3)],

            ] + [
                [f"op0{ob}", BIGC, (lambda ob=ob: outproj_ob(0, ob)),
                 (3, 1, 2 * ob - 4)]
                for ob in range(4, 8)
            ])
            t = attn_head(3, 1, t, av_delay=2)
            t()
            drain_all()
            transposes_half(1, hps=(1,))
            outproj_half(1)
            if dump:
                nc.sync.dma_start(d_qk[:], qk_sb[:])
                nc.sync.dma_start(d_v[:], v_sb[:])
                nc.sync.dma_start(d_vals[:], vals_sb[:])
                nc.sync.dma_start(d_valsT[:], valsT_sb[:])

    nc.compile()
    return nc


def _prep_inputs_causal(x, mask, W_qkv, b_qkv, W_o, b_o):
    scale = np.float32(1.0 / np.sqrt(HD))
    xT = [np.ascontiguousarray(x[b].T).astype(NPBF16) for b in range(B)]
    in_maps = []
    for c in range(NCORES):
        b, g = divmod(c, HPC)
        heads = [HPC * g + i for i in range(HPC)]
        qc = [W_qkv[:, 192 * h : 192 * h + 64] * scale for h in heads]
        kc = [W_qkv[:, 192 * h + 64 : 192 * h + 128] for h in heads]
        # chunk order: q01 | k01 | q23 | k23; each chunk prepacked so the
        # per-partition row (dc, c) is contiguous in DRAM
        chunks = [
            np.concatenate([qc[0], qc[1]], axis=1),
            np.concatenate([kc[0], kc[1]], axis=1),
            np.concatenate([qc[2], qc[3]], axis=1),
            np.concatenate([kc[2], kc[3]], axis=1),
        ]
        wqk = np.stack([
            c.reshape(8, 128, 128).transpose(1, 0, 2).reshape(128, 1024)
            for c in chunks
        ])
        bq = [b_qkv[192 * h : 192 * h + 64] * scale for h in heads]
        bk = [b_qkv[192 * h + 64 : 192 * h + 128] for h in heads]
        bqk_t = np.concatenate(
            [bq[0], bq[1], bk[0], bk[1], bq[2], bq[3], bk[2], bk[3]]
        ).reshape(4, 128).T
        wv = np.concatenate(
            [W_qkv[:, 192 * h + 128 : 192 * h + 192] for h in heads], axis=1)
        bv = np.concatenate(
            [b_qkv[192 * h + 128 : 192 * h + 192] for h in heads])[None, :]
        wo = W_o[256 * g : 256 * (g + 1), :]
        in_maps.append({
            "xT": xT[b],
            "wqk": np.ascontiguousarray(wqk).astype(NPBF16),
            "bqk": np.ascontiguousarray(bqk_t).astype(np.float32),
            "wv": np.ascontiguousarray(wv).astype(NPBF16),
            "bv": np.ascontiguousarray(bv).astype(NPBF16),
            "wo": np.ascontiguousarray(wo).astype(NPBF16),
        })
    return in_maps


# ---------------------------------------------------------------------------
# Generic additive-mask fallback (previous kernel)
# ---------------------------------------------------------------------------

def _classify_mask(maskT):
    """pattern[hf][kb][nn] for [128, 512] tiles of mask^T (k major, q minor)."""
    pat = []
    for hf in range(2):
        rows = []
        for kb in range(16):
            ents = []
            for nn in range(2):
                blk = maskT[kb * 128 : (kb + 1) * 128,
                            hf * 1024 + nn * 512 : hf * 1024 + (nn + 1) * 512]
                if np.all(blk <= -1e8):
                    ents.append(SKIP)
                elif np.all(blk == 0.0):
                    ents.append(FULL)
                else:
                    ents.append(PART)
            rows.append(tuple(ents))
        pat.append(tuple(rows))
    return tuple(tuple(r) for r in pat)


def _build_generic(pattern):
    nc = bacc.Bacc(None, target_bir_lowering=False, debug=False)
    xT = nc.dram_tensor("xT", [D, S], F32R, kind="ExternalInput")
    wqk = nc.dram_tensor("wqk", [D, 512], F32R, kind="ExternalInput")
    bqk = nc.dram_tensor("bqk", [128, 4], F32R, kind="ExternalInput")
    wv = nc.dram_tensor("wv", [D, 256], F32R, kind="ExternalInput")
    bv = nc.dram_tensor("bv", [1, 256], F32R, kind="ExternalInput")
    wo = nc.dram_tensor("wo", [256, D], F32R, kind="ExternalInput")
    maskT = nc.dram_tensor("maskT", [S, S], F32R, kind="ExternalInput")
    outT = nc.dram_tensor("outT", [D, S], F32, kind="ExternalOutput")

    with tile.TileContext(nc) as tc:
        with (
            tc.tile_pool(name="big", bufs=1) as bigp,
            tc.tile_pool(name="wqkp", bufs=1) as wqkp,
            tc.tile_pool(name="wvp", bufs=1) as wvp,
            tc.tile_pool(name="wop", bufs=1) as wop,
            tc.tile_pool(name="qkp", bufs=1) as qkp,
            tc.tile_pool(name="vp", bufs=1) as vp,
            tc.tile_pool(name="valsp", bufs=1) as valsp,
            tc.tile_pool(name="attnp", bufs=3) as attnp,
            tc.tile_pool(name="maskp", bufs=1) as maskp,
            tc.tile_pool(name="smallp", bufs=1) as smallp,
            tc.tile_pool(name="constp", bufs=1) as constp,
        ):
            ones_t = constp.tile([1, 128], F32R)
            nc.gpsimd.memset(ones_t[:].bitcast(F32), 1.0)
            b_sb = constp.tile([128, 4], F32R)
            nc.sync.dma_start(b_sb[:], bqk[:])
            bv_sb = constp.tile([1, 256], F32R)
            nc.sync.dma_start(bv_sb[:], bv[:])

            wqk_sb = wqkp.tile([128, 8, 512], F32R, tag="wqkslot")
            xt_sb = bigp.tile([128, 8, S], F32R, tag="big")
            wv_sb = wvp.tile([128, 8, 256], F32R)
            for dc in range(8):
                nc.sync.dma_start(wqk_sb[:, dc, :], wqk[ds(dc * 128, 128), :])
                nc.sync.dma_start(
                    xt_sb[:, dc, ds(0, 1024)], xT[ds(dc * 128, 128), ds(0, 1024)]
                )
            nc.sync.dma_start(wv_sb[:], wv[:].rearrange("(dc p) c -> p dc c", p=128))

            qk_sb = qkp.tile([128, 4, S], F32R)
            v_sb = vp.tile([128, 16, HPC, 65], F32R)

            nc.gpsimd.memset(v_sb[:, :, :, 64:65].bitcast(F32), 1.0)

            def n_full(hf):
                return sum(c == FULL for kb in pattern[hf] for c in kb)

            hf_order = sorted(range(2), key=lambda hf: n_full(hf))
            part_blocks, slots, mask_tiles = {}, {}, {}
            for hf_i, hf in enumerate(hf_order):
                pb = [
                    (kb, nn)
                    for kb in range(16) for nn in range(2)
                    if pattern[hf][kb][nn] == PART
                ]
                part_blocks[hf] = pb
                slots[hf] = {blk: i for i, blk in enumerate(pb)}
                if len(pb) == 0:
                    mask_tiles[hf] = None
                elif len(pb) <= 8:
                    if hf_i == 0:
                        mask_tiles[hf] = maskp.tile(
                            [128, 8, 512], F32R, tag="mask", name=f"mask{hf}")
                    else:
                        mask_tiles[hf] = wqkp.tile(
                            [128, 8, 512], F32R, tag="wqkslot", name=f"mask{hf}")
                else:
                    mask_tiles[hf] = bigp.tile(
                        [128, 32, 512], F32R, tag="big", name=f"mask{hf}")

            def mask_dmas(hf, qs_):
                pb = part_blocks[hf]
                i = 0
                while i < len(pb):
                    kb0, nn0 = pb[i]
                    j = i + 1
                    while (j < len(pb)
                           and pb[j] == (pb[j - 1][0] + 1, nn0)):
                        j += 1
                    n = j - i
                    nc.sync.dma_start(
                        mask_tiles[hf][:, i : i + n, :],
                        maskT[ds(kb0 * 128, n * 128),
                              ds(qs_ + nn0 * 512, 512)].rearrange(
                            "(b p) q -> p b q", p=128),
                    )
                    i = j
            hf_first = hf_order[0]
            if mask_tiles.get(hf_first) is not None and len(part_blocks[hf_first]) <= 8:
                mask_dmas(hf_first, hf_first * 1024)
            for dc in range(8):
                nc.sync.dma_start(
                    xt_sb[:, dc, ds(1024, 1024)],
                    xT[ds(dc * 128, 128), ds(1024, 1024)],
                )
            wo_sb = wop.tile([128, 2, D], F32R)
            nc.sync.dma_start(wo_sb[:], wo[:].rearrange("(kc p) d -> p kc d", p=128))

            def qk_chunk_half(pool, cc, sh):
                ps = pool.tile([128, 1024], F32, tag="psqk", name=f"qkps{cc}{sh}")
                for dc in range(8):
                    lhsT = wqk_sb[:, dc, ts(cc, 128)]
                    for nn in range(2):
                        nc.tensor.matmul(
                            ps[:, ts(nn, 512)], lhsT,
                            xt_sb[:, dc, ds(sh * 1024 + nn * 512, 512)],
                            start=(dc == 0), stop=(dc == 7),
                        )
                nc.scalar.activation(
                    qk_sb[:, cc, ds(sh * 1024, 1024)], ps[:], AF.Identity,
                    bias=b_sb[:, cc : cc + 1],
                )

            def v_blocks(pool, rng):
                for sb_i in rng:
                    ps = pool.tile([128, 256], F32, tag="psv", name=f"vps{sb_i}")
                    for dc in range(8):
                        nc.tensor.matmul(
                            ps[:], xt_sb[:, dc, ts(sb_i, 128)], wv_sb[:, dc, :],
                            start=(dc == 0), stop=False,
                        )
                    nc.tensor.matmul(ps[:], ones_t[:], bv_sb[:], start=False, stop=True)
                    nc.vector.tensor_copy(
                        v_sb[:, sb_i, :, 0:64],
                        ps[:].rearrange("p (h e) -> p h e", h=HPC),
                    )

            with (
                tc.tile_pool(name="psqk", bufs=2, space="PSUM") as psqk,
                tc.tile_pool(name="psv", bufs=2, space="PSUM") as psv,
            ):
                for cc in (0, 2, 1, 3):
                    qk_chunk_half(psqk, cc, 0)
                v_blocks(psv, range(8))
                for cc in (0, 2, 1, 3):
                    qk_chunk_half(psqk, cc, 1)
                v_blocks(psv, range(8, 16))

            with (
                tc.tile_pool(name="pssc", bufs=3, space="PSUM") as pssc,
                tc.tile_pool(name="psav", bufs=1, space="PSUM") as psav,
            ):
                def outproj(vals_t, qs_):
                    for ob in range(8):
                        ps = pssc.tile([128, 1024], F32, tag="pssc")
                        for kc in range(2):
                            lhsT = wo_sb[:, kc, ts(ob, 128)]
                            for nn in range(2):
                                nc.tensor.matmul(
                                    ps[:, ts(nn, 512)], lhsT,
                                    vals_t[:, kc, ts(nn, 512)],
                                    start=(kc == 0), stop=(kc == 1),
                                )
                        oev = attnp.tile([128, 1024], F32, tag="attn")
                        nc.scalar.activation(oev[:], ps[:], AF.Copy)
                        nc.sync.dma_start(
                            outT[ds(ob * 128, 128), ds(qs_, 1024)], oev[:]
                        )

                pending = None
                for hf_i, hf in enumerate(hf_order):
                    qs = hf * 1024
                    slot = slots[hf]
                    mask_sb = mask_tiles[hf]
                    if mask_sb is not None and (
                        hf_i > 0 or len(part_blocks[hf]) > 8
                    ):
                        mask_dmas(hf, qs)
                    vals_sb = valsp.tile(
                        [128, 2, 1024], F32R, tag="vals", name=f"vals{hf}")
                    kb_order = sorted(
                        (kb for kb in range(16) if pattern[hf][kb] != (SKIP, SKIP)),
                        key=lambda kb: (PART in pattern[hf][kb], kb),
                    )
                    for h in range(HPC):
                        off = 64 * (h % 2)
                        qt = qk_sb[off : off + 64, h // 2, :]
                        kt = qk_sb[off : off + 64, 2 + h // 2, :]
                        act_kbs = [
                            [kb for kb in kb_order if pattern[hf][kb][nn] != SKIP]
                            for nn in range(2)
                        ]
                        ps_av = psav.tile([65, 1024], F32, tag="psav")
                        for kb in kb_order:
                            cls = pattern[hf][kb]
                            ps_sc = pssc.tile([128, 1024], F32, tag="pssc")
                            lhsT = kt[:, ts(kb, 128)]
                            at = attnp.tile([128, 1024], F32R, tag="attn")
                            for nn in range(2):
                                if cls[nn] == SKIP:
                                    continue
                                nc.tensor.matmul(
                                    ps_sc[:, ts(nn, 512)], lhsT,
                                    qt[:, ds(qs + nn * 512, 512)],
                                    start=True, stop=True,
                                )
                            if cls == (FULL, FULL):
                                nc.scalar.activation(at[:], ps_sc[:], AF.Exp)
                            else:
                                for nn in range(2):
                                    if cls[nn] == SKIP:
                                        continue
                                    if cls[nn] == PART:
                                        nc.vector.tensor_tensor(
                                            at[:, ts(nn, 512)], ps_sc[:, ts(nn, 512)],
                                            mask_sb[:, slot[(kb, nn)], :], ADD,
                                        )
                                        nc.scalar.activation(
                                            at[:, ts(nn, 512)], at[:, ts(nn, 512)],
                                            AF.Exp,
                                        )
                                    else:
                                        nc.scalar.activation(
                                            at[:, ts(nn, 512)], ps_sc[:, ts(nn, 512)],
                                            AF.Exp,
                                        )
                            for nn in range(2):
                                if cls[nn] == SKIP:
                                    continue
                                nc.tensor.matmul(
                                    ps_av[:, ts(nn, 512)], v_sb[:, kb, h, :],
                                    at[:, ts(nn, 512)],
                                    start=(kb == act_kbs[nn][0]),
                                    stop=(kb == act_kbs[nn][-1]),
                                )
                        recip = smallp.tile([1, 1024], F32R, tag="recip")
                        with nc.allow_low_precision(
                            reason="float32r has fp32 bits; only PE matmul mode differs"
                        ):
                            nc.vector.reciprocal(recip[:], ps_av[64:65, :])
                        bc_sb = smallp.tile([64, 1024], F32R, tag="bc")
                        nc.gpsimd.partition_broadcast(bc_sb[:], recip[:])
                        nc.vector.tensor_tensor(
                            vals_sb[off : off + 64, h // 2, :],
                            ps_av[0:64, :], bc_sb[:], MULT,
                        )
                        if pending is not None and hf_i == 1 and h == 0:
                            outproj(*pending)
                            pending = None

                    if hf_i == 0 and len(hf_order) > 1:
                        pending = (vals_sb, qs)
                    else:
                        outproj(vals_sb, qs)
                if pending is not None:
                    outproj(*pending)
                    pending = None

    nc.compile()
    return nc


def _prep_inputs_generic(x, mask, W_qkv, b_qkv, W_o, b_o):
    scale = np.float32(1.0 / np.sqrt(HD))
    xT = [np.ascontiguousarray(x[b].T) for b in range(B)]
    maskT = np.ascontiguousarray(mask.T)
    in_maps = []
    for c in range(NCORES):
        b, g = divmod(c, HPC)
        heads = [HPC * g + i for i in range(HPC)]
        qcols = np.concatenate(
            [W_qkv[:, 192 * h : 192 * h + 64] for h in heads], axis=1) * scale
        kcols = np.concatenate(
            [W_qkv[:, 192 * h + 64 : 192 * h + 128] for h in heads], axis=1)
        wqk = np.ascontiguousarray(np.concatenate([qcols, kcols], axis=1))
        bq = np.concatenate([b_qkv[192 * h : 192 * h + 64] for h in heads]) * scale
        bk = np.concatenate([b_qkv[192 * h + 64 : 192 * h + 128] for h in heads])
        bqk_t = np.ascontiguousarray(
            np.concatenate([bq, bk]).reshape(4, 128).T)
        wv = np.ascontiguousarray(np.concatenate(
            [W_qkv[:, 192 * h + 128 : 192 * h + 192] for h in heads], axis=1))
        bv = np.ascontiguousarray(np.concatenate(
            [b_qkv[192 * h + 128 : 192 * h + 192] for h in heads])[None, :])
        wo = np.ascontiguousarray(W_o[256 * g : 256 * (g + 1), :])
        in_maps.append({
            "xT": xT[b], "wqk": wqk, "bqk": bqk_t, "wv": wv, "bv": bv,
            "wo": wo, "maskT": maskT,
        })
    return in_maps


# ---------------------------------------------------------------------------
# Entry point
# ---------------------------------------------------------------------------

def _prep_inputs(x, mask, W_qkv, b_qkv, W_o, b_o):
    if _is_causal(mask):
        return _prep_inputs_causal(x, mask, W_qkv, b_qkv, W_o, b_o)
    return _prep_inputs_generic(x, mask, W_qkv, b_qkv, W_o, b_o)


def kernel(x, mask, W_qkv, b_qkv, W_o, b_o):
    x = np.asarray(x, dtype=np.float32)
    mask = np.asarray(mask, dtype=np.float32)
    W_qkv = np.asarray(W_qkv, dtype=np.float32)
    b_qkv = np.asarray(b_qkv, dtype=np.float32)
    W_o = np.asarray(W_o, dtype=np.float32)
    b_o = np.asarray(b_o, dtype=np.float32)

    if _is_causal(mask):
        key = "causal"
        if key not in _CACHE:
            _CACHE[key] = _build_causal()
        nc = _CACHE[key]
        in_maps = _prep_inputs_causal(x, mask, W_qkv, b_qkv, W_o, b_o)
    else:
        pattern = _classify_mask(np.ascontiguousarray(mask.T))
        key = ("nc", pattern)
        if key not in _CACHE:
            _CACHE[key] = _build_generic(pattern)
        nc = _CACHE[key]
        in_maps = _prep_inputs_generic(x, mask, W_qkv, b_qkv, W_o, b_o)
    _CACHE["nc"] = nc

    res = run_bass_kernel_spmd(nc, in_maps, core_ids=list(range(NCORES)))
    _CACHE["last_result"] = res

    out = np.empty((B, S, D), dtype=np.float32)
    for b in range(B):
        acc = res.results[HPC * b]["outT"].astype(np.float32)
        for g in range(1, HPC):
            acc = acc + res.results[HPC * b + g]["outT"].astype(np.float32)
        out[b] = acc.T + b_o
    return out


# revision 123
# speedup vs baseline: 1.5746x; 1.0008x over previous
"""Multi-head attention (B=2, S=2048, D=1024, H=16) on 8 TRN2 NeuronCores.

Sharding: core c handles batch b = c//4 and head-group g = c%4 (4 heads each).
Each core computes its heads' attention and a partial output projection
(row-parallel W_o); the host sums the 4 partials per batch and adds b_o.

Fast path (causal mask): bf16 operands off-PSUM, live-span score matmuls
(fully-masked columns never computed), masking via affine_select on the one
diagonal 128x128 block per score tile (no mask tensor at all), and a
transposed AV: attention tiles act as the stationary operand so each
accumulation step emits only 65 output columns ([q,64] values + a ones-column
denominator that lands as a per-partition scalar). vals are transposed back
with PE-transpose against an on-device identity for the row-parallel output
projection. DMAs are batched and spread across the SP/DVE/Pool queues.

A generic additive-mask fallback (the previous kernel) is kept for non-causal
masks.
"""

import numpy as np
import ml_dtypes

import concourse.bass as bass  # noqa: F401
import concourse.mybir as mybir
import concourse.tile as tile
from concourse import bacc
from concourse.bass import ds, ts
from concourse.bass_utils import run_bass_kernel_spmd

B, S, D, H = 2, 2048, 1024, 16
HD = D // H  # 64
HPC = 4      # heads per core
NCORES = 8
F32R = mybir.dt.float32r
F32 = mybir.dt.float32
BF16 = mybir.dt.bfloat16
NPBF16 = ml_dtypes.bfloat16
AF = mybir.ActivationFunctionType
ADD = mybir.AluOpType.add
MULT = mybir.AluOpType.mult

SKIP, FULL, PART = 0, 1, 2

_CACHE = {}


def _is_causal(mask):
    tril = np.tril(np.ones((S, S), dtype=bool))
    return bool(np.all(mask[tril] == 0.0) and np.all(mask[~tril] <= -1e8))


# ---------------------------------------------------------------------------
# Causal fast path
# ---------------------------------------------------------------------------

def _build_causal(dump=False):
    nc = bacc.Bacc(None, target_bir_lowering=False, debug=False)
    xT = nc.dram_tensor("xT", [D, S], BF16, kind="ExternalInput")
    # wqk host-prepacked per chunk: wqk[cc][p, dc*128 + c] = W[dc*128+p, cc*128+c]
    wqk = nc.dram_tensor("wqk", [4, 128, 1024], BF16, kind="ExternalInput")
    bqk = nc.dram_tensor("bqk", [128, 4], F32, kind="ExternalInput")
    wv = nc.dram_tensor("wv", [D, 256], BF16, kind="ExternalInput")
    bv = nc.dram_tensor("bv", [1, 256], BF16, kind="ExternalInput")
    wo = nc.dram_tensor("wo", [256, D], BF16, kind="ExternalInput")
    outT = nc.dram_tensor("outT", [D, S], BF16, kind="ExternalOutput")
    if dump:
        d_qk = nc.dram_tensor("d_qk", [128, 4, S], BF16, kind="ExternalOutput")
        d_v = nc.dram_tensor("d_v", [128, 16, HPC, 65], BF16, kind="ExternalOutput")
        d_vals = nc.dram_tensor(
            "d_vals", [128, 16, HPC, 64], BF16, kind="ExternalOutput")
        d_valsT = nc.dram_tensor("d_valsT", [128, 2, S], BF16, kind="ExternalOutput")
        d_at = nc.dram_tensor("d_at", [128, 16, 1024], BF16, kind="ExternalOutput")
        d_den = nc.dram_tensor("d_den", [128, 16, 65], F32, kind="ExternalOutput")

    with tile.TileContext(nc) as tc:
        with (
            tc.tile_pool(name="bigp", bufs=1) as bigp,
            tc.tile_pool(name="constp", bufs=1) as constp,
            tc.tile_pool(name="atp", bufs=20) as atp,
            tc.tile_pool(name="outp", bufs=2) as outp,
            tc.tile_pool(name="smallp", bufs=4) as smallp,
            tc.tile_pool(name="psBig", bufs=3, space="PSUM") as psBig,
            tc.tile_pool(name="psC", bufs=2, space="PSUM") as psC,
        ):
            # --- constants (small DMAs issue before the memset chain so
            # they hit the DMA device first) ---------------------------------
            b_sb = constp.tile([128, 4], F32)
            nc.gpsimd.dma_start(b_sb[:], bqk[:])
            bv_sb = constp.tile([1, 256], BF16)
            nc.gpsimd.dma_start(bv_sb[:], bv[:])
            # broadcast the V bias across partitions once so the projection
            # bias-add rides the PSUM->SBUF copy instead of a rank-1 matmul
            bv32 = constp.tile([1, 256], F32)
            nc.vector.tensor_copy(bv32[:], bv_sb[:])
            bv_bc = constp.tile([128, 256], F32)
            nc.gpsimd.partition_broadcast(bv_bc[:], bv32[:])
            ident = constp.tile([128, 128], BF16)
            nc.gpsimd.memset(ident[:], 1.0)
            nc.gpsimd.affine_select(
                out=ident[:], in_=ident[:],
                compare_op=mybir.AluOpType.is_equal, fill=0.0,
                base=0, pattern=[[-1, 128]], channel_multiplier=1,
            )
            # upper-triangular (incl. diagonal) ones: keeps k <= q when
            # multiplied into the diagonal block of an exp'd score tile
            tri_t = constp.tile([128, 128], BF16)
            nc.gpsimd.memset(tri_t[:], 1.0)
            nc.gpsimd.affine_select(
                out=tri_t[:], in_=tri_t[:],
                compare_op=mybir.AluOpType.is_ge, fill=0.0,
                base=0, pattern=[[1, 128]], channel_multiplier=-1,
            )

            # --- big SBUF tensors -----------------------------------------
            wqk_sb = constp.tile([128, 8, 512], BF16)
            xt_sb = bigp.tile([128, 8, S], BF16)
            wv_sb = constp.tile([128, 8, 256], BF16)
            wo_sb = constp.tile([128, 2, D], BF16)
            qk_sb = bigp.tile([128, 4, S], BF16)
            v_sb = bigp.tile([128, 16, HPC, 65], BF16)
            nc.gpsimd.memset(v_sb[:, :, :, 64:65], 1.0)
            vals_sb = bigp.tile([128, 16, HPC, 64], BF16)
            valsT_sb = bigp.tile([128, 2, S], BF16)

            # --- input DMAs ------------------------------------------------
            # wqk on the ACT queue (idle until the first exp); chunks are
            # host-prepacked contiguous so descriptors stay 2KB
            for cc in range(2):
                nc.scalar.dma_start(
                    wqk_sb[:, :, ts(cc, 128)],
                    wqk[cc].rearrange("p (dc c) -> p dc c", c=128),
                )
            nc.scalar.dma_start(
                wv_sb[:], wv[:].rearrange("(dc p) c -> p dc c", p=128))
            for cc in range(2, 4):
                nc.scalar.dma_start(
                    wqk_sb[:, :, ts(cc, 128)],
                    wqk[cc].rearrange("p (dc c) -> p dc c", c=128),
                )
            # xT + wv + wo on the SP queue: s-half-major then dc-chunks;
            # the first chunk is small so the first projection matmuls can
            # start as early as possible
            for d0, nd in ((0, 1), (1, 1), (2, 2), (4, 4)):
                nc.sync.dma_start(
                    xt_sb[:, ds(d0, nd), ds(0, 1024)],
                    xT[ds(d0 * 128, nd * 128), ds(0, 1024)].rearrange(
                        "(dc p) s -> p dc s", p=128),
                )
            for dh in range(2):
                nc.sync.dma_start(
                    xt_sb[:, ds(dh * 4, 4), ds(1024, 1024)],
                    xT[ds(dh * 512, 512), ds(1024, 1024)].rearrange(
                        "(dc p) s -> p dc s", p=128),
                )
            nc.sync.dma_start(
                wo_sb[:], wo[:].rearrange("(kc p) d -> p kc d", p=128))

            # --- projection helpers ---------------------------------------
            def qk_chunk(cc, sh):
                """qk_sb[:, cc, sh-half] = wqk-chunk-cc^T @ xT-half + bias."""
                ps = psBig.tile([128, 1024], F32, tag="psB", name=f"qkps{cc}{sh}")
                for dc in range(8):
                    lhsT = wqk_sb[:, dc, ts(cc, 128)]
                    for nn in range(2):
                        nc.tensor.matmul(
                            ps[:, ts(nn, 512)], lhsT,
                            xt_sb[:, dc, ds(sh * 1024 + nn * 512, 512)],
                            start=(dc == 0), stop=(dc == 7),
                        )
                nc.vector.tensor_scalar(
                    qk_sb[:, cc, ds(sh * 1024, 1024)], ps[:],
                    b_sb[:, cc : cc + 1], None, ADD,
                )

            def v_blocks(rng):
                for sb_i in rng:
                    ps = psBig.tile([128, 256], F32, tag="psB", name=f"vps{sb_i}")
                    for dc in range(8):
                        nc.tensor.matmul(
                            ps[:], xt_sb[:, dc, ts(sb_i, 128)], wv_sb[:, dc, :],
                            start=(dc == 0), stop=(dc == 7),
                        )
                    nc.vector.tensor_tensor(
                        v_sb[:, sb_i, :, 0:64],
                        ps[:].rearrange("p (h e) -> p h e", h=HPC),
                        bv_bc[:].rearrange("p (h e) -> p h e", h=HPC),
                        ADD,
                    )

            # --- attention for one (head, q-half) -------------------------
            # Filler machinery: the engines execute in-order, so PE starvation
            # during exp-bound attention stretches can only be avoided by
            # interleaving independent PE work (projections, outproj) into
            # the kb loops at fine granularity, paced by the ACT/PE balance.
            filler_q = []   # list of [name, pe_cost_ns, fn, deadline|None]
            balance = [0.0]

            def pop_fillers(here=None):
                # deadline-forced pops (emit everything up to the overdue
                # entry to preserve dependency order), then balance pops
                if here is not None:
                    h, hf, kb = here
                    due = None
                    for i, ent in enumerate(filler_q):
                        dl = ent[3]
                        if dl is not None and dl[0] == h and dl[1] == hf \
                                and kb >= dl[2]:
                            due = i
                    if due is not None:
                        for ent in filler_q[: due + 1]:
                            ent[2]()
                            balance[0] -= ent[1]
                        del filler_q[: due + 1]
                while filler_q and balance[0] >= filler_q[0][1] * 0.4:
                    name, cost, fn, _ = filler_q.pop(0)
                    fn()
                    balance[0] -= cost

            def drain_until(name):
                while filler_q:
                    nm, cost, fn, _ = filler_q.pop(0)
                    fn()
                    balance[0] -= cost
                    if nm == name:
                        break

            def drain_all():
                while filler_q:
                    _, cost, fn, _ = filler_q.pop(0)
                    fn()
                    balance[0] -= cost

            def attn_head(h, hf, pending_tail=None, av_delay=3):
                qs = hf * 1024
                off = 64 * (h % 2)
                qt = qk_sb[off : off + 64, 2 * (h // 2), :]
                kt = qk_sb[off : off + 64, 2 * (h // 2) + 1, :]
                nkb = 8 if hf == 0 else 16
                ats = []

                def av_group(qb_l):
                    # AV, qb-major: at-tiles are the stationary operand, so
                    # each accumulation step emits only 65 output columns; the
                    # ones column of V lands the denominator per-partition.
                    qb_g = hf * 8 + qb_l
                    pv = psC.tile([128, 65], F32, tag="psC", name=f"pav{h}{hf}{qb_l}")
                    for kb2 in range(qb_g + 1):
                        nc.tensor.matmul(
                            pv[:], ats[kb2][:, ts(qb_l, 128)], v_sb[:, kb2, h, :],
                            start=(kb2 == 0), stop=(kb2 == qb_g),
                        )
                    if dump and h == 0:
                        dtmp = smallp.tile([128, 65], F32, tag="dtmp",
                                           name=f"dd{hf}{qb_l}")
                        nc.vector.tensor_copy(dtmp[:], pv[:])
                        nc.sync.dma_start(d_den[:, qb_g, :], dtmp[:])
                    recip = smallp.tile([128, 1], F32, tag="recip")
                    nc.vector.reciprocal(recip[:], pv[:, 64:65])
                    nc.vector.tensor_scalar(
                        vals_sb[:, qb_g, h, :], pv[:, 0:64],
                        recip[:, 0:1], None, MULT,
                    )

                for kb in range(nkb):
                    if kb >= 1:
                        pop_fillers(here=(h, hf, kb))
                    lo = max(0, kb * 128 - qs)
                    if lo < 512:
                        ps = psBig.tile([128, 1024], F32, tag="psB",
                                        name=f"sc{h}{hf}{kb}")
                        base = 0
                    else:
                        ps = psBig.tile([128, 512], F32, tag="psB",
                                        name=f"sc{h}{hf}{kb}")
                        base = 512
                    lhsT = kt[:, ts(kb, 128)]
                    for seg in range(2):
                        a = max(lo, seg * 512)
                        bnd = (seg + 1) * 512
                        if a < bnd:
                            nc.tensor.matmul(
                                ps[:, ds(a - base, bnd - a)], lhsT,
                                qt[:, ds(qs + a, bnd - a)],
                                start=True, stop=True,
                            )
                    at = atp.tile([128, 1024], BF16, tag="at")
                    nc.scalar.activation(
                        at[:, ds(lo, 1024 - lo)], ps[:, ds(lo - base, 1024 - lo)],
                        AF.Exp)
                    if kb * 128 >= qs:
                        dlo = kb * 128 - qs
                        nc.gpsimd.tensor_tensor(
                            at[:, ds(dlo, 128)], at[:, ds(dlo, 128)],
                            tri_t[:], MULT,
                        )
                    if dump and h == 0:
                        nc.sync.dma_start(
                            d_at[:, hf * 8 + kb if hf == 0 else kb,
                                 ds(lo, 1024 - lo)],
                            at[:, ds(lo, 1024 - lo)])
                    ats.append(at)
                    if kb == 1 and pending_tail is not None:
                        pending_tail()
                    # AV groups run a couple of kbs behind so the
                    # scores->exp->mask chain of the diagonal tile never
                    # stalls the PE queue
                    pe_rows = 1024 - lo
                    if kb - hf * 8 - av_delay >= 0:
                        av_group(kb - hf * 8 - av_delay)
                        pe_rows += 65 * (kb - av_delay + 1)
                    balance[0] += ((1024 - lo) * 0.833 + 185
                                   - pe_rows * 0.4167 - 150)

                def tail():
                    for qb_l in range(8 - av_delay, 8):
                        av_group(qb_l)
                return tail

            # --- vals transpose + output projection for one q-half --------
            def transposes_half(hf, qbs=range(8), hps=(0, 1)):
                qs = hf * 1024
                for qb_l in qbs:
                    qb_g = hf * 8 + qb_l
                    for hp in hps:
                        pst = psBig.tile(
                            [128, 128], BF16, tag="psB", name=f"tr{hf}{qb_l}{hp}")
                        nc.tensor.transpose(
                            pst[:], vals_sb[:, qb_g, ds(hp * 2, 2), :], ident[:])
                        nc.vector.tensor_copy(
                            valsT_sb[:, hp, ds(qs + qb_l * 128, 128)], pst[:])

            _ot_state = {}

            def outproj_ob(hf, ob):
                qs = hf * 1024
                obg, obi = divmod(ob, 4)
                if (hf, obg) not in _ot_state:
                    _ot_state[(hf, obg)] = outp.tile(
                        [128, 4, 1024], BF16, tag="out", name=f"ot{hf}{obg}")
                ot = _ot_state[(hf, obg)]
                ps = psBig.tile([128, 1024], F32, tag="psB", name=f"op{hf}{ob}")
                for kc in range(2):
                    lhsT = wo_sb[:, kc, ts(ob, 128)]
                    for nn in range(2):
                        nc.tensor.matmul(
                            ps[:, ts(nn, 512)], lhsT,
                            valsT_sb[:, kc, ds(qs + nn * 512, 512)],
                            start=(kc == 0), stop=(kc == 1),
                        )
                # Pool cannot read PSUM (walrus restriction): hf0 copies go
                # on DVE; hf1 copies (tail, ACT idle) alternate DVE/ACT
                if hf == 1 and obi % 2 == 0:
                    nc.scalar.activation(ot[:, obi, :], ps[:], AF.Copy)
                else:
                    nc.vector.tensor_copy(ot[:, obi, :], ps[:])
                if hf == 1:
                    # per-ob DMAs at the tail, alternating queues, so the
                    # last transfer is small and starts as late as possible
                    dma_eng = nc.sync if obi % 2 == 0 else nc.scalar
                    dma_eng.dma_start(
                        outT[ds(ob * 128, 128), ds(qs, 1024)],
                        ot[:, obi, :],
                    )
                elif obi % 2 == 1:
                    dma_eng = nc.sync if obg % 2 == 0 else nc.scalar
                    dma_eng.dma_start(
                        outT[ds(obg * 512 + (obi - 1) * 128, 256),
                             ds(qs, 1024)].rearrange(
                            "(ob p) q -> p ob q", p=128),
                        ot[:, ds(obi - 1, 2), :],
                    )

            def outproj_half(hf):
                for ob in range(8):
                    outproj_ob(hf, ob)

            def qk_unit(cc, sh, h, hf, kb):
                """A projection chunk as one deadline-pinned filler unit
                (the open psum accumulation group cannot be interleaved with
                other pool allocations, so the chunk stays monolithic)."""
                return [f"qk{cc}{sh}", 10 ** 6,
                        (lambda: qk_chunk(cc, sh)), (h, hf, kb)]

            def qk_unit2(cc, sh, h, hf, kb):
                """A projection chunk as TWO filler units at consecutive kb
                deadlines. Safe against psum-slot recycling because only one
                pool allocation (that kb's score tile) lands between them,
                which is below the pool depth."""
                state = {}

                def half(d0, nd=4):
                    def f():
                        if "ps" not in state:
                            state["ps"] = psBig.tile(
                                [128, 1024], F32, tag="psB",
                                name=f"qkps{cc}{sh}")
                        ps = state["ps"]
                        for dc in range(d0, d0 + nd):
                            lhsT = wqk_sb[:, dc, ts(cc, 128)]
                            for nn in range(2):
                                nc.tensor.matmul(
                                    ps[:, ts(nn, 512)], lhsT,
                                    xt_sb[:, dc, ds(sh * 1024 + nn * 512, 512)],
                                    start=(dc == 0), stop=(dc == 7),
                                )
                        if d0 + nd == 8:
                            nc.vector.tensor_scalar(
                                qk_sb[:, cc, ds(sh * 1024, 1024)], ps[:],
                                b_sb[:, cc : cc + 1], None, ADD,
                            )
                    return f

                nkb_h = 8 if hf == 0 else 16
                return [
                    [f"qk{cc}{sh}_{d0}", 10 ** 6, half(d0, 2),
                     (h, hf, min(kb + (d0 // 2) * 2, nkb_h - 1))]
                    for d0 in (0, 2, 4, 6)
                ]

            # --- emission order: everything that is not on the critical
            # dependency prefix is deadline-pinned into the attention kb
            # loops so the in-order PE queue never runs dry.
            BIGC = 10 ** 6
            qk_chunk(0, 0)
            qk_chunk(1, 0)
            v_blocks(range(8))
            filler_q.extend(qk_unit2(2, 0, 0, 0, 1))
            filler_q.extend(qk_unit2(3, 0, 1, 0, 1))
            t = attn_head(0, 0)
            filler_q.extend(qk_unit2(0, 1, 2, 0, 1))
            filler_q.extend(qk_unit2(1, 1, 3, 0, 1))
            t = attn_head(1, 0, t)
            t = attn_head(2, 0, t)
            t = attn_head(3, 0, t)
            drain_all()
            filler_q.extend(
                [
                    [f"V{sb_i}", BIGC,
                     (lambda sb_i=sb_i: v_blocks([sb_i])),
                     (0, 1, sb_i - 4)]
                    for sb_i in range(8, 16)
                ]
                + [
                    ["tr0a", BIGC, lambda: transposes_half(0, qbs=range(0, 3)),
                     (1, 1, 2)],
                    ["tr0b", BIGC, lambda: transposes_half(0, qbs=range(3, 6)),
                     (1, 1, 3)],
                    ["tr0c", BIGC, lambda: transposes_half(0, qbs=range(6, 8)),
                     (1, 1, 4)],
                ]
                + qk_unit2(2, 1, 1, 1, 7)
            )
            t = attn_head(0, 1, t)
            t = attn_head(1, 1, t)
            drain_all()
            filler_q.append(qk_unit(3, 1, 2, 1, 6))
            filler_q.extend([
                # outproj(0) is the only arbitrarily-deferrable PE work:
                # spread it across the last two heads
                [f"op0{ob}", BIGC, (lambda ob=ob: outproj_ob(0, ob)),
                 (2, 1, 9 + 2 * ob) if ob < 4 else (3, 1, 2 * ob - 4)]
                for ob in range(4)
            ])
            t = attn_head(2, 1, t)
            filler_q.extend([
                ["tr1a", BIGC,
                 lambda: transposes_half(1, qbs=range(0, 4), hps=(0,)),
                 (3, 1, 2)],
                ["tr1b", BIGC,
                 lambda: transposes_half(1, qbs=range(4, 8), hps=(0,)),
                 (3, 1, 3)],

            ] + [
                [f"op0{ob}", BIGC, (lambda ob=ob: outproj_ob(0, ob)),
                 (3, 1, 2 * ob - 4)]
                for ob in range(4, 8)
            ])
            t = attn_head(3, 1, t, av_delay=2)
            t()
            drain_all()
            transposes_half(1, hps=(1,))
            outproj_half(1)
            if dump:
                nc.sync.dma_start(d_qk[:], qk_sb[:])
                nc.sync.dma_start(d_v[:], v_sb[:])
                nc.sync.dma_start(d_vals[:], vals_sb[:])
                nc.sync.dma_start(d_valsT[:], valsT_sb[:])

    nc.compile()
    return nc


def _prep_inputs_causal(x, mask, W_qkv, b_qkv, W_o, b_o):
    scale = np.float32(1.0 / np.sqrt(HD))
    xT = [np.ascontiguousarray(x[b].T).astype(NPBF16) for b in range(B)]
    in_maps = []
    for c in range(NCORES):
        b, g = divmod(c, HPC)
        heads = [HPC * g + i for i in range(HPC)]
        qc = [W_qkv[:, 192 * h : 192 * h + 64] * scale for h in heads]
        kc = [W_qkv[:, 192 * h + 64 : 192 * h + 128] for h in heads]
        # chunk order: q01 | k01 | q23 | k23; each chunk prepacked so the
        # per-partition row (dc, c) is contiguous in DRAM
        chunks = [
            np.concatenate([qc[0], qc[1]], axis=1),
            np.concatenate([kc[0], kc[1]], axis=1),
            np.concatenate([qc[2], qc[3]], axis=1),
            np.concatenate([kc[2], kc[3]], axis=1),
        ]
        wqk = np.stack([
            c.reshape(8, 128, 128).transpose(1, 0, 2).reshape(128, 1024)
            for c in chunks
        ])
        bq = [b_qkv[192 * h : 192 * h + 64] * scale for h in heads]
        bk = [b_qkv[192 * h + 64 : 192 * h + 128] for h in heads]
        bqk_t = np.concatenate(
            [bq[0], bq[1], bk[0], bk[1], bq[2], bq[3], bk[2], bk[3]]
        ).reshape(4, 128).T
        wv = np.concatenate(
            [W_qkv[:, 192 * h + 128 : 192 * h + 192] for h in heads], axis=1)
        bv = np.concatenate(
            [b_qkv[192 * h + 128 : 192 * h + 192] for h in heads])[None, :]
        wo = W_o[256 * g : 256 * (g + 1), :]
        in_maps.append({
            "xT": xT[b],
            "wqk": np.ascontiguousarray(wqk).astype(NPBF16),
            "bqk": np.ascontiguousarray(bqk_t).astype(np.float32),
            "wv": np.ascontiguousarray(wv).astype(NPBF16),
            "bv": np.ascontiguousarray(bv).astype(NPBF16),
            "wo": np.ascontiguousarray(wo).astype(NPBF16),
        })
    return in_maps


# ---------------------------------------------------------------------------
# Generic additive-mask fallback (previous kernel)
# ---------------------------------------------------------------------------

def _classify_mask(maskT):
    """pattern[hf][kb][nn] for [128, 512] tiles of mask^T (k major, q minor)."""
    pat = []
    for hf in range(2):
        rows = []
        for kb in range(16):
            ents = []
            for nn in range(2):
                blk = maskT[kb * 128 : (kb + 1) * 128,
                            hf * 1024 + nn * 512 : hf * 1024 + (nn + 1) * 512]
                if np.all(blk <= -1e8):
                    ents.append(SKIP)
                elif np.all(blk == 0.0):
                    ents.append(FULL)
                else:
                    ents.append(PART)
            rows.append(tuple(ents))
        pat.append(tuple(rows))
    return tuple(tuple(r) for r in pat)


def _build_generic(pattern):
    nc = bacc.Bacc(None, target_bir_lowering=False, debug=False)
    xT = nc.dram_tensor("xT", [D, S], F32R, kind="ExternalInput")
    wqk = nc.dram_tensor("wqk", [D, 512], F32R, kind="ExternalInput")
    bqk = nc.dram_tensor("bqk", [128, 4], F32R, kind="ExternalInput")
    wv = nc.dram_tensor("wv", [D, 256], F32R, kind="ExternalInput")
    bv = nc.dram_tensor("bv", [1, 256], F32R, kind="ExternalInput")
    wo = nc.dram_tensor("wo", [256, D], F32R, kind="ExternalInput")
    maskT = nc.dram_tensor("maskT", [S, S], F32R, kind="ExternalInput")
    outT = nc.dram_tensor("outT", [D, S], F32, kind="ExternalOutput")

    with tile.TileContext(nc) as tc:
        with (
            tc.tile_pool(name="big", bufs=1) as bigp,
            tc.tile_pool(name="wqkp", bufs=1) as wqkp,
            tc.tile_pool(name="wvp", bufs=1) as wvp,
            tc.tile_pool(name="wop", bufs=1) as wop,
            tc.tile_pool(name="qkp", bufs=1) as qkp,
            tc.tile_pool(name="vp", bufs=1) as vp,
            tc.tile_pool(name="valsp", bufs=1) as valsp,
            tc.tile_pool(name="attnp", bufs=3) as attnp,
            tc.tile_pool(name="maskp", bufs=1) as maskp,
            tc.tile_pool(name="smallp", bufs=1) as smallp,
            tc.tile_pool(name="constp", bufs=1) as constp,
        ):
            ones_t = constp.tile([1, 128], F32R)
            nc.gpsimd.memset(ones_t[:].bitcast(F32), 1.0)
            b_sb = constp.tile([128, 4], F32R)
            nc.sync.dma_start(b_sb[:], bqk[:])
            bv_sb = constp.tile([1, 256], F32R)
            nc.sync.dma_start(bv_sb[:], bv[:])

            wqk_sb = wqkp.tile([128, 8, 512], F32R, tag="wqkslot")
            xt_sb = bigp.tile([128, 8, S], F32R, tag="big")
            wv_sb = wvp.tile([128, 8, 256], F32R)
            for dc in range(8):
                nc.sync.dma_start(wqk_sb[:, dc, :], wqk[ds(dc * 128, 128), :])
                nc.sync.dma_start(
                    xt_sb[:, dc, ds(0, 1024)], xT[ds(dc * 128, 128), ds(0, 1024)]
                )
            nc.sync.dma_start(wv_sb[:], wv[:].rearrange("(dc p) c -> p dc c", p=128))

            qk_sb = qkp.tile([128, 4, S], F32R)
            v_sb = vp.tile([128, 16, HPC, 65], F32R)

            nc.gpsimd.memset(v_sb[:, :, :, 64:65].bitcast(F32), 1.0)

            def n_full(hf):
                return sum(c == FULL for kb in pattern[hf] for c in kb)

            hf_order = sorted(range(2), key=lambda hf: n_full(hf))
            part_blocks, slots, mask_tiles = {}, {}, {}
            for hf_i, hf in enumerate(hf_order):
                pb = [
                    (kb, nn)
                    for kb in range(16) for nn in range(2)
                    if pattern[hf][kb][nn] == PART
                ]
                part_blocks[hf] = pb
                slots[hf] = {blk: i for i, blk in enumerate(pb)}
                if len(pb) == 0:
                    mask_tiles[hf] = None
                elif len(pb) <= 8:
                    if hf_i == 0:
                        mask_tiles[hf] = maskp.tile(
                            [128, 8, 512], F32R, tag="mask", name=f"mask{hf}")
                    else:
                        mask_tiles[hf] = wqkp.tile(
                            [128, 8, 512], F32R, tag="wqkslot", name=f"mask{hf}")
                else:
                    mask_tiles[hf] = bigp.tile(
                        [128, 32, 512], F32R, tag="big", name=f"mask{hf}")

            def mask_dmas(hf, qs_):
                pb = part_blocks[hf]
                i = 0
                while i < len(pb):
                    kb0, nn0 = pb[i]
                    j = i + 1
                    while (j < len(pb)
                           and pb[j] == (pb[j - 1][0] + 1, nn0)):
                        j += 1
                    n = j - i
                    nc.sync.dma_start(
                        mask_tiles[hf][:, i : i + n, :],
                        maskT[ds(kb0 * 128, n * 128),
                              ds(qs_ + nn0 * 512, 512)].rearrange(
                            "(b p) q -> p b q", p=128),
                    )
                    i = j
            hf_first = hf_order[0]
            if mask_tiles.get(hf_first) is not None and len(part_blocks[hf_first]) <= 8:
                mask_dmas(hf_first, hf_first * 1024)
            for dc in range(8):
                nc.sync.dma_start(
                    xt_sb[:, dc, ds(1024, 1024)],
                    xT[ds(dc * 128, 128), ds(1024, 1024)],
                )
            wo_sb = wop.tile([128, 2, D], F32R)
            nc.sync.dma_start(wo_sb[:], wo[:].rearrange("(kc p) d -> p kc d", p=128))

            def qk_chunk_half(pool, cc, sh):
                ps = pool.tile([128, 1024], F32, tag="psqk", name=f"qkps{cc}{sh}")
                for dc in range(8):
                    lhsT = wqk_sb[:, dc, ts(cc, 128)]
                    for nn in range(2):
                        nc.tensor.matmul(
                            ps[:, ts(nn, 512)], lhsT,
                            xt_sb[:, dc, ds(sh * 1024 + nn * 512, 512)],
                            start=(dc == 0), stop=(dc == 7),
                        )
                nc.scalar.activation(
                    qk_sb[:, cc, ds(sh * 1024, 1024)], ps[:], AF.Identity,
                    bias=b_sb[:, cc : cc + 1],
                )

            def v_blocks(pool, rng):
                for sb_i in rng:
                    ps = pool.tile([128, 256], F32, tag="psv", name=f"vps{sb_i}")
                    for dc in range(8):
                        nc.tensor.matmul(
                            ps[:], xt_sb[:, dc, ts(sb_i, 128)], wv_sb[:, dc, :],
                            start=(dc == 0), stop=False,
                        )
                    nc.tensor.matmul(ps[:], ones_t[:], bv_sb[:], start=False, stop=True)
                    nc.vector.tensor_copy(
                        v_sb[:, sb_i, :, 0:64],
                        ps[:].rearrange("p (h e) -> p h e", h=HPC),
                    )

            with (
                tc.tile_pool(name="psqk", bufs=2, space="PSUM") as psqk,
                tc.tile_pool(name="psv", bufs=2, space="PSUM") as psv,
            ):
                for cc in (0, 2, 1, 3):
                    qk_chunk_half(psqk, cc, 0)
                v_blocks(psv, range(8))
                for cc in (0, 2, 1, 3):
                    qk_chunk_half(psqk, cc, 1)
                v_blocks(psv, range(8, 16))

            with (
                tc.tile_pool(name="pssc", bufs=3, space="PSUM") as pssc,
                tc.tile_pool(name="psav", bufs=1, space="PSUM") as psav,
            ):
                def outproj(vals_t, qs_):
                    for ob in range(8):
                        ps = pssc.tile([128, 1024], F32, tag="pssc")
                        for kc in range(2):
                            lhsT = wo_sb[:, kc, ts(ob, 128)]
                            for nn in range(2):
                                nc.tensor.matmul(
                                    ps[:, ts(nn, 512)], lhsT,
                                    vals_t[:, kc, ts(nn, 512)],
                                    start=(kc == 0), stop=(kc == 1),
                                )
                        oev = attnp.tile([128, 1024], F32, tag="attn")
                        nc.scalar.activation(oev[:], ps[:], AF.Copy)
                        nc.sync.dma_start(
                            outT[ds(ob * 128, 128), ds(qs_, 1024)], oev[:]
                        )

                pending = None
                for hf_i, hf in enumerate(hf_order):
                    qs = hf * 1024
                    slot = slots[hf]
                    mask_sb = mask_tiles[hf]
                    if mask_sb is not None and (
                        hf_i > 0 or len(part_blocks[hf]) > 8
                    ):
                        mask_dmas(hf, qs)
                    vals_sb = valsp.tile(
                        [128, 2, 1024], F32R, tag="vals", name=f"vals{hf}")
                    kb_order = sorted(
                        (kb for kb in range(16) if pattern[hf][kb] != (SKIP, SKIP)),
                        key=lambda kb: (PART in pattern[hf][kb], kb),
                    )
                    for h in range(HPC):
                        off = 64 * (h % 2)
                        qt = qk_sb[off : off + 64, h // 2, :]
                        kt = qk_sb[off : off + 64, 2 + h // 2, :]
                        act_kbs = [
                            [kb for kb in kb_order if pattern[hf][kb][nn] != SKIP]
                            for nn in range(2)
                        ]
                        ps_av = psav.tile([65, 1024], F32, tag="psav")
                        for kb in kb_order:
                            cls = pattern[hf][kb]
                            ps_sc = pssc.tile([128, 1024], F32, tag="pssc")
                            lhsT = kt[:, ts(kb, 128)]
                            at = attnp.tile([128, 1024], F32R, tag="attn")
                            for nn in range(2):
                                if cls[nn] == SKIP:
                                    continue
                                nc.tensor.matmul(
                                    ps_sc[:, ts(nn, 512)], lhsT,
                                    qt[:, ds(qs + nn * 512, 512)],
                                    start=True, stop=True,
                                )
                            if cls == (FULL, FULL):
                                nc.scalar.activation(at[:], ps_sc[:], AF.Exp)
                            else:
                                for nn in range(2):
                                    if cls[nn] == SKIP:
                                        continue
                                    if cls[nn] == PART:
                                        nc.vector.tensor_tensor(
                                            at[:, ts(nn, 512)], ps_sc[:, ts(nn, 512)],
                                            mask_sb[:, slot[(kb, nn)], :], ADD,
                                        )
                                        nc.scalar.activation(
                                            at[:, ts(nn, 512)], at[:, ts(nn, 512)],
                                            AF.Exp,
                                        )
                                    else:
                                        nc.scalar.activation(
                                            at[:, ts(nn, 512)], ps_sc[:, ts(nn, 512)],
                                            AF.Exp,
                                        )
                            for nn in range(2):
                                if cls[nn] == SKIP:
                                    continue
                                nc.tensor.matmul(
                                    ps_av[:, ts(nn, 512)], v_sb[:, kb, h, :],
                                    at[:, ts(nn, 512)],
                                    start=(kb == act_kbs[nn][0]),
                                    stop=(kb == act_kbs[nn][-1]),
                                )
                        recip = smallp.tile([1, 1024], F32R, tag="recip")
                        with nc.allow_low_precision(
                            reason="float32r has fp32 bits; only PE matmul mode differs"
                        ):
                            nc.vector.reciprocal(recip[:], ps_av[64:65, :])
                        bc_sb = smallp.tile([64, 1024], F32R, tag="bc")
                        nc.gpsimd.partition_broadcast(bc_sb[:], recip[:])
                        nc.vector.tensor_tensor(
                            vals_sb[off : off + 64, h // 2, :],
                            ps_av[0:64, :], bc_sb[:], MULT,
                        )
                        if pending is not None and hf_i == 1 and h == 0:
                            outproj(*pending)
                            pending = None

                    if hf_i == 0 and len(hf_order) > 1:
                        pending = (vals_sb, qs)
                    else:
                        outproj(vals_sb, qs)
                if pending is not None:
                    outproj(*pending)
                    pending = None

    nc.compile()
    return nc


def _prep_inputs_generic(x, mask, W_qkv, b_qkv, W_o, b_o):
    scale = np.float32(1.0 / np.sqrt(HD))
    xT = [np.ascontiguousarray(x[b].T) for b in range(B)]
    maskT = np.ascontiguousarray(mask.T)
    in_maps = []
    for c in range(NCORES):
        b, g = divmod(c, HPC)
        heads = [HPC * g + i for i in range(HPC)]
        qcols = np.concatenate(
            [W_qkv[:, 192 * h : 192 * h + 64] for h in heads], axis=1) * scale
        kcols = np.concatenate(
            [W_qkv[:, 192 * h + 64 : 192 * h + 128] for h in heads], axis=1)
        wqk = np.ascontiguousarray(np.concatenate([qcols, kcols], axis=1))
        bq = np.concatenate([b_qkv[192 * h : 192 * h + 64] for h in heads]) * scale
        bk = np.concatenate([b_qkv[192 * h + 64 : 192 * h + 128] for h in heads])
        bqk_t = np.ascontiguousarray(
            np.concatenate([bq, bk]).reshape(4, 128).T)
        wv = np.ascontiguousarray(np.concatenate(
            [W_qkv[:, 192 * h + 128 : 192 * h + 192] for h in heads], axis=1))
        bv = np.ascontiguousarray(np.concatenate(
            [b_qkv[192 * h + 128 : 192 * h + 192] for h in heads])[None, :])
        wo = np.ascontiguousarray(W_o[256 * g : 256 * (g + 1), :])
        in_maps.append({
            "xT": xT[b], "wqk": wqk, "bqk": bqk_t, "wv": wv, "bv": bv,
            "wo": wo, "maskT": maskT,
        })
    return in_maps


# ---------------------------------------------------------------------------
# Entry point
# ---------------------------------------------------------------------------

def _prep_inputs(x, mask, W_qkv, b_qkv, W_o, b_o):
    if _is_causal(mask):
        return _prep_inputs_causal(x, mask, W_qkv, b_qkv, W_o, b_o)
    return _prep_inputs_generic(x, mask, W_qkv, b_qkv, W_o, b_o)


def kernel(x, mask, W_qkv, b_qkv, W_o, b_o):
    x = np.asarray(x, dtype=np.float32)
    mask = np.asarray(mask, dtype=np.float32)
    W_qkv = np.asarray(W_qkv, dtype=np.float32)
    b_qkv = np.asarray(b_qkv, dtype=np.float32)
    W_o = np.asarray(W_o, dtype=np.float32)
    b_o = np.asarray(b_o, dtype=np.float32)

    if _is_causal(mask):
        key = "causal"
        if key not in _CACHE:
            _CACHE[key] = _build_causal()
        nc = _CACHE[key]
        in_maps = _prep_inputs_causal(x, mask, W_qkv, b_qkv, W_o, b_o)
    else:
        pattern = _classify_mask(np.ascontiguousarray(mask.T))
        key = ("nc", pattern)
        if key not in _CACHE:
            _CACHE[key] = _build_generic(pattern)
        nc = _CACHE[key]
        in_maps = _prep_inputs_generic(x, mask, W_qkv, b_qkv, W_o, b_o)
    _CACHE["nc"] = nc

    res = run_bass_kernel_spmd(nc, in_maps, core_ids=list(range(NCORES)))
    _CACHE["last_result"] = res

    out = np.empty((B, S, D), dtype=np.float32)
    for b in range(B):
        acc = res.results[HPC * b]["outT"].astype(np.float32)
        for g in range(1, HPC):
            acc = acc + res.results[HPC * b + g]["outT"].astype(np.float32)
        out[b] = acc.T + b_o
    return out


# revision 130
# speedup vs baseline: 1.5826x; 1.0051x over previous
"""Multi-head attention (B=2, S=2048, D=1024, H=16) on 8 TRN2 NeuronCores.

Sharding: core c handles batch b = c//4 and head-group g = c%4 (4 heads each).
Each core computes its heads' attention and a partial output projection
(row-parallel W_o); the host sums the 4 partials per batch and adds b_o.

Fast path (causal mask): bf16 operands off-PSUM, live-span score matmuls
(fully-masked columns never computed), masking via affine_select on the one
diagonal 128x128 block per score tile (no mask tensor at all), and a
transposed AV: attention tiles act as the stationary operand so each
accumulation step emits only 65 output columns ([q,64] values + a ones-column
denominator that lands as a per-partition scalar). vals are transposed back
with PE-transpose against an on-device identity for the row-parallel output
projection. DMAs are batched and spread across the SP/DVE/Pool queues.

A generic additive-mask fallback (the previous kernel) is kept for non-causal
masks.
"""

import numpy as np
import ml_dtypes

import concourse.bass as bass  # noqa: F401
import concourse.mybir as mybir
import concourse.tile as tile
from concourse import bacc
from concourse.bass import ds, ts
from concourse.bass_utils import run_bass_kernel_spmd

B, S, D, H = 2, 2048, 1024, 16
HD = D // H  # 64
HPC = 4      # heads per core
NCORES = 8
F32R = mybir.dt.float32r
F32 = mybir.dt.float32
BF16 = mybir.dt.bfloat16
NPBF16 = ml_dtypes.bfloat16
AF = mybir.ActivationFunctionType
ADD = mybir.AluOpType.add
MULT = mybir.AluOpType.mult

SKIP, FULL, PART = 0, 1, 2

_CACHE = {}


def _is_causal(mask):
    tril = np.tril(np.ones((S, S), dtype=bool))
    return bool(np.all(mask[tril] == 0.0) and np.all(mask[~tril] <= -1e8))


# ---------------------------------------------------------------------------
# Causal fast path
# ---------------------------------------------------------------------------

def _build_causal(dump=False):
    nc = bacc.Bacc(None, target_bir_lowering=False, debug=False)
    xT = nc.dram_tensor("xT", [D, S], BF16, kind="ExternalInput")
    # wqk host-prepacked per chunk: wqk[cc][p, dc*128 + c] = W[dc*128+p, cc*128+c]
    wqk = nc.dram_tensor("wqk", [4, 128, 1024], BF16, kind="ExternalInput")
    bqk = nc.dram_tensor("bqk", [128, 4], F32, kind="ExternalInput")
    wv = nc.dram_tensor("wv", [D, 256], BF16, kind="ExternalInput")
    bv = nc.dram_tensor("bv", [1, 256], BF16, kind="ExternalInput")
    wo = nc.dram_tensor("wo", [256, D], BF16, kind="ExternalInput")
    outT = nc.dram_tensor("outT", [D, S], BF16, kind="ExternalOutput")
    if dump:
        d_qk = nc.dram_tensor("d_qk", [128, 4, S], BF16, kind="ExternalOutput")
        d_v = nc.dram_tensor("d_v", [128, 16, HPC, 65], BF16, kind="ExternalOutput")
        d_vals = nc.dram_tensor(
            "d_vals", [128, 16, HPC, 64], BF16, kind="ExternalOutput")
        d_valsT = nc.dram_tensor("d_valsT", [128, 2, S], BF16, kind="ExternalOutput")
        d_at = nc.dram_tensor("d_at", [128, 16, 1024], BF16, kind="ExternalOutput")
        d_den = nc.dram_tensor("d_den", [128, 16, 65], F32, kind="ExternalOutput")

    with tile.TileContext(nc) as tc:
        with (
            tc.tile_pool(name="bigp", bufs=1) as bigp,
            tc.tile_pool(name="constp", bufs=1) as constp,
            tc.tile_pool(name="atp", bufs=20) as atp,
            tc.tile_pool(name="outp", bufs=2) as outp,
            tc.tile_pool(name="smallp", bufs=4) as smallp,
            tc.tile_pool(name="psBig", bufs=3, space="PSUM") as psBig,
            tc.tile_pool(name="psC", bufs=2, space="PSUM") as psC,
        ):
            # --- constants (small DMAs issue before the memset chain so
            # they hit the DMA device first) ---------------------------------
            b_sb = constp.tile([128, 4], F32)
            nc.gpsimd.dma_start(b_sb[:], bqk[:])
            bv_sb = constp.tile([1, 256], BF16)
            nc.gpsimd.dma_start(bv_sb[:], bv[:])
            # broadcast the V bias across partitions once so the projection
            # bias-add rides the PSUM->SBUF copy instead of a rank-1 matmul
            bv32 = constp.tile([1, 256], F32)
            nc.vector.tensor_copy(bv32[:], bv_sb[:])
            bv_bc = constp.tile([128, 256], F32)
            nc.gpsimd.partition_broadcast(bv_bc[:], bv32[:])
            ident = constp.tile([128, 128], BF16)
            nc.gpsimd.memset(ident[:], 1.0)
            nc.gpsimd.affine_select(
                out=ident[:], in_=ident[:],
                compare_op=mybir.AluOpType.is_equal, fill=0.0,
                base=0, pattern=[[-1, 128]], channel_multiplier=1,
            )
            # upper-triangular (incl. diagonal) ones: keeps k <= q when
            # multiplied into the diagonal block of an exp'd score tile
            tri_t = constp.tile([128, 128], BF16)
            nc.gpsimd.memset(tri_t[:], 1.0)
            nc.gpsimd.affine_select(
                out=tri_t[:], in_=tri_t[:],
                compare_op=mybir.AluOpType.is_ge, fill=0.0,
                base=0, pattern=[[1, 128]], channel_multiplier=-1,
            )

            # --- big SBUF tensors -----------------------------------------
            wqk_sb = constp.tile([128, 8, 512], BF16)
            xt_sb = bigp.tile([128, 8, S], BF16)
            wv_sb = constp.tile([128, 8, 256], BF16)
            wo_sb = constp.tile([128, 2, D], BF16)
            qk_sb = bigp.tile([128, 4, S], BF16)
            v_sb = bigp.tile([128, 16, HPC, 65], BF16)
            nc.gpsimd.memset(v_sb[:, :, :, 64:65], 1.0)
            vals_sb = bigp.tile([128, 16, HPC, 64], BF16)
            valsT_sb = bigp.tile([128, 2, S], BF16)

            # --- input DMAs ------------------------------------------------
            # wqk on the ACT queue (idle until the first exp); chunks are
            # host-prepacked contiguous so descriptors stay 2KB
            for cc in range(2):
                nc.scalar.dma_start(
                    wqk_sb[:, :, ts(cc, 128)],
                    wqk[cc].rearrange("p (dc c) -> p dc c", c=128),
                )
            nc.scalar.dma_start(
                wv_sb[:], wv[:].rearrange("(dc p) c -> p dc c", p=128))
            for cc in range(2, 4):
                nc.scalar.dma_start(
                    wqk_sb[:, :, ts(cc, 128)],
                    wqk[cc].rearrange("p (dc c) -> p dc c", c=128),
                )
            # xT + wv + wo on the SP queue: s-half-major then dc-chunks;
            # the first chunk is small so the first projection matmuls can
            # start as early as possible
            for d0, nd in ((0, 1), (1, 1), (2, 2), (4, 4)):
                nc.sync.dma_start(
                    xt_sb[:, ds(d0, nd), ds(0, 1024)],
                    xT[ds(d0 * 128, nd * 128), ds(0, 1024)].rearrange(
                        "(dc p) s -> p dc s", p=128),
                )
            for dh in range(2):
                nc.sync.dma_start(
                    xt_sb[:, ds(dh * 4, 4), ds(1024, 1024)],
                    xT[ds(dh * 512, 512), ds(1024, 1024)].rearrange(
                        "(dc p) s -> p dc s", p=128),
                )
            nc.sync.dma_start(
                wo_sb[:], wo[:].rearrange("(kc p) d -> p kc d", p=128))

            # --- projection helpers ---------------------------------------
            def qk_chunk(cc, sh):
                """qk_sb[:, cc, sh-half] = wqk-chunk-cc^T @ xT-half + bias."""
                ps = psBig.tile([128, 1024], F32, tag="psB", name=f"qkps{cc}{sh}")
                for dc in range(8):
                    lhsT = wqk_sb[:, dc, ts(cc, 128)]
                    for nn in range(2):
                        nc.tensor.matmul(
                            ps[:, ts(nn, 512)], lhsT,
                            xt_sb[:, dc, ds(sh * 1024 + nn * 512, 512)],
                            start=(dc == 0), stop=(dc == 7),
                        )
                nc.vector.tensor_scalar(
                    qk_sb[:, cc, ds(sh * 1024, 1024)], ps[:],
                    b_sb[:, cc : cc + 1], None, ADD,
                )

            def v_blocks(rng):
                for sb_i in rng:
                    ps = psBig.tile([128, 256], F32, tag="psB", name=f"vps{sb_i}")
                    for dc in range(8):
                        nc.tensor.matmul(
                            ps[:], xt_sb[:, dc, ts(sb_i, 128)], wv_sb[:, dc, :],
                            start=(dc == 0), stop=(dc == 7),
                        )
                    nc.vector.tensor_tensor(
                        v_sb[:, sb_i, :, 0:64],
                        ps[:].rearrange("p (h e) -> p h e", h=HPC),
                        bv_bc[:].rearrange("p (h e) -> p h e", h=HPC),
                        ADD,
                    )

            # --- attention for one (head, q-half) -------------------------
            # Filler machinery: the engines execute in-order, so PE starvation
            # during exp-bound attention stretches can only be avoided by
            # interleaving independent PE work (projections, outproj) into
            # the kb loops at fine granularity, paced by the ACT/PE balance.
            filler_q = []   # list of [name, pe_cost_ns, fn, deadline|None]
            balance = [0.0]

            def pop_fillers(here=None):
                # deadline-forced pops (emit everything up to the overdue
                # entry to preserve dependency order), then balance pops
                if here is not None:
                    h, hf, kb = here
                    due = None
                    for i, ent in enumerate(filler_q):
                        dl = ent[3]
                        if dl is not None and dl[0] == h and dl[1] == hf \
                                and kb >= dl[2]:
                            due = i
                    if due is not None:
                        for ent in filler_q[: due + 1]:
                            ent[2]()
                            balance[0] -= ent[1]
                        del filler_q[: due + 1]
                while filler_q and balance[0] >= filler_q[0][1] * 0.4:
                    name, cost, fn, _ = filler_q.pop(0)
                    fn()
                    balance[0] -= cost

            def drain_until(name):
                while filler_q:
                    nm, cost, fn, _ = filler_q.pop(0)
                    fn()
                    balance[0] -= cost
                    if nm == name:
                        break

            def drain_all():
                while filler_q:
                    _, cost, fn, _ = filler_q.pop(0)
                    fn()
                    balance[0] -= cost

            def attn_head(h, hf, pending_tail=None, av_delay=3):
                qs = hf * 1024
                off = 64 * (h % 2)
                qt = qk_sb[off : off + 64, 2 * (h // 2), :]
                kt = qk_sb[off : off + 64, 2 * (h // 2) + 1, :]
                nkb = 8 if hf == 0 else 16
                ats = []

                def av_group(qb_l):
                    # AV, qb-major: at-tiles are the stationary operand, so
                    # each accumulation step emits only 65 output columns; the
                    # ones column of V lands the denominator per-partition.
                    qb_g = hf * 8 + qb_l
                    pv = psC.tile([128, 65], F32, tag="psC", name=f"pav{h}{hf}{qb_l}")
                    for kb2 in range(qb_g + 1):
                        nc.tensor.matmul(
                            pv[:], ats[kb2][:, ts(qb_l, 128)], v_sb[:, kb2, h, :],
                            start=(kb2 == 0), stop=(kb2 == qb_g),
                        )
                    if dump and h == 0:
                        dtmp = smallp.tile([128, 65], F32, tag="dtmp",
                                           name=f"dd{hf}{qb_l}")
                        nc.vector.tensor_copy(dtmp[:], pv[:])
                        nc.sync.dma_start(d_den[:, qb_g, :], dtmp[:])
                    recip = smallp.tile([128, 1], F32, tag="recip")
                    nc.vector.reciprocal(recip[:], pv[:, 64:65])
                    nc.vector.tensor_scalar(
                        vals_sb[:, qb_g, h, :], pv[:, 0:64],
                        recip[:, 0:1], None, MULT,
                    )

                for kb in range(nkb):
                    if kb >= 1:
                        pop_fillers(here=(h, hf, kb))
                    lo = max(0, kb * 128 - qs)
                    if lo < 512:
                        ps = psBig.tile([128, 1024], F32, tag="psB",
                                        name=f"sc{h}{hf}{kb}")
                        base = 0
                    else:
                        ps = psBig.tile([128, 512], F32, tag="psB",
                                        name=f"sc{h}{hf}{kb}")
                        base = 512
                    lhsT = kt[:, ts(kb, 128)]
                    for seg in range(2):
                        a = max(lo, seg * 512)
                        bnd = (seg + 1) * 512
                        if a < bnd:
                            nc.tensor.matmul(
                                ps[:, ds(a - base, bnd - a)], lhsT,
                                qt[:, ds(qs + a, bnd - a)],
                                start=True, stop=True,
                            )
                    at = atp.tile([128, 1024], BF16, tag="at")
                    nc.scalar.activation(
                        at[:, ds(lo, 1024 - lo)], ps[:, ds(lo - base, 1024 - lo)],
                        AF.Exp)
                    if kb * 128 >= qs:
                        dlo = kb * 128 - qs
                        nc.gpsimd.tensor_tensor(
                            at[:, ds(dlo, 128)], at[:, ds(dlo, 128)],
                            tri_t[:], MULT,
                        )
                    if dump and h == 0:
                        nc.sync.dma_start(
                            d_at[:, hf * 8 + kb if hf == 0 else kb,
                                 ds(lo, 1024 - lo)],
                            at[:, ds(lo, 1024 - lo)])
                    ats.append(at)
                    if kb == 1 and pending_tail is not None:
                        pending_tail()
                    # AV groups run a couple of kbs behind so the
                    # scores->exp->mask chain of the diagonal tile never
                    # stalls the PE queue
                    pe_rows = 1024 - lo
                    if kb - hf * 8 - av_delay >= 0:
                        av_group(kb - hf * 8 - av_delay)
                        pe_rows += 65 * (kb - av_delay + 1)
                    balance[0] += ((1024 - lo) * 0.833 + 185
                                   - pe_rows * 0.4167 - 150)

                def tail():
                    for qb_l in range(8 - av_delay, 8):
                        av_group(qb_l)
                return tail

            # --- vals transpose + output projection for one q-half --------
            def transposes_half(hf, qbs=range(8), hps=(0, 1)):
                qs = hf * 1024
                for qb_l in qbs:
                    qb_g = hf * 8 + qb_l
                    for hp in hps:
                        pst = psBig.tile(
                            [128, 128], BF16, tag="psB", name=f"tr{hf}{qb_l}{hp}")
                        nc.tensor.transpose(
                            pst[:], vals_sb[:, qb_g, ds(hp * 2, 2), :], ident[:])
                        nc.vector.tensor_copy(
                            valsT_sb[:, hp, ds(qs + qb_l * 128, 128)], pst[:])

            _ot_state = {}

            def outproj_ob(hf, ob):
                qs = hf * 1024
                obg, obi = divmod(ob, 4)
                if (hf, obg) not in _ot_state:
                    _ot_state[(hf, obg)] = outp.tile(
                        [128, 4, 1024], BF16, tag="out", name=f"ot{hf}{obg}")
                ot = _ot_state[(hf, obg)]
                ps = psBig.tile([128, 1024], F32, tag="psB", name=f"op{hf}{ob}")
                for kc in range(2):
                    lhsT = wo_sb[:, kc, ts(ob, 128)]
                    for nn in range(2):
                        nc.tensor.matmul(
                            ps[:, ts(nn, 512)], lhsT,
                            valsT_sb[:, kc, ds(qs + nn * 512, 512)],
                            start=(kc == 0), stop=(kc == 1),
                        )
                # Pool cannot read PSUM (walrus restriction): hf0 copies go
                # on DVE; hf1 copies (tail, ACT idle) alternate DVE/ACT
                if hf == 1 and obi % 2 == 0:
                    nc.scalar.activation(ot[:, obi, :], ps[:], AF.Copy)
                else:
                    nc.vector.tensor_copy(ot[:, obi, :], ps[:])
                if hf == 1:
                    # per-ob DMAs at the tail, alternating queues, so the
                    # last transfer is small and starts as late as possible
                    dma_eng = nc.sync if obi % 2 == 0 else nc.scalar
                    dma_eng.dma_start(
                        outT[ds(ob * 128, 128), ds(qs, 1024)],
                        ot[:, obi, :],
                    )
                elif obi % 2 == 1:
                    # keep hf0 out-DMAs off the ACT queue: mid-attention a
                    # DMA holds the sequencer ~2.3us and stalls exp issue
                    dma_eng = nc.sync if obg % 2 == 0 else nc.gpsimd
                    dma_eng.dma_start(
                        outT[ds(obg * 512 + (obi - 1) * 128, 256),
                             ds(qs, 1024)].rearrange(
                            "(ob p) q -> p ob q", p=128),
                        ot[:, ds(obi - 1, 2), :],
                    )

            def outproj_half(hf):
                for ob in range(8):
                    outproj_ob(hf, ob)

            def qk_unit(cc, sh, h, hf, kb):
                """A projection chunk as one deadline-pinned filler unit
                (the open psum accumulation group cannot be interleaved with
                other pool allocations, so the chunk stays monolithic)."""
                return [f"qk{cc}{sh}", 10 ** 6,
                        (lambda: qk_chunk(cc, sh)), (h, hf, kb)]

            def qk_unit2(cc, sh, h, hf, kb):
                """A projection chunk as TWO filler units at consecutive kb
                deadlines. Safe against psum-slot recycling because only one
                pool allocation (that kb's score tile) lands between them,
                which is below the pool depth."""
                state = {}

                def half(d0, nd=4):
                    def f():
                        if "ps" not in state:
                            state["ps"] = psBig.tile(
                                [128, 1024], F32, tag="psB",
                                name=f"qkps{cc}{sh}")
                        ps = state["ps"]
                        for dc in range(d0, d0 + nd):
                            lhsT = wqk_sb[:, dc, ts(cc, 128)]
                            for nn in range(2):
                                nc.tensor.matmul(
                                    ps[:, ts(nn, 512)], lhsT,
                                    xt_sb[:, dc, ds(sh * 1024 + nn * 512, 512)],
                                    start=(dc == 0), stop=(dc == 7),
                                )
                        if d0 + nd == 8:
                            nc.vector.tensor_scalar(
                                qk_sb[:, cc, ds(sh * 1024, 1024)], ps[:],
                                b_sb[:, cc : cc + 1], None, ADD,
                            )
                    return f

                nkb_h = 8 if hf == 0 else 16
                return [
                    [f"qk{cc}{sh}_{d0}", 10 ** 6, half(d0, 2),
                     (h, hf, min(kb + (d0 // 2) * 2, nkb_h - 1))]
                    for d0 in (0, 2, 4, 6)
                ]

            # --- emission order: everything that is not on the critical
            # dependency prefix is deadline-pinned into the attention kb
            # loops so the in-order PE queue never runs dry.
            BIGC = 10 ** 6
            qk_chunk(0, 0)
            qk_chunk(1, 0)
            v_blocks(range(8))
            filler_q.extend(qk_unit2(2, 0, 0, 0, 1))
            filler_q.extend(qk_unit2(3, 0, 1, 0, 1))
            t = attn_head(0, 0)
            filler_q.extend(qk_unit2(0, 1, 2, 0, 1))
            filler_q.extend(qk_unit2(1, 1, 3, 0, 1))
            t = attn_head(1, 0, t)
            t = attn_head(2, 0, t)
            t = attn_head(3, 0, t)
            drain_all()
            filler_q.extend(
                [
                    [f"V{sb_i}", BIGC,
                     (lambda sb_i=sb_i: v_blocks([sb_i])),
                     (0, 1, sb_i - 4)]
                    for sb_i in range(8, 16)
                ]
                + [
                    ["tr0a", BIGC, lambda: transposes_half(0, qbs=range(0, 3)),
                     (1, 1, 2)],
                    ["tr0b", BIGC, lambda: transposes_half(0, qbs=range(3, 6)),
                     (1, 1, 3)],
                    ["tr0c", BIGC, lambda: transposes_half(0, qbs=range(6, 8)),
                     (1, 1, 4)],
                ]
                + qk_unit2(2, 1, 1, 1, 7)
            )
            t = attn_head(0, 1, t)
            t = attn_head(1, 1, t)
            drain_all()
            filler_q.append(qk_unit(3, 1, 2, 1, 6))
            filler_q.extend([
                # outproj(0) is the only arbitrarily-deferrable PE work:
                # spread it across the last two heads
                [f"op0{ob}", BIGC, (lambda ob=ob: outproj_ob(0, ob)),
                 (2, 1, 9 + 2 * ob) if ob < 4 else (3, 1, 2 * ob - 4)]
                for ob in range(4)
            ])
            t = attn_head(2, 1, t)
            filler_q.extend([
                ["tr1a", BIGC,
                 lambda: transposes_half(1, qbs=range(0, 4), hps=(0,)),
                 (3, 1, 2)],
                ["tr1b", BIGC,
                 lambda: transposes_half(1, qbs=range(4, 8), hps=(0,)),
                 (3, 1, 3)],

            ] + [
                [f"op0{ob}", BIGC, (lambda ob=ob: outproj_ob(0, ob)),
                 (3, 1, 2 * ob - 4)]
                for ob in range(4, 8)
            ])
            t = attn_head(3, 1, t, av_delay=2)
            t()
            drain_all()
            transposes_half(1, hps=(1,))
            outproj_half(1)
            if dump:
                nc.sync.dma_start(d_qk[:], qk_sb[:])
                nc.sync.dma_start(d_v[:], v_sb[:])
                nc.sync.dma_start(d_vals[:], vals_sb[:])
                nc.sync.dma_start(d_valsT[:], valsT_sb[:])

    nc.compile()
    return nc


def _prep_inputs_causal(x, mask, W_qkv, b_qkv, W_o, b_o):
    scale = np.float32(1.0 / np.sqrt(HD))
    xT = [np.ascontiguousarray(x[b].T).astype(NPBF16) for b in range(B)]
    in_maps = []
    for c in range(NCORES):
        b, g = divmod(c, HPC)
        heads = [HPC * g + i for i in range(HPC)]
        qc = [W_qkv[:, 192 * h : 192 * h + 64] * scale for h in heads]
        kc = [W_qkv[:, 192 * h + 64 : 192 * h + 128] for h in heads]
        # chunk order: q01 | k01 | q23 | k23; each chunk prepacked so the
        # per-partition row (dc, c) is contiguous in DRAM
        chunks = [
            np.concatenate([qc[0], qc[1]], axis=1),
            np.concatenate([kc[0], kc[1]], axis=1),
            np.concatenate([qc[2], qc[3]], axis=1),
            np.concatenate([kc[2], kc[3]], axis=1),
        ]
        wqk = np.stack([
            c.reshape(8, 128, 128).transpose(1, 0, 2).reshape(128, 1024)
            for c in chunks
        ])
        bq = [b_qkv[192 * h : 192 * h + 64] * scale for h in heads]
        bk = [b_qkv[192 * h + 64 : 192 * h + 128] for h in heads]
        bqk_t = np.concatenate(
            [bq[0], bq[1], bk[0], bk[1], bq[2], bq[3], bk[2], bk[3]]
        ).reshape(4, 128).T
        wv = np.concatenate(
            [W_qkv[:, 192 * h + 128 : 192 * h + 192] for h in heads], axis=1)
        bv = np.concatenate(
            [b_qkv[192 * h + 128 : 192 * h + 192] for h in heads])[None, :]
        wo = W_o[256 * g : 256 * (g + 1), :]
        in_maps.append({
            "xT": xT[b],
            "wqk": np.ascontiguousarray(wqk).astype(NPBF16),
            "bqk": np.ascontiguousarray(bqk_t).astype(np.float32),
            "wv": np.ascontiguousarray(wv).astype(NPBF16),
            "bv": np.ascontiguousarray(bv).astype(NPBF16),
            "wo": np.ascontiguousarray(wo).astype(NPBF16),
        })
    return in_maps


# ---------------------------------------------------------------------------
# Generic additive-mask fallback (previous kernel)
# ---------------------------------------------------------------------------

def _classify_mask(maskT):
    """pattern[hf][kb][nn] for [128, 512] tiles of mask^T (k major, q minor)."""
    pat = []
    for hf in range(2):
        rows = []
        for kb in range(16):
            ents = []
            for nn in range(2):
                blk = maskT[kb * 128 : (kb + 1) * 128,
                            hf * 1024 + nn * 512 : hf * 1024 + (nn + 1) * 512]
                if np.all(blk <= -1e8):
                    ents.append(SKIP)
                elif np.all(blk == 0.0):
                    ents.append(FULL)
                else:
                    ents.append(PART)
            rows.append(tuple(ents))
        pat.append(tuple(rows))
    return tuple(tuple(r) for r in pat)


def _build_generic(pattern):
    nc = bacc.Bacc(None, target_bir_lowering=False, debug=False)
    xT = nc.dram_tensor("xT", [D, S], F32R, kind="ExternalInput")
    wqk = nc.dram_tensor("wqk", [D, 512], F32R, kind="ExternalInput")
    bqk = nc.dram_tensor("bqk", [128, 4], F32R, kind="ExternalInput")
    wv = nc.dram_tensor("wv", [D, 256], F32R, kind="ExternalInput")
    bv = nc.dram_tensor("bv", [1, 256], F32R, kind="ExternalInput")
    wo = nc.dram_tensor("wo", [256, D], F32R, kind="ExternalInput")
    maskT = nc.dram_tensor("maskT", [S, S], F32R, kind="ExternalInput")
    outT = nc.dram_tensor("outT", [D, S], F32, kind="ExternalOutput")

    with tile.TileContext(nc) as tc:
        with (
            tc.tile_pool(name="big", bufs=1) as bigp,
            tc.tile_pool(name="wqkp", bufs=1) as wqkp,
            tc.tile_pool(name="wvp", bufs=1) as wvp,
            tc.tile_pool(name="wop", bufs=1) as wop,
            tc.tile_pool(name="qkp", bufs=1) as qkp,
            tc.tile_pool(name="vp", bufs=1) as vp,
            tc.tile_pool(name="valsp", bufs=1) as valsp,
            tc.tile_pool(name="attnp", bufs=3) as attnp,
            tc.tile_pool(name="maskp", bufs=1) as maskp,
            tc.tile_pool(name="smallp", bufs=1) as smallp,
            tc.tile_pool(name="constp", bufs=1) as constp,
        ):
            ones_t = constp.tile([1, 128], F32R)
            nc.gpsimd.memset(ones_t[:].bitcast(F32), 1.0)
            b_sb = constp.tile([128, 4], F32R)
            nc.sync.dma_start(b_sb[:], bqk[:])
            bv_sb = constp.tile([1, 256], F32R)
            nc.sync.dma_start(bv_sb[:], bv[:])

            wqk_sb = wqkp.tile([128, 8, 512], F32R, tag="wqkslot")
            xt_sb = bigp.tile([128, 8, S], F32R, tag="big")
            wv_sb = wvp.tile([128, 8, 256], F32R)
            for dc in range(8):
                nc.sync.dma_start(wqk_sb[:, dc, :], wqk[ds(dc * 128, 128), :])
                nc.sync.dma_start(
                    xt_sb[:, dc, ds(0, 1024)], xT[ds(dc * 128, 128), ds(0, 1024)]
                )
            nc.sync.dma_start(wv_sb[:], wv[:].rearrange("(dc p) c -> p dc c", p=128))

            qk_sb = qkp.tile([128, 4, S], F32R)
            v_sb = vp.tile([128, 16, HPC, 65], F32R)

            nc.gpsimd.memset(v_sb[:, :, :, 64:65].bitcast(F32), 1.0)

            def n_full(hf):
                return sum(c == FULL for kb in pattern[hf] for c in kb)

            hf_order = sorted(range(2), key=lambda hf: n_full(hf))
            part_blocks, slots, mask_tiles = {}, {}, {}
            for hf_i, hf in enumerate(hf_order):
                pb = [
                    (kb, nn)
                    for kb in range(16) for nn in range(2)
                    if pattern[hf][kb][nn] == PART
                ]
                part_blocks[hf] = pb
                slots[hf] = {blk: i for i, blk in enumerate(pb)}
                if len(pb) == 0:
                    mask_tiles[hf] = None
                elif len(pb) <= 8:
                    if hf_i == 0:
                        mask_tiles[hf] = maskp.tile(
                            [128, 8, 512], F32R, tag="mask", name=f"mask{hf}")
                    else:
                        mask_tiles[hf] = wqkp.tile(
                            [128, 8, 512], F32R, tag="wqkslot", name=f"mask{hf}")
                else:
                    mask_tiles[hf] = bigp.tile(
                        [128, 32, 512], F32R, tag="big", name=f"mask{hf}")

            def mask_dmas(hf, qs_):
                pb = part_blocks[hf]
                i = 0
                while i < len(pb):
                    kb0, nn0 = pb[i]
                    j = i + 1
                    while (j < len(pb)
                           and pb[j] == (pb[j - 1][0] + 1, nn0)):
                        j += 1
                    n = j - i
                    nc.sync.dma_start(
                        mask_tiles[hf][:, i : i + n, :],
                        maskT[ds(kb0 * 128, n * 128),
                              ds(qs_ + nn0 * 512, 512)].rearrange(
                            "(b p) q -> p b q", p=128),
                    )
                    i = j
            hf_first = hf_order[0]
            if mask_tiles.get(hf_first) is not None and len(part_blocks[hf_first]) <= 8:
                mask_dmas(hf_first, hf_first * 1024)
            for dc in range(8):
                nc.sync.dma_start(
                    xt_sb[:, dc, ds(1024, 1024)],
                    xT[ds(dc * 128, 128), ds(1024, 1024)],
                )
            wo_sb = wop.tile([128, 2, D], F32R)
            nc.sync.dma_start(wo_sb[:], wo[:].rearrange("(kc p) d -> p kc d", p=128))

            def qk_chunk_half(pool, cc, sh):
                ps = pool.tile([128, 1024], F32, tag="psqk", name=f"qkps{cc}{sh}")
                for dc in range(8):
                    lhsT = wqk_sb[:, dc, ts(cc, 128)]
                    for nn in range(2):
                        nc.tensor.matmul(
                            ps[:, ts(nn, 512)], lhsT,
                            xt_sb[:, dc, ds(sh * 1024 + nn * 512, 512)],
                            start=(dc == 0), stop=(dc == 7),
                        )
                nc.scalar.activation(
                    qk_sb[:, cc, ds(sh * 1024, 1024)], ps[:], AF.Identity,
                    bias=b_sb[:, cc : cc + 1],
                )

            def v_blocks(pool, rng):
                for sb_i in rng:
                    ps = pool.tile([128, 256], F32, tag="psv", name=f"vps{sb_i}")
                    for dc in range(8):
                        nc.tensor.matmul(
                            ps[:], xt_sb[:, dc, ts(sb_i, 128)], wv_sb[:, dc, :],
                            start=(dc == 0), stop=False,
                        )
                    nc.tensor.matmul(ps[:], ones_t[:], bv_sb[:], start=False, stop=True)
                    nc.vector.tensor_copy(
                        v_sb[:, sb_i, :, 0:64],
                        ps[:].rearrange("p (h e) -> p h e", h=HPC),
                    )

            with (
                tc.tile_pool(name="psqk", bufs=2, space="PSUM") as psqk,
                tc.tile_pool(name="psv", bufs=2, space="PSUM") as psv,
            ):
                for cc in (0, 2, 1, 3):
                    qk_chunk_half(psqk, cc, 0)
                v_blocks(psv, range(8))
                for cc in (0, 2, 1, 3):
                    qk_chunk_half(psqk, cc, 1)
                v_blocks(psv, range(8, 16))

            with (
                tc.tile_pool(name="pssc", bufs=3, space="PSUM") as pssc,
                tc.tile_pool(name="psav", bufs=1, space="PSUM") as psav,
            ):
                def outproj(vals_t, qs_):
                    for ob in range(8):
                        ps = pssc.tile([128, 1024], F32, tag="pssc")
                        for kc in range(2):
                            lhsT = wo_sb[:, kc, ts(ob, 128)]
                            for nn in range(2):
                                nc.tensor.matmul(
                                    ps[:, ts(nn, 512)], lhsT,
                                    vals_t[:, kc, ts(nn, 512)],
                                    start=(kc == 0), stop=(kc == 1),
                                )
                        oev = attnp.tile([128, 1024], F32, tag="attn")
                        nc.scalar.activation(oev[:], ps[:], AF.Copy)
                        nc.sync.dma_start(
                            outT[ds(ob * 128, 128), ds(qs_, 1024)], oev[:]
                        )

                pending = None
                for hf_i, hf in enumerate(hf_order):
                    qs = hf * 1024
                    slot = slots[hf]
                    mask_sb = mask_tiles[hf]
                    if mask_sb is not None and (
                        hf_i > 0 or len(part_blocks[hf]) > 8
                    ):
                        mask_dmas(hf, qs)
                    vals_sb = valsp.tile(
                        [128, 2, 1024], F32R, tag="vals", name=f"vals{hf}")
                    kb_order = sorted(
                        (kb for kb in range(16) if pattern[hf][kb] != (SKIP, SKIP)),
                        key=lambda kb: (PART in pattern[hf][kb], kb),
                    )
                    for h in range(HPC):
                        off = 64 * (h % 2)
                        qt = qk_sb[off : off + 64, h // 2, :]
                        kt = qk_sb[off : off + 64, 2 + h // 2, :]
                        act_kbs = [
                            [kb for kb in kb_order if pattern[hf][kb][nn] != SKIP]
                            for nn in range(2)
                        ]
                        ps_av = psav.tile([65, 1024], F32, tag="psav")
                        for kb in kb_order:
                            cls = pattern[hf][kb]
                            ps_sc = pssc.tile([128, 1024], F32, tag="pssc")
                            lhsT = kt[:, ts(kb, 128)]
                            at = attnp.tile([128, 1024], F32R, tag="attn")
                            for nn in range(2):
                                if cls[nn] == SKIP:
                                    continue
                                nc.tensor.matmul(
                                    ps_sc[:, ts(nn, 512)], lhsT,
                                    qt[:, ds(qs + nn * 512, 512)],
                                    start=True, stop=True,
                                )
                            if cls == (FULL, FULL):
                                nc.scalar.activation(at[:], ps_sc[:], AF.Exp)
                            else:
                                for nn in range(2):
                                    if cls[nn] == SKIP:
                                        continue
                                    if cls[nn] == PART:
                                        nc.vector.tensor_tensor(
                                            at[:, ts(nn, 512)], ps_sc[:, ts(nn, 512)],
                                            mask_sb[:, slot[(kb, nn)], :], ADD,
                                        )
                                        nc.scalar.activation(
                                            at[:, ts(nn, 512)], at[:, ts(nn, 512)],
                                            AF.Exp,
                                        )
                                    else:
                                        nc.scalar.activation(
                                            at[:, ts(nn, 512)], ps_sc[:, ts(nn, 512)],
                                            AF.Exp,
                                        )
                            for nn in range(2):
                                if cls[nn] == SKIP:
                                    continue
                                nc.tensor.matmul(
                                    ps_av[:, ts(nn, 512)], v_sb[:, kb, h, :],
                                    at[:, ts(nn, 512)],
                                    start=(kb == act_kbs[nn][0]),
                                    stop=(kb == act_kbs[nn][-1]),
                                )
                        recip = smallp.tile([1, 1024], F32R, tag="recip")
                        with nc.allow_low_precision(
                            reason="float32r has fp32 bits; only PE matmul mode differs"
                        ):
                            nc.vector.reciprocal(recip[:], ps_av[64:65, :])
                        bc_sb = smallp.tile([64, 1024], F32R, tag="bc")
                        nc.gpsimd.partition_broadcast(bc_sb[:], recip[:])
                        nc.vector.tensor_tensor(
                            vals_sb[off : off + 64, h // 2, :],
                            ps_av[0:64, :], bc_sb[:], MULT,
                        )
                        if pending is not None and hf_i == 1 and h == 0:
                            outproj(*pending)
                            pending = None

                    if hf_i == 0 and len(hf_order) > 1:
                        pending = (vals_sb, qs)
                    else:
                        outproj(vals_sb, qs)
                if pending is not None:
                    outproj(*pending)
                    pending = None

    nc.compile()
    return nc


def _prep_inputs_generic(x, mask, W_qkv, b_qkv, W_o, b_o):
    scale = np.float32(1.0 / np.sqrt(HD))
    xT = [np.ascontiguousarray(x[b].T) for b in range(B)]
    maskT = np.ascontiguousarray(mask.T)
    in_maps = []
    for c in range(NCORES):
        b, g = divmod(c, HPC)
        heads = [HPC * g + i for i in range(HPC)]
        qcols = np.concatenate(
            [W_qkv[:, 192 * h : 192 * h + 64] for h in heads], axis=1) * scale
        kcols = np.concatenate(
            [W_qkv[:, 192 * h + 64 : 192 * h + 128] for h in heads], axis=1)
        wqk = np.ascontiguousarray(np.concatenate([qcols, kcols], axis=1))
        bq = np.concatenate([b_qkv[192 * h : 192 * h + 64] for h in heads]) * scale
        bk = np.concatenate([b_qkv[192 * h + 64 : 192 * h + 128] for h in heads])
        bqk_t = np.ascontiguousarray(
            np.concatenate([bq, bk]).reshape(4, 128).T)
        wv = np.ascontiguousarray(np.concatenate(
            [W_qkv[:, 192 * h + 128 : 192 * h + 192] for h in heads], axis=1))
        bv = np.ascontiguousarray(np.concatenate(
            [b_qkv[192 * h + 128 : 192 * h + 192] for h in heads])[None, :])
        wo = np.ascontiguousarray(W_o[256 * g : 256 * (g + 1), :])
        in_maps.append({
            "xT": xT[b], "wqk": wqk, "bqk": bqk_t, "wv": wv, "bv": bv,
            "wo": wo, "maskT": maskT,
        })
    return in_maps


# ---------------------------------------------------------------------------
# Entry point
# ---------------------------------------------------------------------------

def _prep_inputs(x, mask, W_qkv, b_qkv, W_o, b_o):
    if _is_causal(mask):
        return _prep_inputs_causal(x, mask, W_qkv, b_qkv, W_o, b_o)
    return _prep_inputs_generic(x, mask, W_qkv, b_qkv, W_o, b_o)


def kernel(x, mask, W_qkv, b_qkv, W_o, b_o):
    x = np.asarray(x, dtype=np.float32)
    mask = np.asarray(mask, dtype=np.float32)
    W_qkv = np.asarray(W_qkv, dtype=np.float32)
    b_qkv = np.asarray(b_qkv, dtype=np.float32)
    W_o = np.asarray(W_o, dtype=np.float32)
    b_o = np.asarray(b_o, dtype=np.float32)

    if _is_causal(mask):
        key = "causal"
        if key not in _CACHE:
            _CACHE[key] = _build_causal()
        nc = _CACHE[key]
        in_maps = _prep_inputs_causal(x, mask, W_qkv, b_qkv, W_o, b_o)
    else:
        pattern = _classify_mask(np.ascontiguousarray(mask.T))
        key = ("nc", pattern)
        if key not in _CACHE:
            _CACHE[key] = _build_generic(pattern)
        nc = _CACHE[key]
        in_maps = _prep_inputs_generic(x, mask, W_qkv, b_qkv, W_o, b_o)
    _CACHE["nc"] = nc

    res = run_bass_kernel_spmd(nc, in_maps, core_ids=list(range(NCORES)))
    _CACHE["last_result"] = res

    out = np.empty((B, S, D), dtype=np.float32)
    for b in range(B):
        acc = res.results[HPC * b]["outT"].astype(np.float32)
        for g in range(1, HPC):
            acc = acc + res.results[HPC * b + g]["outT"].astype(np.float32)
        out[b] = acc.T + b_o
    return out
